# revision 1
# baseline (speedup 1.0000x reference)
"""BilateralFilter (SqueezeSeg mc condensing-kernel gaussians) on 8 TRN2 cores.

Reference computes, for x: [16, 64, 512, 3] (B, Z, A, C=xyz):
    nbr   = 14 spatial neighbors of each pixel in a 3x5 window (zero-padded)
    diff2 = sum_c (x - nbr)^2                           [B, Z, A, 14]
    out   = exp(-diff2 / (2 * theta_r^2))               [B, Z, A, 14, 4]
with THETA_R = [0.015, 0.015, 0.01, 0.01] (only 2 distinct values).

Strategy (pure batch data-parallel, 2 batches per core):
  - partitions p = b*64 + z  (128), free dim = azimuth chunks (AC wide).
  - squared differences via a runtime-registered fused custom DVE op
    (out = (in0-in1)^2), channel sums via tensor_reduce.
  - mirror symmetry: m_k(q) = |x(q) - x(q+off_k)|^2 for the 7 "negative"
    offsets k=0..6 gives the other 7 via diff2_{13-k}(q) = m_k(q - off_k);
    the z+1-partition read (engines cannot shift partitions by 1) is
    materialized on the idle TensorE as an exact 0/1 permutation matmul
    into PSUM, with the phantom z=64 boundary row (out-of-image neighbor
    => diff2 = |x(center)|^2, from s = sum_c x^2) accumulated by a second
    selector matmul. (PE_SHIFT=False falls back to partition-remap DMAs.)
  - ACT computes exp with the free scale immediate; each exp is written to
    both classes of its theta pair via a stride-0 input axis, directly into
    the interleaved [a, k, c] staging layout.
  - the staging tile matches DRAM layout exactly, so the store is one
    contiguous 128-partition DMA (28 KB/partition runs at AC=128).
"""

import numpy as np

import concourse.bass as bass
import concourse.tile as tile
from concourse import bacc, mybir
from concourse.bass_utils import run_bass_kernel_spmd

N_CORES = 8
B, Z, A, C = 16, 64, 512, 3
K, NCLS = 14, 4
LB = B // N_CORES            # local batches per core = 2
P = LB * Z                   # 128 partitions
AC = 128                     # azimuth chunk
BUFS = 3                     # tile pool buffers
PE_SHIFT = True              # z+1 partition shift via PE matmul vs SBUF DMA
XDN_PE = True                # derive x_dn on PE too (no duplicate DRAM read)
F32 = mybir.dt.float32
BF16 = mybir.dt.bfloat16


def _host_shift_mats():
    """SH2[k, m] = 1 iff k == m+1 (and not m == 63: batch boundary);
    SEL[k, m] = 1 iff k == m in {63, 127} (phantom z=64 row selector);
    SHD[k, m] = 1 iff k == m-1 (and not m in {0, 64}: z=0 rows stay 0)."""
    sh = np.zeros((P, P), np.float32)
    for m in range(P - 1):
        if m != Z - 1:
            sh[m + 1, m] = 1.0
    sel = np.zeros((P, P), np.float32)
    sel[Z - 1, Z - 1] = 1.0
    sel[P - 1, P - 1] = 1.0
    shd = np.zeros((P, P), np.float32)
    for m in range(1, P):
        if m != Z:
            shd[m - 1, m] = 1.0
    return sh, sel, shd

# exp scales: -1 / (2 * theta^2), theta pairs (0.015, 0.01), f32 semantics
_t0 = np.float32(0.015)
_t1 = np.float32(0.01)
SC0 = -float(1.0 / np.float32(np.float32(2.0) * _t0 * _t0))
SC1 = -float(1.0 / np.float32(np.float32(2.0) * _t1 * _t1))

# DRAM strides (elements) of out [LB, Z, A, K, NCLS]
O_A = K * NCLS               # 56
O_Z = A * O_A                # 28672
O_B = Z * O_Z                # 1835008
X_Z = A * C                  # 1536
X_B = Z * X_Z


def _ap(t, poff, pcnt, foff, pairs, pstep=1):
    """AP on tile t: partitions [poff, poff+pcnt) (stride pstep rows), free
    `pairs` ([step, count] in elements) based at element foff."""
    row = t.ap[0][0]
    return bass.AP(tensor=t.tensor, offset=t.offset + poff * row + foff,
                   ap=[[pstep * row, pcnt]] + [list(p) for p in pairs])


_SQDIFF = None


def _get_sqdiff():
    """Register a runtime custom DVE op: out = (in0 - in1)^2 (fp32, one
    instruction instead of subtract + multiply)."""
    global _SQDIFF
    if _SQDIFF is not None:
        return _SQDIFF
    from concourse import dve_ops
    from concourse.dve_spec import Spec, Src0, Src1, sq, lower, _has_src1
    from concourse.dve_uop import DveOpSpec

    name = "SQDIFF_BILAT_ANT"
    if name not in dve_ops._SUB_OPCODE_FOR_NAME:
        spec = Spec(
            body=sq(Src0 - Src1),
            reference=lambda in0, in1, c0, c1, c2:
                (in0.astype(np.float32) - in1.astype(np.float32)) ** 2)
        row = 1 + len(dve_ops.OPS)
        assert row < 0x20
        shas = {}
        for ver in ("v3",):
            tmp = DveOpSpec(name=name, opcode=row, uops=lower(spec, ver=ver),
                            rd1_en=_has_src1(spec))
            shas[ver] = tmp.sha(ver)
        op = dve_ops.DveOp(name, spec, subdim=False, uops_sha=shas)
        dve_ops.OPS.append(op)
        dve_ops.CUSTOM_DVE_SPECS[name] = spec
        dve_ops._SUB_OPCODE_FOR_NAME[name] = row
    else:
        op = next(o for o in dve_ops.OPS if o.name == name)
    _SQDIFF = op
    return op


def _build(ac=AC, bufs=BUFS, reps=1, pe_shift=PE_SHIFT, xdn_pe=XDN_PE,
           chunks=None, psum_bufs=3, store_rings=3, dt_mode="planes",
           amir_dve=0, sq_dve=False, ring_wts=None):
    # chunk schedule: list of (a0, width).  Uniform chunks minimize the
    # per-chunk fixed instruction overhead (~185 ns per ACT instruction,
    # 6 of them per chunk); with deep load prefetch the pipeline fill no
    # longer needs smaller leading chunks, and fill amortizes over reps.
    if chunks is None:
        chunks = [(a0, ac) for a0 in range(0, A, ac)]
    assert sum(w for _, w in chunks) == A
    NCH = len(chunks)
    nc = bacc.Bacc("TRN2", target_bir_lowering=False, debug=False,
                   num_devices=N_CORES)
    x_h = nc.dram_tensor("x", [LB, Z, A, C], F32, kind="ExternalInput")
    o_h = nc.dram_tensor("out", [LB, Z, A, K, NCLS], F32, kind="ExternalOutput")
    x_ap, o_ap = x_h.ap(), o_h.ap()
    if pe_shift:
        # 0/1 shift matrices are exact in bf16 (2x PE when M is bf16)
        CDT = F32 if dt_mode == "reduce" else BF16
        shm_h = nc.dram_tensor("shm", [P, P], CDT, kind="ExternalInput")
        sel_h = nc.dram_tensor("sel", [P, P], CDT, kind="ExternalInput")
        if xdn_pe:
            shd_h = nc.dram_tensor("shd", [P, P], F32, kind="ExternalInput")
    # bench mode: reps > 1 re-runs the whole kernel; each non-final pass
    # stores to its own DRAM scratch so stores are real traffic
    scratch_aps = [
        nc.dram_tensor(f"scr{r}", [LB, Z, A, K, NCLS], F32).ap()
        for r in range(reps - 1)]

    from contextlib import ExitStack
    with tile.TileContext(nc) as tc, ExitStack() as es:
        if pe_shift:
            consts = es.enter_context(tc.tile_pool(name="consts", bufs=1))
            psum = es.enter_context(
                tc.tile_pool(name="psum", bufs=psum_bufs, space="PSUM"))
        with tc.tile_pool(name="pool", bufs=bufs) as pool:
            if pe_shift:
                sh_t = consts.tile([P, P], CDT, name="sh_t")
                nc.sync.dma_start(sh_t[:], shm_h.ap()[:])
                sel_t = consts.tile([P, P], CDT, name="sel_t")
                nc.sync.dma_start(sel_t[:], sel_h.ap()[:])
                if xdn_pe:
                    shd_t = consts.tile([P, P], F32, name="shd_t")
                    nc.sync.dma_start(shd_t[:], shd_h.ap()[:])
            N = NCH * reps

            def _geom(ci):
                a0, ac = chunks[ci % NCH]
                XW = ac + 8          # x window (halo 4 each side)
                lo, hi = max(0, a0 - 4), min(A, a0 + ac + 4)
                c_lo = (lo - (a0 - 4)) * C          # first valid xt col
                c_hi = (hi - (a0 - 4)) * C
                return a0, ac, XW, lo, hi, c_lo, c_hi

            def emit_load(ci):
                # ---- load x window (zero halo at image borders) ----
                # (b, z) rows are contiguous in DRAM: one 128-partition DMA.
                # Loads issue on the (otherwise idle) gpsimd SWDGE so they
                # are not program-ordered behind the big store issues on SP
                # — the next chunks' loads must cut ahead of queued stores
                # or compute stalls behind them.
                # deep rotation: loads must be queued well before the big
                # stores they contend with, or they wait out a full 10 us
                # store before landing (xt is tiny: 1.6 KB/partition/buf)
                _, _, XW, lo, hi, c_lo, c_hi = _geom(ci)
                xt = pool.tile([P, XW * C], F32, name="xt", bufs=8)
                if c_lo > 0:
                    nc.gpsimd.memset(_ap(xt, 0, P, 0, [[1, c_lo]]), 0.0)
                if c_hi < XW * C:
                    nc.gpsimd.memset(
                        _ap(xt, 0, P, c_hi, [[1, XW * C - c_hi]]), 0.0)
                nc.gpsimd.dma_start(
                    _ap(xt, 0, P, c_lo, [[C, hi - lo], [1, C]]),
                    bass.AP(tensor=x_ap.tensor, offset=lo * C,
                            ap=[[X_Z, P], [C, hi - lo], [1, C]]))
                return xt

            def emit_xdn(ci, xt):
                # ---- x_dn[p] = x at (z-1) (zeros at z=0 rows): exact PE
                # permutation shift of xt into PSUM; the zero columns of SHD
                # give the z=0 rows (and the xt halo the image-border zeros)
                # for free.  Emitted one chunk AHEAD of the consuming chunk:
                # the PE is in-order, so x_dn(i+1) must precede M_up(i) or
                # the serial loop DVE(i) -> M_up(i) -> x_dn(i+1) -> DVE(i+1)
                # paces the pipeline above the store rate.  bufs=2 so the
                # psum pool fits 8 banks (M_up 3x2 + x_dn 2x1).
                _, _, XW, _, _, _, _ = _geom(ci)
                x_dn = psum.tile([P, XW * C], F32, name="x_dn_ps", bufs=2)
                for n0 in range(0, XW * C, 512):
                    n1 = min(XW * C, n0 + 512)
                    nc.tensor.matmul(
                        _ap(x_dn, 0, P, n0, [[1, n1 - n0]]),
                        shd_t[:], _ap(xt, 0, P, n0, [[1, n1 - n0]]),
                        start=True, stop=True)
                return x_dn

            PF = 7               # load prefetch distance (chunks ahead)
            xts, xdns = {}, {}
            for j in range(min(PF, N)):
                xts[j] = emit_load(j)
            if pe_shift and xdn_pe:
                xdns[0] = emit_xdn(0, xts[0])

            for ci in range(N):
                a0, ac, XW, lo, hi, c_lo, c_hi = _geom(ci)
                MW = ac + 4          # m window (halo 2 each side)

                if ci + PF < N:
                    xts[ci + PF] = emit_load(ci + PF)
                if pe_shift and xdn_pe and ci + 1 < N:
                    xdns[ci + 1] = emit_xdn(ci + 1, xts[ci + 1])
                xt = xts.pop(ci)

                if pe_shift and xdn_pe:
                    x_dn = xdns.pop(ci)
                else:
                    x_dn = pool.tile([P, XW * C], F32, name="x_dn")
                    nc.gpsimd.memset(x_dn[:], 0.0)
                    for b in range(LB):
                        nc.gpsimd.dma_start(
                            _ap(x_dn, b * Z + 1, Z - 1, c_lo,
                                [[C, hi - lo], [1, C]]),
                            bass.AP(tensor=x_ap.tensor, offset=b * X_B + lo * C,
                                    ap=[[X_Z, Z - 1], [C, hi - lo], [1, C]]))

                # ---- s = sum_c x^2 ; m_k maps over a-window [a0-2, ...)
                # k=0..4: dz=-1, da=k-2 ; k=5,6: dz=0, da=k-7
                # d2 = (x - x_nbr)^2 in one fused custom op per k.
                # dt_mode picks how the c-sum is done:
                #  "reduce": f32 interleaved + TensorReduce (no fast mode)
                #  "iadds":  bf16 interleaved (packed writes) + 2 stride-3
                #            tensor_adds — each add processes N/3 elements,
                #            beating the reduce's full-N stream
                #  "planes": custom writes c-outer packed bf16 planes; the
                #            adds are fully packed and hit the DVE 2x mode
                # bf16 rounds only d^2 / s / M (<=0.4% rel => ~2e-3 max abs
                # on the exp output, far inside the 2e-2 tolerance).
                sqdiff = _get_sqdiff()
                MDT = F32 if dt_mode == "reduce" else BF16
                M = pool.tile([P, 7 * MW], MDT, name="M")
                if dt_mode == "reduce":
                    sqx = pool.tile([P, XW * C], F32, name="sqx")
                    nc.scalar.square(sqx[:], xt[:])
                    st = pool.tile([P, XW], F32, name="st")
                    nc.vector.tensor_reduce(
                        st[:], _ap(sqx, 0, P, 0, [[C, XW], [1, C]]),
                        axis=mybir.AxisListType.X, op=mybir.AluOpType.add)
                    dt5 = pool.tile([P, 5 * MW * C], F32, name="dt5")
                    for k in range(5):
                        nc.vector._custom_dve(
                            sqdiff,
                            out=_ap(dt5, 0, P, k * MW * C, [[C, MW], [1, C]]),
                            in0=_ap(xt, 0, P, 2 * C, [[C, MW], [1, C]]),
                            in1=_ap(x_dn, 0, P, k * C, [[C, MW], [1, C]]))
                    nc.vector.tensor_reduce(
                        _ap(M, 0, P, 0, [[1, 5 * MW]]),
                        _ap(dt5, 0, P, 0, [[C, 5 * MW], [1, C]]),
                        axis=mybir.AxisListType.X, op=mybir.AluOpType.add)
                    dt2 = pool.tile([P, 2 * MW * C], F32, name="dt2")
                    for k in (5, 6):
                        nc.vector._custom_dve(
                            sqdiff,
                            out=_ap(dt2, 0, P, (k - 5) * MW * C,
                                    [[C, MW], [1, C]]),
                            in0=_ap(xt, 0, P, 2 * C, [[C, MW], [1, C]]),
                            in1=_ap(xt, 0, P, (k - 5) * C, [[C, MW], [1, C]]))
                    nc.vector.tensor_reduce(
                        _ap(M, 0, P, 5 * MW, [[1, 2 * MW]]),
                        _ap(dt2, 0, P, 0, [[C, 2 * MW], [1, C]]),
                        axis=mybir.AxisListType.X, op=mybir.AluOpType.add)
                else:
                    # custom-dve APs are rank<=3: one call per map k
                    dt = pool.tile([P, 3 * 7 * MW], BF16, name="dt")
                    if dt_mode == "planes":
                        # c-outer stream: strided f32 reads, PACKED bf16
                        # plane writes (scattered 2-byte writes would RMW)
                        d_out = lambda k: _ap(dt, 0, P, k * MW,
                                              [[7 * MW, C], [1, MW]])
                        d_in = lambda t, off: _ap(t, 0, P, off,
                                                  [[1, C], [C, MW]])
                        add_ap = lambda c: _ap(dt, 0, P, c * 7 * MW,
                                               [[1, 7 * MW]])
                    else:  # iadds: natural interleaved stream, packed writes
                        d_out = lambda k: _ap(dt, 0, P, 3 * k * MW,
                                              [[C, MW], [1, C]])
                        d_in = lambda t, off: _ap(t, 0, P, off,
                                                  [[C, MW], [1, C]])
                        add_ap = lambda c: _ap(dt, 0, P, c,
                                               [[C, 7 * MW]])
                    for k in range(7):
                        src, off = (x_dn, k * C) if k < 5 else (xt, (k - 5) * C)
                        nc.vector._custom_dve(
                            sqdiff, out=d_out(k),
                            in0=d_in(xt, 2 * C), in1=d_in(src, off))
                    dts = pool.tile([P, 7 * MW], BF16, name="dts")
                    nc.vector.tensor_add(dts[:], add_ap(0), add_ap(1))
                    nc.vector.tensor_add(M[:], dts[:], add_ap(2))

                    # s = sum_c x^2 via the same layout trick
                    sqx = pool.tile([P, 3 * XW], BF16, name="sqx")
                    if dt_mode == "planes":
                        sq_out = _ap(sqx, 0, P, 0, [[XW, C], [1, XW]])
                        sq_in = _ap(xt, 0, P, 0, [[1, C], [C, XW]])
                        s_ap = lambda c: _ap(sqx, 0, P, c * XW, [[1, XW]])
                    else:
                        sq_out, sq_in = sqx[:], xt[:]
                        s_ap = lambda c: _ap(sqx, 0, P, c, [[C, XW]])
                    if sq_dve:
                        nc.vector.tensor_mul(sq_out, sq_in, sq_in)
                    else:
                        nc.scalar.square(sq_out, sq_in)
                    stt = pool.tile([P, XW], BF16, name="stt")
                    st = pool.tile([P, XW], BF16, name="st")
                    nc.vector.tensor_add(stt[:], s_ap(0), s_ap(1))
                    nc.vector.tensor_add(st[:], stt[:], s_ap(2))

                # ---- M_up[p] = M[p+1] for k=0..4 cols; phantom z=64 rows
                # ({63,127}) = s(z=63 row) with k-dependent a-shift ----
                if pe_shift:
                    # PE permutation matmul: M_up = SH2^T.T @ M + SEL.T @ SD
                    # (exact for 0/1 matrices, also in bf16); phantom rows
                    # ride the second accumulating matmul through SD.  In
                    # planes mode everything is bf16 => 2x PE rate and a 4x
                    # TensorCopy for SD.
                    SD = pool.tile([P, 5 * MW], MDT, name="SD")
                    nc.vector.tensor_copy(
                        _ap(SD, 0, P, 0, [[MW, 5], [1, MW]]),
                        _ap(st, 0, P, 0, [[1, 5], [1, MW]]))
                    M_up = psum.tile([P, 5 * MW], F32, name="M_up_ps")
                    for n0 in range(0, 5 * MW, 512):
                        n1 = min(5 * MW, n0 + 512)
                        nc.tensor.matmul(
                            _ap(M_up, 0, P, n0, [[1, n1 - n0]]),
                            sh_t[:], _ap(M, 0, P, n0, [[1, n1 - n0]]),
                            start=True, stop=False)
                        nc.tensor.matmul(
                            _ap(M_up, 0, P, n0, [[1, n1 - n0]]),
                            sel_t[:], _ap(SD, 0, P, n0, [[1, n1 - n0]]),
                            start=False, stop=True)
                else:
                    M_up = pool.tile([P, 5 * MW], F32, name="M_up")
                    # disjoint remaps per batch so the phantom DMA runs parallel
                    for b in range(LB):
                        nc.sync.dma_start(
                            _ap(M_up, b * Z, Z - 1, 0, [[1, 5 * MW]]),
                            _ap(M, b * Z + 1, Z - 1, 0, [[1, 5 * MW]]))
                    # phantom: M_up[{63,127}, k*MW + ar] = st[{63,127}, ar + k]
                    nc.sync.dma_start(
                        _ap(M_up, Z - 1, 2, 0, [[MW, 5], [1, MW]], pstep=Z),
                        _ap(st, Z - 1, 2, 0, [[1, 5], [1, MW]], pstep=Z))

                # ---- exps into O staging [p, ar*56 + k*4 + c] ----
                # amir_dve: the a-mirror slots k'=7,8 duplicate the direct
                # k=6,5 exps at shifted a — a same-partition DVE copy
                # (rebalances element writes from the bottleneck ACT onto
                # DVE), with a 2-column ACT patch at the chunk edge where
                # the copy source falls outside this O tile.
                O = pool.tile([P, ac * O_A], F32, name="O",
                              bufs=(1 if ac >= 512 else
                                    2 if ac >= 256 else None))
                EXP = mybir.ActivationFunctionType.Exp
                for th, sc in ((0, SC0), (1, SC1)):
                    co = 2 * th
                    # direct k=0..6: in M[p, k*MW + ar + 2]
                    nc.scalar.activation(
                        _ap(O, 0, P, co, [[4, 7], [O_A, ac], [1, 2]]),
                        _ap(M, 0, P, 2, [[MW, 7], [1, ac], [0, 2]]),
                        EXP, scale=sc)
                    if amir_dve:
                        # boundary patch: k'=7,8 at a in {ac-2, ac-1}
                        nc.scalar.activation(
                            _ap(O, 0, P, 28 + co + (ac - 2) * O_A,
                                [[4, 2], [O_A, 2], [1, 2]]),
                            _ap(M, 0, P, 6 * MW + 3 + (ac - 2),
                                [[-(MW - 1), 2], [1, 2], [0, 2]]),
                            EXP, scale=sc)
                    else:
                        # a-mirrors k'=7,8 <- k=6,5: col = k*MW + ar + (9-k)
                        nc.scalar.activation(
                            _ap(O, 0, P, 28 + co, [[4, 2], [O_A, ac], [1, 2]]),
                            _ap(M, 0, P, 6 * MW + 3,
                                [[-(MW - 1), 2], [1, ac], [0, 2]]),
                            EXP, scale=sc)
                    # dz-mirrors k'=9..13 <- k=4..0: M_up[p, k*MW + ar + 4-k]
                    # (partition-shifted values, not copyable within a lane)
                    nc.scalar.activation(
                        _ap(O, 0, P, 36 + co, [[4, 5], [O_A, ac], [1, 2]]),
                        _ap(M_up, 0, P, 4 * (MW - 1) + 4,
                            [[-(MW - 1), 5], [1, ac], [0, 2]]),
                        EXP, scale=sc)
                if amir_dve:
                    # O[a, 7+j, c] = O[a+1+j, 6-j, c] for a < ac-2, all c
                    nc.vector.tensor_copy(
                        _ap(O, 0, P, 28, [[4, 2], [O_A, ac - 2], [1, 4]]),
                        _ap(O, 0, P, 80, [[52, 2], [O_A, ac - 2], [1, 4]]))

                # ---- store: contiguous 128-partition DMAs.  Each HWDGE
                # ring sustains ~418 GB/s independently (SP + ACT pair
                # ~800 aggregate), so the store of EVERY chunk is split
                # into equal per-ring pieces — alternating whole chunks
                # leaves one ring with 62.5% of the bytes and that ring's
                # 21.9 us/pass becomes the kernel's floor.  store_rings=3
                # adds the gpsimd SWDGE ring. ----
                rep_i = ci // NCH
                dst_ap = o_ap if rep_i == reps - 1 else scratch_aps[rep_i]
                engs = (nc.sync, nc.scalar, nc.gpsimd)[:store_rings]
                wts = ring_wts or (1,) * store_rings
                tot = sum(wts)
                cum = [sum(wts[:r]) for r in range(store_rings + 1)]
                edges = [c * ac // tot for c in cum]
                for r, eng in enumerate(engs):
                    w0, w1 = edges[r], edges[r + 1]
                    eng.dma_start(
                        bass.AP(tensor=dst_ap.tensor,
                                offset=(a0 + w0) * O_A,
                                ap=[[O_Z, P], [1, (w1 - w0) * O_A]]),
                        _ap(O, 0, P, w0 * O_A, [[1, (w1 - w0) * O_A]]))

    nc.compile()
    return nc


class _Runner:
    """Compile once; reuse the jitted sharded executable across calls.

    Mirrors bass2jax.run_bass_via_pjrt's multi-core path, but without
    donated output buffers (the kernel writes every output element, so the
    zero "output operands" are passed once from device-resident buffers and
    reused)."""

    def __init__(self, nc=None):
        import jax
        from jax.sharding import Mesh, PartitionSpec, NamedSharding
        try:
            from jax.experimental.shard_map import shard_map
        except ImportError:
            from jax.shard_map import shard_map  # newer jax
        from concourse import bass2jax

        bass2jax.install_neuronx_cc_hook()
        if nc is None:
            nc = _build()
        self.nc = nc

        partition_name = (nc.partition_id_tensor.name
                          if nc.partition_id_tensor else None)
        in_names, out_names, out_avals = [], [], []
        in_dtypes = {}
        for alloc in nc.m.functions[0].allocations:
            if not isinstance(alloc, mybir.MemoryLocationSet):
                continue
            name = alloc.memorylocations[0].name
            if alloc.kind == "ExternalInput":
                if name != partition_name:
                    in_names.append(name)
                    in_dtypes[name] = mybir.dt.np(alloc.dtype)
            elif alloc.kind == "ExternalOutput":
                out_names.append(name)
                out_avals.append(jax.core.ShapedArray(
                    tuple(alloc.tensor_shape), mybir.dt.np(alloc.dtype)))
        self.in_dtypes = in_dtypes
        assert set(in_names) <= {"x", "shm", "sel", "shd"}, in_names
        assert out_names == ["out"], out_names
        all_in_names = in_names + out_names
        if partition_name is not None:
            all_in_names = all_in_names + [partition_name]
        self.in_names = in_names

        def _body(*args):
            operands = list(args)
            if partition_name is not None:
                operands.append(bass2jax.partition_id_tensor())
            return tuple(bass2jax._bass_exec_p.bind(
                *operands,
                out_avals=tuple(out_avals),
                in_names=tuple(all_in_names),
                out_names=tuple(out_names),
                lowering_input_output_aliases=(),
                sim_require_finite=True,
                sim_require_nnan=True,
                nc=nc,
            ))

        devices = jax.devices()[:N_CORES]
        assert len(devices) == N_CORES
        self.mesh = Mesh(np.asarray(devices), ("core",))
        spec = PartitionSpec("core")
        rep = PartitionSpec()
        self.sharding = NamedSharding(self.mesh, spec)
        in_specs = tuple(spec if n == "x" else rep for n in in_names) + (spec,)
        self.jitted = jax.jit(shard_map(
            _body, mesh=self.mesh, in_specs=in_specs, out_specs=(spec,),
            check_rep=False))
        # device-resident constant operands, created once
        self.zeros_dev = jax.device_put(
            np.zeros((N_CORES * LB, Z, A, K, NCLS), np.float32), self.sharding)
        consts = {}
        if "shm" in in_names:
            shm, sel, shd = _host_shift_mats()
            rep_sh = NamedSharding(self.mesh, rep)
            for n, arr in (("shm", shm), ("sel", sel), ("shd", shd)):
                if n in in_names:
                    consts[n] = jax.device_put(
                        arr.astype(in_dtypes[n]), rep_sh)
        self.consts = consts
        self._jax = jax

    def put(self, x: np.ndarray):
        return self._jax.device_put(
            np.ascontiguousarray(np.asarray(x, np.float32)), self.sharding)

    def run_dev(self, x_dev):
        """Execute; returns device array (not fetched)."""
        args = [x_dev if n == "x" else self.consts[n] for n in self.in_names]
        return self.jitted(*args, self.zeros_dev)[0]

    def __call__(self, x: np.ndarray) -> np.ndarray:
        return np.asarray(self.run_dev(self.put(x)))


_RUNNER = None


def _get_runner():
    global _RUNNER
    if _RUNNER is None:
        _RUNNER = _Runner()
    return _RUNNER


def kernel(x: np.ndarray) -> np.ndarray:
    x = np.asarray(x, dtype=np.float32)
    assert x.shape == (B, Z, A, C), x.shape
    try:
        return _get_runner()(x)
    except Exception:
        # fallback: reference-quality but slower dispatch path
        nc = _build()
        extra = {}
        if PE_SHIFT:
            shm, sel, shd = _host_shift_mats()
            cdt = mybir.dt.np(BF16)  # matches _build(planes=True) default
            extra = {"shm": shm.astype(cdt), "sel": sel.astype(cdt),
                     "shd": shd}
        in_maps = [{"x": np.ascontiguousarray(x[i * LB:(i + 1) * LB]), **extra}
                   for i in range(N_CORES)]
        res = run_bass_kernel_spmd(nc, in_maps, list(range(N_CORES)))
        return np.concatenate(
            [res.results[i]["out"] for i in range(N_CORES)], axis=0)



# revision 38
# speedup vs baseline: 1.1806x; 1.1806x over previous
"""BilateralFilter (SqueezeSeg mc condensing-kernel gaussians) on 8 TRN2 cores.

Reference computes, for x: [16, 64, 512, 3] (B, Z, A, C=xyz):
    nbr   = 14 spatial neighbors of each pixel in a 3x5 window (zero-padded)
    diff2 = sum_c (x - nbr)^2                           [B, Z, A, 14]
    out   = exp(-diff2 / (2 * theta_r^2))               [B, Z, A, 14, 4]
with THETA_R = [0.015, 0.015, 0.01, 0.01] (only 2 distinct values).

Strategy (pure batch data-parallel, 2 batches per core):
  - partitions p = b*64 + z  (128), free dim = azimuth chunks (AC wide).
  - squared differences via a runtime-registered fused custom DVE op
    (out = (in0-in1)^2), channel sums via tensor_reduce.
  - mirror symmetry: m_k(q) = |x(q) - x(q+off_k)|^2 for the 7 "negative"
    offsets k=0..6 gives the other 7 via diff2_{13-k}(q) = m_k(q - off_k);
    the z+1-partition read (engines cannot shift partitions by 1) is
    materialized on the idle TensorE as an exact 0/1 permutation matmul
    into PSUM, with the phantom z=64 boundary row (out-of-image neighbor
    => diff2 = |x(center)|^2, from s = sum_c x^2) accumulated by a second
    selector matmul. (PE_SHIFT=False falls back to partition-remap DMAs.)
  - ACT computes exp with the free scale immediate; each exp is written to
    both classes of its theta pair via a stride-0 input axis, directly into
    the interleaved [a, k, c] staging layout.
  - the staging tile matches DRAM layout exactly, so the store is one
    contiguous 128-partition DMA (28 KB/partition runs at AC=128).
"""

import numpy as np

import concourse.bass as bass
import concourse.tile as tile
from concourse import bacc, mybir
from concourse.bass_utils import run_bass_kernel_spmd

N_CORES = 8
B, Z, A, C = 16, 64, 512, 3
K, NCLS = 14, 4
LB = B // N_CORES            # local batches per core = 2
P = LB * Z                   # 128 partitions
AC = 128                     # azimuth chunk
BUFS = 3                     # tile pool buffers
PE_SHIFT = True              # z+1 partition shift via PE matmul vs SBUF DMA
XDN_PE = True                # derive x_dn on PE too (no duplicate DRAM read)
F32 = mybir.dt.float32
BF16 = mybir.dt.bfloat16


def _host_shift_mats():
    """SH2[k, m] = 1 iff k == m+1 (and not m == 63: batch boundary);
    SEL[k, m] = 1 iff k == m in {63, 127} (phantom z=64 row selector);
    SHD[k, m] = 1 iff k == m-1 (and not m in {0, 64}: z=0 rows stay 0)."""
    sh = np.zeros((P, P), np.float32)
    for m in range(P - 1):
        if m != Z - 1:
            sh[m + 1, m] = 1.0
    sel = np.zeros((P, P), np.float32)
    sel[Z - 1, Z - 1] = 1.0
    sel[P - 1, P - 1] = 1.0
    shd = np.zeros((P, P), np.float32)
    for m in range(1, P):
        if m != Z:
            shd[m - 1, m] = 1.0
    return sh, sel, shd

# exp scales: -1 / (2 * theta^2), theta pairs (0.015, 0.01), f32 semantics
_t0 = np.float32(0.015)
_t1 = np.float32(0.01)
SC0 = -float(1.0 / np.float32(np.float32(2.0) * _t0 * _t0))
SC1 = -float(1.0 / np.float32(np.float32(2.0) * _t1 * _t1))

# DRAM strides (elements) of out [LB, Z, A, K, NCLS]
O_A = K * NCLS               # 56
O_Z = A * O_A                # 28672
O_B = Z * O_Z                # 1835008
X_Z = A * C                  # 1536
X_B = Z * X_Z


def _ap(t, poff, pcnt, foff, pairs, pstep=1):
    """AP on tile t: partitions [poff, poff+pcnt) (stride pstep rows), free
    `pairs` ([step, count] in elements) based at element foff."""
    row = t.ap[0][0]
    return bass.AP(tensor=t.tensor, offset=t.offset + poff * row + foff,
                   ap=[[pstep * row, pcnt]] + [list(p) for p in pairs])


_SQDIFF = None


def _get_sqdiff():
    """Register a runtime custom DVE op: out = (in0 - in1)^2 (fp32, one
    instruction instead of subtract + multiply)."""
    global _SQDIFF
    if _SQDIFF is not None:
        return _SQDIFF
    from concourse import dve_ops
    from concourse.dve_spec import Spec, Src0, Src1, sq, lower, _has_src1
    from concourse.dve_uop import DveOpSpec

    name = "SQDIFF_BILAT_ANT"
    if name not in dve_ops._SUB_OPCODE_FOR_NAME:
        spec = Spec(
            body=sq(Src0 - Src1),
            reference=lambda in0, in1, c0, c1, c2:
                (in0.astype(np.float32) - in1.astype(np.float32)) ** 2)
        row = 1 + len(dve_ops.OPS)
        assert row < 0x20
        shas = {}
        for ver in ("v3",):
            tmp = DveOpSpec(name=name, opcode=row, uops=lower(spec, ver=ver),
                            rd1_en=_has_src1(spec))
            shas[ver] = tmp.sha(ver)
        op = dve_ops.DveOp(name, spec, subdim=False, uops_sha=shas)
        dve_ops.OPS.append(op)
        dve_ops.CUSTOM_DVE_SPECS[name] = spec
        dve_ops._SUB_OPCODE_FOR_NAME[name] = row
    else:
        op = next(o for o in dve_ops.OPS if o.name == name)
    _SQDIFF = op
    return op


def _build(ac=AC, bufs=BUFS, reps=1, pe_shift=PE_SHIFT, xdn_pe=XDN_PE,
           chunks=None, psum_bufs=3, store_rings=3, dt_mode="planes",
           amir_dve=0, sq_dve=False, ring_wts=None):
    # chunk schedule: list of (a0, width).  Uniform chunks minimize the
    # per-chunk fixed instruction overhead (~185 ns per ACT instruction,
    # 6 of them per chunk); with deep load prefetch the pipeline fill no
    # longer needs smaller leading chunks, and fill amortizes over reps.
    if chunks is None:
        chunks = [(a0, ac) for a0 in range(0, A, ac)]
    assert sum(w for _, w in chunks) == A
    NCH = len(chunks)
    nc = bacc.Bacc("TRN2", target_bir_lowering=False, debug=False,
                   num_devices=N_CORES)
    x_h = nc.dram_tensor("x", [LB, Z, A, C], F32, kind="ExternalInput")
    o_h = nc.dram_tensor("out", [LB, Z, A, K, NCLS], F32, kind="ExternalOutput")
    x_ap, o_ap = x_h.ap(), o_h.ap()
    if pe_shift:
        # 0/1 shift matrices are exact in bf16 (2x PE when M is bf16)
        CDT = F32 if dt_mode == "reduce" else BF16
        shm_h = nc.dram_tensor("shm", [P, P], CDT, kind="ExternalInput")
        sel_h = nc.dram_tensor("sel", [P, P], CDT, kind="ExternalInput")
        if xdn_pe:
            shd_h = nc.dram_tensor("shd", [P, P], F32, kind="ExternalInput")
    # bench mode: reps > 1 re-runs the whole kernel; each non-final pass
    # stores to its own DRAM scratch so stores are real traffic
    scratch_aps = [
        nc.dram_tensor(f"scr{r}", [LB, Z, A, K, NCLS], F32).ap()
        for r in range(reps - 1)]

    from contextlib import ExitStack
    with tile.TileContext(nc) as tc, ExitStack() as es:
        if pe_shift:
            consts = es.enter_context(tc.tile_pool(name="consts", bufs=1))
            psum = es.enter_context(
                tc.tile_pool(name="psum", bufs=psum_bufs, space="PSUM"))
        with tc.tile_pool(name="pool", bufs=bufs) as pool:
            if pe_shift:
                sh_t = consts.tile([P, P], CDT, name="sh_t")
                nc.sync.dma_start(sh_t[:], shm_h.ap()[:])
                sel_t = consts.tile([P, P], CDT, name="sel_t")
                nc.sync.dma_start(sel_t[:], sel_h.ap()[:])
                if xdn_pe:
                    shd_t = consts.tile([P, P], F32, name="shd_t")
                    nc.sync.dma_start(shd_t[:], shd_h.ap()[:])
            N = NCH * reps

            def _geom(ci):
                a0, ac = chunks[ci % NCH]
                XW = ac + 8          # x window (halo 4 each side)
                lo, hi = max(0, a0 - 4), min(A, a0 + ac + 4)
                c_lo = (lo - (a0 - 4)) * C          # first valid xt col
                c_hi = (hi - (a0 - 4)) * C
                return a0, ac, XW, lo, hi, c_lo, c_hi

            def emit_load(ci):
                # ---- load x window (zero halo at image borders) ----
                # (b, z) rows are contiguous in DRAM: one 128-partition DMA.
                # Loads issue on the (otherwise idle) gpsimd SWDGE so they
                # are not program-ordered behind the big store issues on SP
                # — the next chunks' loads must cut ahead of queued stores
                # or compute stalls behind them.
                # deep rotation: loads must be queued well before the big
                # stores they contend with, or they wait out a full 10 us
                # store before landing (xt is tiny: 1.6 KB/partition/buf)
                _, _, XW, lo, hi, c_lo, c_hi = _geom(ci)
                xt = pool.tile([P, XW * C], F32, name="xt", bufs=8)
                if c_lo > 0:
                    nc.gpsimd.memset(_ap(xt, 0, P, 0, [[1, c_lo]]), 0.0)
                if c_hi < XW * C:
                    nc.gpsimd.memset(
                        _ap(xt, 0, P, c_hi, [[1, XW * C - c_hi]]), 0.0)
                nc.gpsimd.dma_start(
                    _ap(xt, 0, P, c_lo, [[C, hi - lo], [1, C]]),
                    bass.AP(tensor=x_ap.tensor, offset=lo * C,
                            ap=[[X_Z, P], [C, hi - lo], [1, C]]))
                return xt

            def emit_xdn(ci, xt):
                # ---- x_dn[p] = x at (z-1) (zeros at z=0 rows): exact PE
                # permutation shift of xt into PSUM; the zero columns of SHD
                # give the z=0 rows (and the xt halo the image-border zeros)
                # for free.  Emitted one chunk AHEAD of the consuming chunk:
                # the PE is in-order, so x_dn(i+1) must precede M_up(i) or
                # the serial loop DVE(i) -> M_up(i) -> x_dn(i+1) -> DVE(i+1)
                # paces the pipeline above the store rate.  bufs=2 so the
                # psum pool fits 8 banks (M_up 3x2 + x_dn 2x1).
                _, _, XW, _, _, _, _ = _geom(ci)
                x_dn = psum.tile([P, XW * C], F32, name="x_dn_ps", bufs=2)
                for n0 in range(0, XW * C, 512):
                    n1 = min(XW * C, n0 + 512)
                    nc.tensor.matmul(
                        _ap(x_dn, 0, P, n0, [[1, n1 - n0]]),
                        shd_t[:], _ap(xt, 0, P, n0, [[1, n1 - n0]]),
                        start=True, stop=True)
                return x_dn

            PF = 7               # load prefetch distance (chunks ahead)
            xts, xdns = {}, {}
            for j in range(min(PF, N)):
                xts[j] = emit_load(j)
            if pe_shift and xdn_pe:
                xdns[0] = emit_xdn(0, xts[0])

            for ci in range(N):
                a0, ac, XW, lo, hi, c_lo, c_hi = _geom(ci)
                MW = ac + 4          # m window (halo 2 each side)

                if ci + PF < N:
                    xts[ci + PF] = emit_load(ci + PF)
                if pe_shift and xdn_pe and ci + 1 < N:
                    xdns[ci + 1] = emit_xdn(ci + 1, xts[ci + 1])
                xt = xts.pop(ci)

                if pe_shift and xdn_pe:
                    x_dn = xdns.pop(ci)
                else:
                    x_dn = pool.tile([P, XW * C], F32, name="x_dn")
                    nc.gpsimd.memset(x_dn[:], 0.0)
                    for b in range(LB):
                        nc.gpsimd.dma_start(
                            _ap(x_dn, b * Z + 1, Z - 1, c_lo,
                                [[C, hi - lo], [1, C]]),
                            bass.AP(tensor=x_ap.tensor, offset=b * X_B + lo * C,
                                    ap=[[X_Z, Z - 1], [C, hi - lo], [1, C]]))

                # ---- s = sum_c x^2 ; m_k maps over a-window [a0-2, ...)
                # k=0..4: dz=-1, da=k-2 ; k=5,6: dz=0, da=k-7
                # d2 = (x - x_nbr)^2 in one fused custom op per k.
                # dt_mode picks how the c-sum is done:
                #  "reduce": f32 interleaved + TensorReduce (no fast mode)
                #  "iadds":  bf16 interleaved (packed writes) + 2 stride-3
                #            tensor_adds — each add processes N/3 elements,
                #            beating the reduce's full-N stream
                #  "planes": custom writes c-outer packed bf16 planes; the
                #            adds are fully packed and hit the DVE 2x mode
                # bf16 rounds only d^2 / s / M (<=0.4% rel => ~2e-3 max abs
                # on the exp output, far inside the 2e-2 tolerance).
                sqdiff = _get_sqdiff()
                MDT = F32 if dt_mode == "reduce" else BF16
                M = pool.tile([P, 7 * MW], MDT, name="M")
                if dt_mode == "reduce":
                    sqx = pool.tile([P, XW * C], F32, name="sqx")
                    nc.scalar.square(sqx[:], xt[:])
                    st = pool.tile([P, XW], F32, name="st")
                    nc.vector.tensor_reduce(
                        st[:], _ap(sqx, 0, P, 0, [[C, XW], [1, C]]),
                        axis=mybir.AxisListType.X, op=mybir.AluOpType.add)
                    dt5 = pool.tile([P, 5 * MW * C], F32, name="dt5")
                    for k in range(5):
                        nc.vector._custom_dve(
                            sqdiff,
                            out=_ap(dt5, 0, P, k * MW * C, [[C, MW], [1, C]]),
                            in0=_ap(xt, 0, P, 2 * C, [[C, MW], [1, C]]),
                            in1=_ap(x_dn, 0, P, k * C, [[C, MW], [1, C]]))
                    nc.vector.tensor_reduce(
                        _ap(M, 0, P, 0, [[1, 5 * MW]]),
                        _ap(dt5, 0, P, 0, [[C, 5 * MW], [1, C]]),
                        axis=mybir.AxisListType.X, op=mybir.AluOpType.add)
                    dt2 = pool.tile([P, 2 * MW * C], F32, name="dt2")
                    for k in (5, 6):
                        nc.vector._custom_dve(
                            sqdiff,
                            out=_ap(dt2, 0, P, (k - 5) * MW * C,
                                    [[C, MW], [1, C]]),
                            in0=_ap(xt, 0, P, 2 * C, [[C, MW], [1, C]]),
                            in1=_ap(xt, 0, P, (k - 5) * C, [[C, MW], [1, C]]))
                    nc.vector.tensor_reduce(
                        _ap(M, 0, P, 5 * MW, [[1, 2 * MW]]),
                        _ap(dt2, 0, P, 0, [[C, 2 * MW], [1, C]]),
                        axis=mybir.AxisListType.X, op=mybir.AluOpType.add)
                else:
                    # custom-dve APs are rank<=3: one call per map k
                    dt = pool.tile([P, 3 * 7 * MW], BF16, name="dt")
                    if dt_mode == "planes":
                        # c-outer stream: strided f32 reads, PACKED bf16
                        # plane writes (scattered 2-byte writes would RMW)
                        d_out = lambda k: _ap(dt, 0, P, k * MW,
                                              [[7 * MW, C], [1, MW]])
                        d_in = lambda t, off: _ap(t, 0, P, off,
                                                  [[1, C], [C, MW]])
                        add_ap = lambda c: _ap(dt, 0, P, c * 7 * MW,
                                               [[1, 7 * MW]])
                    else:  # iadds: natural interleaved stream, packed writes
                        d_out = lambda k: _ap(dt, 0, P, 3 * k * MW,
                                              [[C, MW], [1, C]])
                        d_in = lambda t, off: _ap(t, 0, P, off,
                                                  [[C, MW], [1, C]])
                        add_ap = lambda c: _ap(dt, 0, P, c,
                                               [[C, 7 * MW]])
                    for k in range(7):
                        src, off = (x_dn, k * C) if k < 5 else (xt, (k - 5) * C)
                        nc.vector._custom_dve(
                            sqdiff, out=d_out(k),
                            in0=d_in(xt, 2 * C), in1=d_in(src, off))
                    dts = pool.tile([P, 7 * MW], BF16, name="dts")
                    nc.vector.tensor_add(dts[:], add_ap(0), add_ap(1))
                    nc.vector.tensor_add(M[:], dts[:], add_ap(2))

                    # s = sum_c x^2 via the same layout trick
                    sqx = pool.tile([P, 3 * XW], BF16, name="sqx")
                    if dt_mode == "planes":
                        sq_out = _ap(sqx, 0, P, 0, [[XW, C], [1, XW]])
                        sq_in = _ap(xt, 0, P, 0, [[1, C], [C, XW]])
                        s_ap = lambda c: _ap(sqx, 0, P, c * XW, [[1, XW]])
                    else:
                        sq_out, sq_in = sqx[:], xt[:]
                        s_ap = lambda c: _ap(sqx, 0, P, c, [[C, XW]])
                    if sq_dve:
                        nc.vector.tensor_mul(sq_out, sq_in, sq_in)
                    else:
                        nc.scalar.square(sq_out, sq_in)
                    stt = pool.tile([P, XW], BF16, name="stt")
                    st = pool.tile([P, XW], BF16, name="st")
                    nc.vector.tensor_add(stt[:], s_ap(0), s_ap(1))
                    nc.vector.tensor_add(st[:], stt[:], s_ap(2))

                # ---- M_up[p] = M[p+1] for k=0..4 cols; phantom z=64 rows
                # ({63,127}) = s(z=63 row) with k-dependent a-shift ----
                if pe_shift:
                    # PE permutation matmul: M_up = SH2^T.T @ M + SEL.T @ SD
                    # (exact for 0/1 matrices, also in bf16); phantom rows
                    # ride the second accumulating matmul through SD.  In
                    # planes mode everything is bf16 => 2x PE rate and a 4x
                    # TensorCopy for SD.
                    SD = pool.tile([P, 5 * MW], MDT, name="SD")
                    nc.vector.tensor_copy(
                        _ap(SD, 0, P, 0, [[MW, 5], [1, MW]]),
                        _ap(st, 0, P, 0, [[1, 5], [1, MW]]))
                    M_up = psum.tile([P, 5 * MW], F32, name="M_up_ps")
                    for n0 in range(0, 5 * MW, 512):
                        n1 = min(5 * MW, n0 + 512)
                        nc.tensor.matmul(
                            _ap(M_up, 0, P, n0, [[1, n1 - n0]]),
                            sh_t[:], _ap(M, 0, P, n0, [[1, n1 - n0]]),
                            start=True, stop=False)
                        nc.tensor.matmul(
                            _ap(M_up, 0, P, n0, [[1, n1 - n0]]),
                            sel_t[:], _ap(SD, 0, P, n0, [[1, n1 - n0]]),
                            start=False, stop=True)
                else:
                    M_up = pool.tile([P, 5 * MW], F32, name="M_up")
                    # disjoint remaps per batch so the phantom DMA runs parallel
                    for b in range(LB):
                        nc.sync.dma_start(
                            _ap(M_up, b * Z, Z - 1, 0, [[1, 5 * MW]]),
                            _ap(M, b * Z + 1, Z - 1, 0, [[1, 5 * MW]]))
                    # phantom: M_up[{63,127}, k*MW + ar] = st[{63,127}, ar + k]
                    nc.sync.dma_start(
                        _ap(M_up, Z - 1, 2, 0, [[MW, 5], [1, MW]], pstep=Z),
                        _ap(st, Z - 1, 2, 0, [[1, 5], [1, MW]], pstep=Z))

                # ---- exps into O staging [p, ar*56 + k*4 + c] ----
                # amir_dve: the a-mirror slots k'=7,8 duplicate the direct
                # k=6,5 exps at shifted a — a same-partition DVE copy
                # (rebalances element writes from the bottleneck ACT onto
                # DVE), with a 2-column ACT patch at the chunk edge where
                # the copy source falls outside this O tile.
                O = pool.tile([P, ac * O_A], F32, name="O",
                              bufs=(1 if ac >= 512 else
                                    2 if ac >= 256 else None))
                EXP = mybir.ActivationFunctionType.Exp
                for th, sc in ((0, SC0), (1, SC1)):
                    co = 2 * th
                    # direct k=0..6: in M[p, k*MW + ar + 2]
                    nc.scalar.activation(
                        _ap(O, 0, P, co, [[4, 7], [O_A, ac], [1, 2]]),
                        _ap(M, 0, P, 2, [[MW, 7], [1, ac], [0, 2]]),
                        EXP, scale=sc)
                    if amir_dve:
                        # boundary patch: k'=7,8 at a in {ac-2, ac-1}
                        nc.scalar.activation(
                            _ap(O, 0, P, 28 + co + (ac - 2) * O_A,
                                [[4, 2], [O_A, 2], [1, 2]]),
                            _ap(M, 0, P, 6 * MW + 3 + (ac - 2),
                                [[-(MW - 1), 2], [1, 2], [0, 2]]),
                            EXP, scale=sc)
                    else:
                        # a-mirrors k'=7,8 <- k=6,5: col = k*MW + ar + (9-k)
                        nc.scalar.activation(
                            _ap(O, 0, P, 28 + co, [[4, 2], [O_A, ac], [1, 2]]),
                            _ap(M, 0, P, 6 * MW + 3,
                                [[-(MW - 1), 2], [1, ac], [0, 2]]),
                            EXP, scale=sc)
                    # dz-mirrors k'=9..13 <- k=4..0: M_up[p, k*MW + ar + 4-k]
                    # (partition-shifted values, not copyable within a lane)
                    nc.scalar.activation(
                        _ap(O, 0, P, 36 + co, [[4, 5], [O_A, ac], [1, 2]]),
                        _ap(M_up, 0, P, 4 * (MW - 1) + 4,
                            [[-(MW - 1), 5], [1, ac], [0, 2]]),
                        EXP, scale=sc)
                if amir_dve:
                    # O[a, 7+j, c] = O[a+1+j, 6-j, c] for a < ac-2, all c
                    nc.vector.tensor_copy(
                        _ap(O, 0, P, 28, [[4, 2], [O_A, ac - 2], [1, 4]]),
                        _ap(O, 0, P, 80, [[52, 2], [O_A, ac - 2], [1, 4]]))

                # ---- store: contiguous 128-partition DMAs.  Each HWDGE
                # ring sustains ~418 GB/s independently (SP + ACT pair
                # ~800 aggregate), so the store of EVERY chunk is split
                # into equal per-ring pieces — alternating whole chunks
                # leaves one ring with 62.5% of the bytes and that ring's
                # 21.9 us/pass becomes the kernel's floor.  store_rings=3
                # adds the gpsimd SWDGE ring. ----
                rep_i = ci // NCH
                dst_ap = o_ap if rep_i == reps - 1 else scratch_aps[rep_i]
                if store_rings == 0:   # timing probe: no output stores
                    continue
                engs = (nc.sync, nc.scalar, nc.gpsimd)[:store_rings]
                wts = ring_wts or (1,) * store_rings
                tot = sum(wts)
                cum = [sum(wts[:r]) for r in range(store_rings + 1)]
                edges = [c * ac // tot for c in cum]
                for r, eng in enumerate(engs):
                    w0, w1 = edges[r], edges[r + 1]
                    eng.dma_start(
                        bass.AP(tensor=dst_ap.tensor,
                                offset=(a0 + w0) * O_A,
                                ap=[[O_Z, P], [1, (w1 - w0) * O_A]]),
                        _ap(O, 0, P, w0 * O_A, [[1, (w1 - w0) * O_A]]))

    nc.compile()
    return nc


def _build2(ac=512, bufs=2, reps=1, chunks=None, psum_bufs=1, store_rings=3,
            fuse_sq=True, pool_s=True, xdn_dma=True, pool_xpose=False,
            xpose_act=1, pf=3, dup_xt=False, ring_wts=None):
    """v2: dedup bf16 output [LB, Z, 2, K, A] -- classes within a theta pair
    are bit-identical (host broadcasts 2 -> 4 classes and permutes axes).
    Everything on chip lives in PLANE layout ([t*14+k][a], a innermost):
    every engine access streams long contiguous runs -- 2-byte scattered
    accesses on ACT cost 2-4x.  x is transposed once per chunk into c-planes
    (split Q7/ACT) so the fused per-c DVE sqdiff streams fully packed; sq on
    gpsimd, s-adds on DVE; dz-mirror planes built on PE in j-layout (one
    PSUM J tile, matmul pieces at bank boundaries, phantom z=64 row via a
    stride-0 moving axis over st); x_dn via SBUF partition-remap DMAs;
    stores split by plane across the three DMA queues (512B runs)."""
    if chunks is None:
        chunks = [(a0, ac) for a0 in range(0, A, ac)]
    assert sum(w for _, w in chunks) == A
    NCH = len(chunks)
    O_T = K * A                  # out plane-group stride (t axis)
    O_P = 2 * K * A              # out partition stride (z axis)
    nc = bacc.Bacc("TRN2", target_bir_lowering=False, debug=False,
                   num_devices=N_CORES)
    x_h = nc.dram_tensor("x", [LB, Z, A, C], F32, kind="ExternalInput")
    o_h = nc.dram_tensor("out", [LB, Z, 2, K, A], BF16, kind="ExternalOutput")
    x_ap, o_ap = x_h.ap(), o_h.ap()
    shm_h = nc.dram_tensor("shm", [P, P], BF16, kind="ExternalInput")
    sel_h = nc.dram_tensor("sel", [P, P], BF16, kind="ExternalInput")
    shd_h = None
    if not xdn_dma:
        shd_h = nc.dram_tensor("shd", [P, P], F32, kind="ExternalInput")
    scratch_aps = [
        nc.dram_tensor(f"scr{r}", [LB, Z, 2, K, A], BF16).ap()
        for r in range(reps - 1)]

    sqdiff = _get_sqdiff()
    from contextlib import ExitStack
    with tile.TileContext(nc) as tc, ExitStack() as es:
        consts = es.enter_context(tc.tile_pool(name="consts", bufs=1))
        psum = es.enter_context(
            tc.tile_pool(name="psum", bufs=psum_bufs, space="PSUM"))
        with tc.tile_pool(name="pool", bufs=bufs) as pool:
            sh_t = consts.tile([P, P], BF16, name="sh_t")
            nc.sync.dma_start(sh_t[:], shm_h.ap()[:])
            sel_t = consts.tile([P, P], BF16, name="sel_t")
            nc.sync.dma_start(sel_t[:], sel_h.ap()[:])
            if xdn_dma:
                # zero source rows for x_dn partitions {0, 64} (z=0 rows)
                zt = consts.tile([P, 3 * (ac + 8)], F32, name="zt")
                nc.gpsimd.memset(zt[:], 0.0)
            else:
                shd_t = consts.tile([P, P], F32, name="shd_t")
                nc.sync.dma_start(shd_t[:], shd_h.ap()[:])
            N = NCH * reps

            def _geom(ci):
                a0, acw = chunks[ci % NCH]
                XW = acw + 8
                lo, hi = max(0, a0 - 4), min(A, a0 + acw + 4)
                wlo = lo - (a0 - 4)       # first valid a in window coords
                whi = hi - (a0 - 4)
                return a0, acw, XW, lo, hi, wlo, whi

            def emit_load(ci):
                # interleaved [a, c] x window; halo memsets happen on the
                # transposed tile, so only the DMA here.
                _, _, XW, lo, hi, wlo, whi = _geom(ci)
                xt = pool.tile([P, XW * C], F32, name="xt", bufs=pf + 1)
                nc.gpsimd.dma_start(
                    _ap(xt, 0, P, wlo * C, [[C, hi - lo], [1, C]]),
                    bass.AP(tensor=x_ap.tensor, offset=lo * C,
                            ap=[[X_Z, P], [C, hi - lo], [1, C]]))
                return xt

            def emit_xpose(ci, xt):
                # xt_T[c*XW + a] = xt[a*C + c]; zero halo columns per plane.
                # The strided-read copy is split between Q7 (pool) and ACT:
                # xpose_act c-planes on ACT, the rest on Q7.
                _, _, XW, lo, hi, wlo, whi = _geom(ci)
                xT = pool.tile([P, C * XW], F32, name="xT",
                               bufs=(4 if ac < 512 else 3))
                if wlo > 0:
                    nc.gpsimd.memset(
                        _ap(xT, 0, P, 0, [[XW, C], [1, wlo]]), 0.0)
                if whi < XW:
                    nc.gpsimd.memset(
                        _ap(xT, 0, P, whi, [[XW, C], [1, XW - whi]]), 0.0)
                na = xpose_act if not pool_xpose else C
                # Q7 planes [0, C-na), ACT planes [C-na, C)
                nq = C - na if not pool_xpose else C
                w = whi - wlo
                if nq > 0:
                    nc.gpsimd.tensor_copy(
                        _ap(xT, 0, P, wlo, [[XW, nq], [1, w]]),
                        _ap(xt, 0, P, wlo * C, [[1, nq], [C, w]]))
                if not pool_xpose and na > 0:
                    nc.scalar.copy(
                        _ap(xT, 0, P, nq * XW + wlo, [[XW, na], [1, w]]),
                        _ap(xt, 0, P, wlo * C + nq, [[1, na], [C, w]]))
                if not dup_xt:
                    return xT, xT
                # packed duplicate so the k5,6 sqdiff's two read streams hit
                # different SBUF tiles (same-tile dual reads halve DVE rate)
                xU = pool.tile([P, C * XW], F32, name="xU",
                               bufs=(4 if ac < 512 else 2))
                nc.gpsimd.tensor_copy(xU[:], xT[:])
                return xT, xU

            def emit_xdn(ci, xT):
                # x_dn[c][a] = x(z-1) planes; rows z=0 (p in {0,64}) zeroed.
                _, _, XW, _, _, _, _ = _geom(ci)
                if xdn_dma:
                    xd = pool.tile([P, C * XW], F32, name="xd",
                                   bufs=(3 if ac < 512 else 2))
                    nc.sync.dma_start(
                        _ap(xd, 0, 2, 0, [[1, C * XW]], pstep=Z),
                        _ap(zt, 0, 2, 0, [[1, C * XW]]))
                    for b in range(LB):
                        nc.sync.dma_start(
                            _ap(xd, b * Z + 1, Z - 1, 0, [[1, C * XW]]),
                            _ap(xT, b * Z, Z - 1, 0, [[1, C * XW]]))
                else:
                    xd = psum.tile([P, C * XW], F32, name="xd_ps", bufs=2)
                    nc.tensor.matmul(
                        _ap(xd, 0, P, 0, [[1, C * XW]]),
                        shd_t[:], _ap(xT, 0, P, 0, [[1, C * XW]]),
                        start=True, stop=True)
                return xd

            PF = pf
            xts, xTs, xds = {}, {}, {}
            for j in range(min(PF, N)):
                xts[j] = emit_load(j)
            for j in range(min(2, N)):
                xTs[j] = emit_xpose(j, xts[j])
            xds[0] = emit_xdn(0, xTs[0][0])

            for ci in range(N):
                a0, acw, XW, lo, hi, wlo, whi = _geom(ci)
                MW = acw + 4

                if ci + PF < N:
                    xts[ci + PF] = emit_load(ci + PF)
                if ci + 2 < N:
                    xTs[ci + 2] = emit_xpose(ci + 2, xts[ci + 2])
                if ci + 1 < N:
                    xds[ci + 1] = emit_xdn(ci + 1, xTs[ci + 1][0])
                xt = xts.pop(ci)
                xT, xU = xTs.pop(ci)
                xd = xds.pop(ci)

                # ---- d^2 planes [c][k][a], bf16, fully packed streams ----
                dt = pool.tile([P, C * 7 * MW], BF16, name="dt")
                if fuse_sq:
                    # custom-dve APs: <=2 free dims -> one call per c plane,
                    # k folded in via a stride-0 (center) / stride-1 axis.
                    # k5,6's in1 reads the INTERLEAVED xt tile: both streams
                    # from xT (same tile) halve DVE rate on SBUF port
                    # conflicts; a strided 4-byte read elsewhere is cheaper.
                    for c in range(C):
                        nc.vector._custom_dve(
                            sqdiff,
                            out=_ap(dt, 0, P, c * 7 * MW, [[MW, 5], [1, MW]]),
                            in0=_ap(xT, 0, P, c * XW + 2, [[0, 5], [1, MW]]),
                            in1=_ap(xd, 0, P, c * XW, [[1, 5], [1, MW]]))
                        nc.vector._custom_dve(
                            sqdiff,
                            out=_ap(dt, 0, P, c * 7 * MW + 5 * MW,
                                    [[MW, 2], [1, MW]]),
                            in0=_ap(xT, 0, P, c * XW + 2, [[0, 2], [1, MW]]),
                            in1=_ap(xU, 0, P, c * XW, [[1, 2], [1, MW]]))
                else:
                    for k in range(7):
                        src, off = (xd, k) if k < 5 else (xT, k - 5)
                        nc.vector._custom_dve(
                            sqdiff,
                            out=_ap(dt, 0, P, k * MW, [[7 * MW, C], [1, MW]]),
                            in0=_ap(xT, 0, P, 2, [[XW, C], [1, MW]]),
                            in1=_ap(src, 0, P, off, [[XW, C], [1, MW]]))

                # ---- M[k][a] = sum_c d^2 (packed bf16 adds, 2x DVE) ----
                dts = pool.tile([P, 7 * MW], BF16, name="dts")
                M = pool.tile([P, 7 * MW], BF16, name="M")
                add_ap = lambda c: _ap(dt, 0, P, c * 7 * MW, [[1, 7 * MW]])
                nc.vector.tensor_add(dts[:], add_ap(0), add_ap(1))
                nc.vector.tensor_add(M[:], dts[:], add_ap(2))

                # ---- x^2 planes (gpsimd); the c-sum for the phantom row
                # rides the SEL matmuls below, so no s-adds at all ----
                seng = nc.gpsimd if pool_s else nc.vector
                sqx = pool.tile([P, C * XW], BF16, name="sqx")
                seng.tensor_mul(sqx[:], xT[:], xT[:])

                # ---- dz-mirror planes in j-layout: J[j][a] = M(z+1-shift)
                # [plane 4-j][a+j] = SH2 @ M + SEL @ (sum_c x^2) (phantom
                # z=64 row: J[j][a] = s(a+4), stride-0 moving j axis; the
                # c-sum is 3 accumulating SEL matmuls over sqx planes).
                # Piece width <= 512 cols and PSUM-bank-aligned outputs.
                def _jmm(mu, off, jn, m_off):
                    nc.tensor.matmul(
                        _ap(mu, 0, P, off, [[1, jn * acw]]),
                        sh_t[:],
                        _ap(M, 0, P, m_off,
                            [[-(MW - 1), jn], [1, acw]] if jn > 1
                            else [[1, acw]]),
                        start=True, stop=False)
                    for c in range(C):
                        nc.tensor.matmul(
                            _ap(mu, 0, P, off, [[1, jn * acw]]),
                            sel_t[:],
                            _ap(sqx, 0, P, c * XW + 4,
                                [[0, jn], [1, acw]] if jn > 1
                                else [[1, acw]]),
                            start=False, stop=(c == C - 1))

                # matmul piece boundaries: multiples of 512 f32 (PSUM bank)
                # that are also j-plane boundaries; acw in {128, 256} works.
                J = psum.tile([P, 5 * acw], F32, name="J_ps")
                jper = max(1, 512 // acw)
                for j0 in range(0, 5, jper):
                    jn = min(jper, 5 - j0)
                    _jmm(J, j0 * acw, jn, (4 - j0) * MW + j0)

                # ---- exps into O planes [(t*14+k)*acw + a], bf16: all
                # operands stream a-innermost (contiguous runs) ----
                O = pool.tile([P, acw * 2 * K], BF16, name="O",
                              bufs=(2 if acw >= 256 else None))
                EXP = mybir.ActivationFunctionType.Exp
                for th, sc in ((0, SC0), (1, SC1)):
                    o0 = th * K * acw
                    nc.scalar.activation(
                        _ap(O, 0, P, o0, [[acw, 7], [1, acw]]),
                        _ap(M, 0, P, 2, [[MW, 7], [1, acw]]),
                        EXP, scale=sc)
                    # a-mirrors k'=7,8 <- k=6,5 at a+1+j
                    nc.scalar.activation(
                        _ap(O, 0, P, o0 + 7 * acw, [[acw, 2], [1, acw]]),
                        _ap(M, 0, P, 6 * MW + 3, [[-(MW - 1), 2], [1, acw]]),
                        EXP, scale=sc)
                    # dz-mirrors k'=9+j read J[j][a] directly
                    nc.scalar.activation(
                        _ap(O, 0, P, o0 + 9 * acw, [[acw, 5], [1, acw]]),
                        _ap(J, 0, P, 0, [[acw, 5], [1, acw]]),
                        EXP, scale=sc)

                # ---- store: split by (t,k) plane across the three DMA
                # queues; each descriptor run is acw*2 bytes ----
                rep_i = ci // NCH
                dst_ap = o_ap if rep_i == reps - 1 else scratch_aps[rep_i]
                if store_rings == 0:
                    continue
                engs = (nc.sync, nc.scalar, nc.gpsimd)[:store_rings]
                wts = ring_wts or ((10, 10, 8) if store_rings == 3
                                   else (1,) * store_rings)
                tot = sum(wts)
                cum = [sum(wts[:r]) for r in range(store_rings + 1)]
                edges = [c * 2 * K // tot for c in cum]
                for r, eng in enumerate(engs):
                    p0, p1 = edges[r], edges[r + 1]
                    if p1 == p0:
                        continue
                    eng.dma_start(
                        bass.AP(tensor=dst_ap.tensor,
                                offset=p0 * A + a0,
                                ap=[[O_P, P], [A, p1 - p0], [1, acw]]),
                        _ap(O, 0, P, p0 * acw, [[1, (p1 - p0) * acw]]))

    nc.compile()
    return nc


class _Runner:
    """Compile once; reuse the jitted sharded executable across calls.

    Mirrors bass2jax.run_bass_via_pjrt's multi-core path, but without
    donated output buffers (the kernel writes every output element, so the
    zero "output operands" are passed once from device-resident buffers and
    reused)."""

    def __init__(self, nc=None):
        import jax
        from jax.sharding import Mesh, PartitionSpec, NamedSharding
        try:
            from jax.experimental.shard_map import shard_map
        except ImportError:
            from jax.shard_map import shard_map  # newer jax
        from concourse import bass2jax

        bass2jax.install_neuronx_cc_hook()
        if nc is None:
            nc = _build2()
        self.nc = nc

        partition_name = (nc.partition_id_tensor.name
                          if nc.partition_id_tensor else None)
        in_names, out_names, out_avals = [], [], []
        in_dtypes = {}
        for alloc in nc.m.functions[0].allocations:
            if not isinstance(alloc, mybir.MemoryLocationSet):
                continue
            name = alloc.memorylocations[0].name
            if alloc.kind == "ExternalInput":
                if name != partition_name:
                    in_names.append(name)
                    in_dtypes[name] = mybir.dt.np(alloc.dtype)
            elif alloc.kind == "ExternalOutput":
                out_names.append(name)
                out_avals.append(jax.core.ShapedArray(
                    tuple(alloc.tensor_shape), mybir.dt.np(alloc.dtype)))
        self.in_dtypes = in_dtypes
        assert set(in_names) <= {"x", "shm", "sel", "shd"}, in_names
        assert out_names == ["out"], out_names
        all_in_names = in_names + out_names
        if partition_name is not None:
            all_in_names = all_in_names + [partition_name]
        self.in_names = in_names

        def _body(*args):
            operands = list(args)
            if partition_name is not None:
                operands.append(bass2jax.partition_id_tensor())
            return tuple(bass2jax._bass_exec_p.bind(
                *operands,
                out_avals=tuple(out_avals),
                in_names=tuple(all_in_names),
                out_names=tuple(out_names),
                lowering_input_output_aliases=(),
                sim_require_finite=True,
                sim_require_nnan=True,
                nc=nc,
            ))

        devices = jax.devices()[:N_CORES]
        assert len(devices) == N_CORES
        self.mesh = Mesh(np.asarray(devices), ("core",))
        spec = PartitionSpec("core")
        rep = PartitionSpec()
        self.sharding = NamedSharding(self.mesh, spec)
        in_specs = tuple(spec if n == "x" else rep for n in in_names) + (spec,)
        self.jitted = jax.jit(shard_map(
            _body, mesh=self.mesh, in_specs=in_specs, out_specs=(spec,),
            check_rep=False))
        # device-resident constant operands, created once
        oav = out_avals[0]
        full_shape = (N_CORES * oav.shape[0],) + tuple(oav.shape[1:])
        self.out_shape, self.out_dtype = full_shape, oav.dtype
        self.zeros_dev = jax.device_put(
            np.zeros(full_shape, oav.dtype), self.sharding)
        consts = {}
        if "shm" in in_names:
            shm, sel, shd = _host_shift_mats()
            rep_sh = NamedSharding(self.mesh, rep)
            for n, arr in (("shm", shm), ("sel", sel), ("shd", shd)):
                if n in in_names:
                    consts[n] = jax.device_put(
                        arr.astype(in_dtypes[n]), rep_sh)
        self.consts = consts
        self._jax = jax

    def put(self, x: np.ndarray):
        return self._jax.device_put(
            np.ascontiguousarray(np.asarray(x, np.float32)), self.sharding)

    def run_dev(self, x_dev):
        """Execute; returns device array (not fetched)."""
        args = [x_dev if n == "x" else self.consts[n] for n in self.in_names]
        return self.jitted(*args, self.zeros_dev)[0]

    def __call__(self, x: np.ndarray) -> np.ndarray:
        return _expand_out(np.asarray(self.run_dev(self.put(x))))


_RUNNER = None


def _expand_out(o: np.ndarray) -> np.ndarray:
    """Device output -> reference layout/dtype.  The two classes of each
    theta pair share one exp value (THETA_R pairs are equal), so the device
    stores [.., 2, K, A] (theta/k-plane-major so device writes and stores
    stream contiguous runs); permute to [.., A, K, 2], broadcast to
    [.., A, K, 4], widen bf16 -> f32."""
    if o.shape[-3:] == (2, K, A):
        o32 = np.asarray(o, np.float32).transpose(0, 1, 4, 3, 2)
        o = np.broadcast_to(o32[..., None],
                            o32.shape + (2,)).reshape(o32.shape[:-1] + (4,))
    return np.ascontiguousarray(o.astype(np.float32, copy=False))


def _get_runner():
    global _RUNNER
    if _RUNNER is None:
        _RUNNER = _Runner()
    return _RUNNER


def kernel(x: np.ndarray) -> np.ndarray:
    x = np.asarray(x, dtype=np.float32)
    assert x.shape == (B, Z, A, C), x.shape
    try:
        return _get_runner()(x)
    except Exception:
        # fallback: reference-quality but slower dispatch path
        nc = _build2()
        shm, sel, _ = _host_shift_mats()
        cdt = mybir.dt.np(BF16)
        extra = {"shm": shm.astype(cdt), "sel": sel.astype(cdt)}
        in_maps = [{"x": np.ascontiguousarray(x[i * LB:(i + 1) * LB]), **extra}
                   for i in range(N_CORES)]
        res = run_bass_kernel_spmd(nc, in_maps, list(range(N_CORES)))
        return _expand_out(np.concatenate(
            [res.results[i]["out"] for i in range(N_CORES)], axis=0))



# revision 40
# speedup vs baseline: 1.3688x; 1.1594x over previous
"""BilateralFilter (SqueezeSeg mc condensing-kernel gaussians) on 8 TRN2 cores.

Reference computes, for x: [16, 64, 512, 3] (B, Z, A, C=xyz):
    nbr   = 14 spatial neighbors of each pixel in a 3x5 window (zero-padded)
    diff2 = sum_c (x - nbr)^2                           [B, Z, A, 14]
    out   = exp(-diff2 / (2 * theta_r^2))               [B, Z, A, 14, 4]
with THETA_R = [0.015, 0.015, 0.01, 0.01] (only 2 distinct values).

Active implementation: _build2 (see its docstring).  Key ideas on top of the
v1 baseline (_build, kept for A/B): the two classes of a theta pair are
bit-identical, so the device computes/stores only 28 unique bf16 values per
pixel in (theta, k)-plane layout [LB, Z, 2, K, A] (4x less ACT exp work,
4x fewer store bytes) and the host broadcasts/permutes to [B, Z, A, 14, 4]
f32; all on-chip tensors are a-innermost planes because 2-byte scattered
accesses run 2-4x slow on ACT; x is transposed once per chunk into c-planes
so the fused custom DVE sqdiff streams fully packed.

v1 strategy notes (pure batch data-parallel, 2 batches per core):
  - partitions p = b*64 + z  (128), free dim = azimuth chunks (AC wide).
  - squared differences via a runtime-registered fused custom DVE op
    (out = (in0-in1)^2), channel sums via tensor_reduce.
  - mirror symmetry: m_k(q) = |x(q) - x(q+off_k)|^2 for the 7 "negative"
    offsets k=0..6 gives the other 7 via diff2_{13-k}(q) = m_k(q - off_k);
    the z+1-partition read (engines cannot shift partitions by 1) is
    materialized on the idle TensorE as an exact 0/1 permutation matmul
    into PSUM, with the phantom z=64 boundary row (out-of-image neighbor
    => diff2 = |x(center)|^2, from s = sum_c x^2) accumulated by a second
    selector matmul. (PE_SHIFT=False falls back to partition-remap DMAs.)
  - ACT computes exp with the free scale immediate; each exp is written to
    both classes of its theta pair via a stride-0 input axis, directly into
    the interleaved [a, k, c] staging layout.
  - the staging tile matches DRAM layout exactly, so the store is one
    contiguous 128-partition DMA (28 KB/partition runs at AC=128).
"""

import numpy as np

import concourse.bass as bass
import concourse.tile as tile
from concourse import bacc, mybir
from concourse.bass_utils import run_bass_kernel_spmd

N_CORES = 8
B, Z, A, C = 16, 64, 512, 3
K, NCLS = 14, 4
LB = B // N_CORES            # local batches per core = 2
P = LB * Z                   # 128 partitions
AC = 128                     # azimuth chunk
BUFS = 3                     # tile pool buffers
PE_SHIFT = True              # z+1 partition shift via PE matmul vs SBUF DMA
XDN_PE = True                # derive x_dn on PE too (no duplicate DRAM read)
F32 = mybir.dt.float32
BF16 = mybir.dt.bfloat16


def _host_shift_mats():
    """SH2[k, m] = 1 iff k == m+1 (and not m == 63: batch boundary);
    SEL[k, m] = 1 iff k == m in {63, 127} (phantom z=64 row selector);
    SHD[k, m] = 1 iff k == m-1 (and not m in {0, 64}: z=0 rows stay 0)."""
    sh = np.zeros((P, P), np.float32)
    for m in range(P - 1):
        if m != Z - 1:
            sh[m + 1, m] = 1.0
    sel = np.zeros((P, P), np.float32)
    sel[Z - 1, Z - 1] = 1.0
    sel[P - 1, P - 1] = 1.0
    shd = np.zeros((P, P), np.float32)
    for m in range(1, P):
        if m != Z:
            shd[m - 1, m] = 1.0
    return sh, sel, shd

# exp scales: -1 / (2 * theta^2), theta pairs (0.015, 0.01), f32 semantics
_t0 = np.float32(0.015)
_t1 = np.float32(0.01)
SC0 = -float(1.0 / np.float32(np.float32(2.0) * _t0 * _t0))
SC1 = -float(1.0 / np.float32(np.float32(2.0) * _t1 * _t1))

# DRAM strides (elements) of out [LB, Z, A, K, NCLS]
O_A = K * NCLS               # 56
O_Z = A * O_A                # 28672
O_B = Z * O_Z                # 1835008
X_Z = A * C                  # 1536
X_B = Z * X_Z


def _ap(t, poff, pcnt, foff, pairs, pstep=1):
    """AP on tile t: partitions [poff, poff+pcnt) (stride pstep rows), free
    `pairs` ([step, count] in elements) based at element foff."""
    row = t.ap[0][0]
    return bass.AP(tensor=t.tensor, offset=t.offset + poff * row + foff,
                   ap=[[pstep * row, pcnt]] + [list(p) for p in pairs])


_SQDIFF = None


def _get_sqdiff():
    """Register a runtime custom DVE op: out = (in0 - in1)^2 (fp32, one
    instruction instead of subtract + multiply)."""
    global _SQDIFF
    if _SQDIFF is not None:
        return _SQDIFF
    from concourse import dve_ops
    from concourse.dve_spec import Spec, Src0, Src1, sq, lower, _has_src1
    from concourse.dve_uop import DveOpSpec

    name = "SQDIFF_BILAT_ANT"
    if name not in dve_ops._SUB_OPCODE_FOR_NAME:
        spec = Spec(
            body=sq(Src0 - Src1),
            reference=lambda in0, in1, c0, c1, c2:
                (in0.astype(np.float32) - in1.astype(np.float32)) ** 2)
        row = 1 + len(dve_ops.OPS)
        assert row < 0x20
        shas = {}
        for ver in ("v3",):
            tmp = DveOpSpec(name=name, opcode=row, uops=lower(spec, ver=ver),
                            rd1_en=_has_src1(spec))
            shas[ver] = tmp.sha(ver)
        op = dve_ops.DveOp(name, spec, subdim=False, uops_sha=shas)
        dve_ops.OPS.append(op)
        dve_ops.CUSTOM_DVE_SPECS[name] = spec
        dve_ops._SUB_OPCODE_FOR_NAME[name] = row
    else:
        op = next(o for o in dve_ops.OPS if o.name == name)
    _SQDIFF = op
    return op


def _build(ac=AC, bufs=BUFS, reps=1, pe_shift=PE_SHIFT, xdn_pe=XDN_PE,
           chunks=None, psum_bufs=3, store_rings=3, dt_mode="planes",
           amir_dve=0, sq_dve=False, ring_wts=None):
    # chunk schedule: list of (a0, width).  Uniform chunks minimize the
    # per-chunk fixed instruction overhead (~185 ns per ACT instruction,
    # 6 of them per chunk); with deep load prefetch the pipeline fill no
    # longer needs smaller leading chunks, and fill amortizes over reps.
    if chunks is None:
        chunks = [(a0, ac) for a0 in range(0, A, ac)]
    assert sum(w for _, w in chunks) == A
    NCH = len(chunks)
    nc = bacc.Bacc("TRN2", target_bir_lowering=False, debug=False,
                   num_devices=N_CORES)
    x_h = nc.dram_tensor("x", [LB, Z, A, C], F32, kind="ExternalInput")
    o_h = nc.dram_tensor("out", [LB, Z, A, K, NCLS], F32, kind="ExternalOutput")
    x_ap, o_ap = x_h.ap(), o_h.ap()
    if pe_shift:
        # 0/1 shift matrices are exact in bf16 (2x PE when M is bf16)
        CDT = F32 if dt_mode == "reduce" else BF16
        shm_h = nc.dram_tensor("shm", [P, P], CDT, kind="ExternalInput")
        sel_h = nc.dram_tensor("sel", [P, P], CDT, kind="ExternalInput")
        if xdn_pe:
            shd_h = nc.dram_tensor("shd", [P, P], F32, kind="ExternalInput")
    # bench mode: reps > 1 re-runs the whole kernel; each non-final pass
    # stores to its own DRAM scratch so stores are real traffic
    scratch_aps = [
        nc.dram_tensor(f"scr{r}", [LB, Z, A, K, NCLS], F32).ap()
        for r in range(reps - 1)]

    from contextlib import ExitStack
    with tile.TileContext(nc) as tc, ExitStack() as es:
        if pe_shift:
            consts = es.enter_context(tc.tile_pool(name="consts", bufs=1))
            psum = es.enter_context(
                tc.tile_pool(name="psum", bufs=psum_bufs, space="PSUM"))
        with tc.tile_pool(name="pool", bufs=bufs) as pool:
            if pe_shift:
                sh_t = consts.tile([P, P], CDT, name="sh_t")
                nc.sync.dma_start(sh_t[:], shm_h.ap()[:])
                sel_t = consts.tile([P, P], CDT, name="sel_t")
                nc.sync.dma_start(sel_t[:], sel_h.ap()[:])
                if xdn_pe:
                    shd_t = consts.tile([P, P], F32, name="shd_t")
                    nc.sync.dma_start(shd_t[:], shd_h.ap()[:])
            N = NCH * reps

            def _geom(ci):
                a0, ac = chunks[ci % NCH]
                XW = ac + 8          # x window (halo 4 each side)
                lo, hi = max(0, a0 - 4), min(A, a0 + ac + 4)
                c_lo = (lo - (a0 - 4)) * C          # first valid xt col
                c_hi = (hi - (a0 - 4)) * C
                return a0, ac, XW, lo, hi, c_lo, c_hi

            def emit_load(ci):
                # ---- load x window (zero halo at image borders) ----
                # (b, z) rows are contiguous in DRAM: one 128-partition DMA.
                # Loads issue on the (otherwise idle) gpsimd SWDGE so they
                # are not program-ordered behind the big store issues on SP
                # — the next chunks' loads must cut ahead of queued stores
                # or compute stalls behind them.
                # deep rotation: loads must be queued well before the big
                # stores they contend with, or they wait out a full 10 us
                # store before landing (xt is tiny: 1.6 KB/partition/buf)
                _, _, XW, lo, hi, c_lo, c_hi = _geom(ci)
                xt = pool.tile([P, XW * C], F32, name="xt", bufs=8)
                if c_lo > 0:
                    nc.gpsimd.memset(_ap(xt, 0, P, 0, [[1, c_lo]]), 0.0)
                if c_hi < XW * C:
                    nc.gpsimd.memset(
                        _ap(xt, 0, P, c_hi, [[1, XW * C - c_hi]]), 0.0)
                nc.gpsimd.dma_start(
                    _ap(xt, 0, P, c_lo, [[C, hi - lo], [1, C]]),
                    bass.AP(tensor=x_ap.tensor, offset=lo * C,
                            ap=[[X_Z, P], [C, hi - lo], [1, C]]))
                return xt

            def emit_xdn(ci, xt):
                # ---- x_dn[p] = x at (z-1) (zeros at z=0 rows): exact PE
                # permutation shift of xt into PSUM; the zero columns of SHD
                # give the z=0 rows (and the xt halo the image-border zeros)
                # for free.  Emitted one chunk AHEAD of the consuming chunk:
                # the PE is in-order, so x_dn(i+1) must precede M_up(i) or
                # the serial loop DVE(i) -> M_up(i) -> x_dn(i+1) -> DVE(i+1)
                # paces the pipeline above the store rate.  bufs=2 so the
                # psum pool fits 8 banks (M_up 3x2 + x_dn 2x1).
                _, _, XW, _, _, _, _ = _geom(ci)
                x_dn = psum.tile([P, XW * C], F32, name="x_dn_ps", bufs=2)
                for n0 in range(0, XW * C, 512):
                    n1 = min(XW * C, n0 + 512)
                    nc.tensor.matmul(
                        _ap(x_dn, 0, P, n0, [[1, n1 - n0]]),
                        shd_t[:], _ap(xt, 0, P, n0, [[1, n1 - n0]]),
                        start=True, stop=True)
                return x_dn

            PF = 7               # load prefetch distance (chunks ahead)
            xts, xdns = {}, {}
            for j in range(min(PF, N)):
                xts[j] = emit_load(j)
            if pe_shift and xdn_pe:
                xdns[0] = emit_xdn(0, xts[0])

            for ci in range(N):
                a0, ac, XW, lo, hi, c_lo, c_hi = _geom(ci)
                MW = ac + 4          # m window (halo 2 each side)

                if ci + PF < N:
                    xts[ci + PF] = emit_load(ci + PF)
                if pe_shift and xdn_pe and ci + 1 < N:
                    xdns[ci + 1] = emit_xdn(ci + 1, xts[ci + 1])
                xt = xts.pop(ci)

                if pe_shift and xdn_pe:
                    x_dn = xdns.pop(ci)
                else:
                    x_dn = pool.tile([P, XW * C], F32, name="x_dn")
                    nc.gpsimd.memset(x_dn[:], 0.0)
                    for b in range(LB):
                        nc.gpsimd.dma_start(
                            _ap(x_dn, b * Z + 1, Z - 1, c_lo,
                                [[C, hi - lo], [1, C]]),
                            bass.AP(tensor=x_ap.tensor, offset=b * X_B + lo * C,
                                    ap=[[X_Z, Z - 1], [C, hi - lo], [1, C]]))

                # ---- s = sum_c x^2 ; m_k maps over a-window [a0-2, ...)
                # k=0..4: dz=-1, da=k-2 ; k=5,6: dz=0, da=k-7
                # d2 = (x - x_nbr)^2 in one fused custom op per k.
                # dt_mode picks how the c-sum is done:
                #  "reduce": f32 interleaved + TensorReduce (no fast mode)
                #  "iadds":  bf16 interleaved (packed writes) + 2 stride-3
                #            tensor_adds — each add processes N/3 elements,
                #            beating the reduce's full-N stream
                #  "planes": custom writes c-outer packed bf16 planes; the
                #            adds are fully packed and hit the DVE 2x mode
                # bf16 rounds only d^2 / s / M (<=0.4% rel => ~2e-3 max abs
                # on the exp output, far inside the 2e-2 tolerance).
                sqdiff = _get_sqdiff()
                MDT = F32 if dt_mode == "reduce" else BF16
                M = pool.tile([P, 7 * MW], MDT, name="M")
                if dt_mode == "reduce":
                    sqx = pool.tile([P, XW * C], F32, name="sqx")
                    nc.scalar.square(sqx[:], xt[:])
                    st = pool.tile([P, XW], F32, name="st")
                    nc.vector.tensor_reduce(
                        st[:], _ap(sqx, 0, P, 0, [[C, XW], [1, C]]),
                        axis=mybir.AxisListType.X, op=mybir.AluOpType.add)
                    dt5 = pool.tile([P, 5 * MW * C], F32, name="dt5")
                    for k in range(5):
                        nc.vector._custom_dve(
                            sqdiff,
                            out=_ap(dt5, 0, P, k * MW * C, [[C, MW], [1, C]]),
                            in0=_ap(xt, 0, P, 2 * C, [[C, MW], [1, C]]),
                            in1=_ap(x_dn, 0, P, k * C, [[C, MW], [1, C]]))
                    nc.vector.tensor_reduce(
                        _ap(M, 0, P, 0, [[1, 5 * MW]]),
                        _ap(dt5, 0, P, 0, [[C, 5 * MW], [1, C]]),
                        axis=mybir.AxisListType.X, op=mybir.AluOpType.add)
                    dt2 = pool.tile([P, 2 * MW * C], F32, name="dt2")
                    for k in (5, 6):
                        nc.vector._custom_dve(
                            sqdiff,
                            out=_ap(dt2, 0, P, (k - 5) * MW * C,
                                    [[C, MW], [1, C]]),
                            in0=_ap(xt, 0, P, 2 * C, [[C, MW], [1, C]]),
                            in1=_ap(xt, 0, P, (k - 5) * C, [[C, MW], [1, C]]))
                    nc.vector.tensor_reduce(
                        _ap(M, 0, P, 5 * MW, [[1, 2 * MW]]),
                        _ap(dt2, 0, P, 0, [[C, 2 * MW], [1, C]]),
                        axis=mybir.AxisListType.X, op=mybir.AluOpType.add)
                else:
                    # custom-dve APs are rank<=3: one call per map k
                    dt = pool.tile([P, 3 * 7 * MW], BF16, name="dt")
                    if dt_mode == "planes":
                        # c-outer stream: strided f32 reads, PACKED bf16
                        # plane writes (scattered 2-byte writes would RMW)
                        d_out = lambda k: _ap(dt, 0, P, k * MW,
                                              [[7 * MW, C], [1, MW]])
                        d_in = lambda t, off: _ap(t, 0, P, off,
                                                  [[1, C], [C, MW]])
                        add_ap = lambda c: _ap(dt, 0, P, c * 7 * MW,
                                               [[1, 7 * MW]])
                    else:  # iadds: natural interleaved stream, packed writes
                        d_out = lambda k: _ap(dt, 0, P, 3 * k * MW,
                                              [[C, MW], [1, C]])
                        d_in = lambda t, off: _ap(t, 0, P, off,
                                                  [[C, MW], [1, C]])
                        add_ap = lambda c: _ap(dt, 0, P, c,
                                               [[C, 7 * MW]])
                    for k in range(7):
                        src, off = (x_dn, k * C) if k < 5 else (xt, (k - 5) * C)
                        nc.vector._custom_dve(
                            sqdiff, out=d_out(k),
                            in0=d_in(xt, 2 * C), in1=d_in(src, off))
                    dts = pool.tile([P, 7 * MW], BF16, name="dts")
                    nc.vector.tensor_add(dts[:], add_ap(0), add_ap(1))
                    nc.vector.tensor_add(M[:], dts[:], add_ap(2))

                    # s = sum_c x^2 via the same layout trick
                    sqx = pool.tile([P, 3 * XW], BF16, name="sqx")
                    if dt_mode == "planes":
                        sq_out = _ap(sqx, 0, P, 0, [[XW, C], [1, XW]])
                        sq_in = _ap(xt, 0, P, 0, [[1, C], [C, XW]])
                        s_ap = lambda c: _ap(sqx, 0, P, c * XW, [[1, XW]])
                    else:
                        sq_out, sq_in = sqx[:], xt[:]
                        s_ap = lambda c: _ap(sqx, 0, P, c, [[C, XW]])
                    if sq_dve:
                        nc.vector.tensor_mul(sq_out, sq_in, sq_in)
                    else:
                        nc.scalar.square(sq_out, sq_in)
                    stt = pool.tile([P, XW], BF16, name="stt")
                    st = pool.tile([P, XW], BF16, name="st")
                    nc.vector.tensor_add(stt[:], s_ap(0), s_ap(1))
                    nc.vector.tensor_add(st[:], stt[:], s_ap(2))

                # ---- M_up[p] = M[p+1] for k=0..4 cols; phantom z=64 rows
                # ({63,127}) = s(z=63 row) with k-dependent a-shift ----
                if pe_shift:
                    # PE permutation matmul: M_up = SH2^T.T @ M + SEL.T @ SD
                    # (exact for 0/1 matrices, also in bf16); phantom rows
                    # ride the second accumulating matmul through SD.  In
                    # planes mode everything is bf16 => 2x PE rate and a 4x
                    # TensorCopy for SD.
                    SD = pool.tile([P, 5 * MW], MDT, name="SD")
                    nc.vector.tensor_copy(
                        _ap(SD, 0, P, 0, [[MW, 5], [1, MW]]),
                        _ap(st, 0, P, 0, [[1, 5], [1, MW]]))
                    M_up = psum.tile([P, 5 * MW], F32, name="M_up_ps")
                    for n0 in range(0, 5 * MW, 512):
                        n1 = min(5 * MW, n0 + 512)
                        nc.tensor.matmul(
                            _ap(M_up, 0, P, n0, [[1, n1 - n0]]),
                            sh_t[:], _ap(M, 0, P, n0, [[1, n1 - n0]]),
                            start=True, stop=False)
                        nc.tensor.matmul(
                            _ap(M_up, 0, P, n0, [[1, n1 - n0]]),
                            sel_t[:], _ap(SD, 0, P, n0, [[1, n1 - n0]]),
                            start=False, stop=True)
                else:
                    M_up = pool.tile([P, 5 * MW], F32, name="M_up")
                    # disjoint remaps per batch so the phantom DMA runs parallel
                    for b in range(LB):
                        nc.sync.dma_start(
                            _ap(M_up, b * Z, Z - 1, 0, [[1, 5 * MW]]),
                            _ap(M, b * Z + 1, Z - 1, 0, [[1, 5 * MW]]))
                    # phantom: M_up[{63,127}, k*MW + ar] = st[{63,127}, ar + k]
                    nc.sync.dma_start(
                        _ap(M_up, Z - 1, 2, 0, [[MW, 5], [1, MW]], pstep=Z),
                        _ap(st, Z - 1, 2, 0, [[1, 5], [1, MW]], pstep=Z))

                # ---- exps into O staging [p, ar*56 + k*4 + c] ----
                # amir_dve: the a-mirror slots k'=7,8 duplicate the direct
                # k=6,5 exps at shifted a — a same-partition DVE copy
                # (rebalances element writes from the bottleneck ACT onto
                # DVE), with a 2-column ACT patch at the chunk edge where
                # the copy source falls outside this O tile.
                O = pool.tile([P, ac * O_A], F32, name="O",
                              bufs=(1 if ac >= 512 else
                                    2 if ac >= 256 else None))
                EXP = mybir.ActivationFunctionType.Exp
                for th, sc in ((0, SC0), (1, SC1)):
                    co = 2 * th
                    # direct k=0..6: in M[p, k*MW + ar + 2]
                    nc.scalar.activation(
                        _ap(O, 0, P, co, [[4, 7], [O_A, ac], [1, 2]]),
                        _ap(M, 0, P, 2, [[MW, 7], [1, ac], [0, 2]]),
                        EXP, scale=sc)
                    if amir_dve:
                        # boundary patch: k'=7,8 at a in {ac-2, ac-1}
                        nc.scalar.activation(
                            _ap(O, 0, P, 28 + co + (ac - 2) * O_A,
                                [[4, 2], [O_A, 2], [1, 2]]),
                            _ap(M, 0, P, 6 * MW + 3 + (ac - 2),
                                [[-(MW - 1), 2], [1, 2], [0, 2]]),
                            EXP, scale=sc)
                    else:
                        # a-mirrors k'=7,8 <- k=6,5: col = k*MW + ar + (9-k)
                        nc.scalar.activation(
                            _ap(O, 0, P, 28 + co, [[4, 2], [O_A, ac], [1, 2]]),
                            _ap(M, 0, P, 6 * MW + 3,
                                [[-(MW - 1), 2], [1, ac], [0, 2]]),
                            EXP, scale=sc)
                    # dz-mirrors k'=9..13 <- k=4..0: M_up[p, k*MW + ar + 4-k]
                    # (partition-shifted values, not copyable within a lane)
                    nc.scalar.activation(
                        _ap(O, 0, P, 36 + co, [[4, 5], [O_A, ac], [1, 2]]),
                        _ap(M_up, 0, P, 4 * (MW - 1) + 4,
                            [[-(MW - 1), 5], [1, ac], [0, 2]]),
                        EXP, scale=sc)
                if amir_dve:
                    # O[a, 7+j, c] = O[a+1+j, 6-j, c] for a < ac-2, all c
                    nc.vector.tensor_copy(
                        _ap(O, 0, P, 28, [[4, 2], [O_A, ac - 2], [1, 4]]),
                        _ap(O, 0, P, 80, [[52, 2], [O_A, ac - 2], [1, 4]]))

                # ---- store: contiguous 128-partition DMAs.  Each HWDGE
                # ring sustains ~418 GB/s independently (SP + ACT pair
                # ~800 aggregate), so the store of EVERY chunk is split
                # into equal per-ring pieces — alternating whole chunks
                # leaves one ring with 62.5% of the bytes and that ring's
                # 21.9 us/pass becomes the kernel's floor.  store_rings=3
                # adds the gpsimd SWDGE ring. ----
                rep_i = ci // NCH
                dst_ap = o_ap if rep_i == reps - 1 else scratch_aps[rep_i]
                if store_rings == 0:   # timing probe: no output stores
                    continue
                engs = (nc.sync, nc.scalar, nc.gpsimd)[:store_rings]
                wts = ring_wts or (1,) * store_rings
                tot = sum(wts)
                cum = [sum(wts[:r]) for r in range(store_rings + 1)]
                edges = [c * ac // tot for c in cum]
                for r, eng in enumerate(engs):
                    w0, w1 = edges[r], edges[r + 1]
                    eng.dma_start(
                        bass.AP(tensor=dst_ap.tensor,
                                offset=(a0 + w0) * O_A,
                                ap=[[O_Z, P], [1, (w1 - w0) * O_A]]),
                        _ap(O, 0, P, w0 * O_A, [[1, (w1 - w0) * O_A]]))

    nc.compile()
    return nc


def _build2(ac=512, bufs=2, reps=1, chunks=None, psum_bufs=1, store_rings=3,
            fuse_sq=True, pool_s=True, xdn_dma=True, pool_xpose=False,
            xpose_act=1, pf=3, dup_xt=False, ring_wts=None):
    """v2: dedup bf16 output [LB, Z, 2, K, A] -- classes within a theta pair
    are bit-identical (host broadcasts 2 -> 4 classes and permutes axes).
    Everything on chip lives in PLANE layout ([t*14+k][a], a innermost):
    every engine access streams long contiguous runs -- 2-byte scattered
    accesses on ACT cost 2-4x.  x is transposed once per chunk into c-planes
    (split Q7/ACT) so the fused per-c DVE sqdiff streams fully packed; sq on
    gpsimd, s-adds on DVE; dz-mirror planes built on PE in j-layout (one
    PSUM J tile, matmul pieces at bank boundaries, phantom z=64 row via a
    stride-0 moving axis over st); x_dn via SBUF partition-remap DMAs;
    stores split by plane across the three DMA queues (512B runs)."""
    if chunks is None:
        chunks = [(a0, ac) for a0 in range(0, A, ac)]
    assert sum(w for _, w in chunks) == A
    NCH = len(chunks)
    O_T = K * A                  # out plane-group stride (t axis)
    O_P = 2 * K * A              # out partition stride (z axis)
    nc = bacc.Bacc("TRN2", target_bir_lowering=False, debug=False,
                   num_devices=N_CORES)
    x_h = nc.dram_tensor("x", [LB, Z, A, C], F32, kind="ExternalInput")
    o_h = nc.dram_tensor("out", [LB, Z, 2, K, A], BF16, kind="ExternalOutput")
    x_ap, o_ap = x_h.ap(), o_h.ap()
    shm_h = nc.dram_tensor("shm", [P, P], BF16, kind="ExternalInput")
    sel_h = nc.dram_tensor("sel", [P, P], BF16, kind="ExternalInput")
    shd_h = None
    if not xdn_dma:
        shd_h = nc.dram_tensor("shd", [P, P], F32, kind="ExternalInput")
    scratch_aps = [
        nc.dram_tensor(f"scr{r}", [LB, Z, 2, K, A], BF16).ap()
        for r in range(reps - 1)]

    sqdiff = _get_sqdiff()
    from contextlib import ExitStack
    with tile.TileContext(nc) as tc, ExitStack() as es:
        consts = es.enter_context(tc.tile_pool(name="consts", bufs=1))
        psum = es.enter_context(
            tc.tile_pool(name="psum", bufs=psum_bufs, space="PSUM"))
        with tc.tile_pool(name="pool", bufs=bufs) as pool:
            sh_t = consts.tile([P, P], BF16, name="sh_t")
            nc.sync.dma_start(sh_t[:], shm_h.ap()[:])
            sel_t = consts.tile([P, P], BF16, name="sel_t")
            nc.sync.dma_start(sel_t[:], sel_h.ap()[:])
            if xdn_dma:
                # zero source rows for x_dn partitions {0, 64} (z=0 rows)
                zt = consts.tile([P, 3 * (ac + 8)], F32, name="zt")
                nc.gpsimd.memset(zt[:], 0.0)
            else:
                shd_t = consts.tile([P, P], F32, name="shd_t")
                nc.sync.dma_start(shd_t[:], shd_h.ap()[:])
            N = NCH * reps

            def _geom(ci):
                a0, acw = chunks[ci % NCH]
                XW = acw + 8
                lo, hi = max(0, a0 - 4), min(A, a0 + acw + 4)
                wlo = lo - (a0 - 4)       # first valid a in window coords
                whi = hi - (a0 - 4)
                return a0, acw, XW, lo, hi, wlo, whi

            def emit_load(ci):
                # interleaved [a, c] x window; halo memsets happen on the
                # transposed tile, so only the DMA here.
                _, _, XW, lo, hi, wlo, whi = _geom(ci)
                xt = pool.tile([P, XW * C], F32, name="xt", bufs=pf + 1)
                nc.gpsimd.dma_start(
                    _ap(xt, 0, P, wlo * C, [[C, hi - lo], [1, C]]),
                    bass.AP(tensor=x_ap.tensor, offset=lo * C,
                            ap=[[X_Z, P], [C, hi - lo], [1, C]]))
                return xt

            def emit_xpose(ci, xt):
                # xt_T[c*XW + a] = xt[a*C + c]; zero halo columns per plane.
                # The strided-read copy is split between Q7 (pool) and ACT:
                # xpose_act c-planes on ACT, the rest on Q7.
                _, _, XW, lo, hi, wlo, whi = _geom(ci)
                xT = pool.tile([P, C * XW], F32, name="xT",
                               bufs=(4 if ac < 512 else 3))
                if wlo > 0:
                    nc.gpsimd.memset(
                        _ap(xT, 0, P, 0, [[XW, C], [1, wlo]]), 0.0)
                if whi < XW:
                    nc.gpsimd.memset(
                        _ap(xT, 0, P, whi, [[XW, C], [1, XW - whi]]), 0.0)
                na = xpose_act if not pool_xpose else C
                # Q7 planes [0, C-na), ACT planes [C-na, C)
                nq = C - na if not pool_xpose else C
                w = whi - wlo
                if nq > 0:
                    nc.gpsimd.tensor_copy(
                        _ap(xT, 0, P, wlo, [[XW, nq], [1, w]]),
                        _ap(xt, 0, P, wlo * C, [[1, nq], [C, w]]))
                if not pool_xpose and na > 0:
                    nc.scalar.copy(
                        _ap(xT, 0, P, nq * XW + wlo, [[XW, na], [1, w]]),
                        _ap(xt, 0, P, wlo * C + nq, [[1, na], [C, w]]))
                if not dup_xt:
                    return xT, xT
                # packed duplicate so the k5,6 sqdiff's two read streams hit
                # different SBUF tiles (same-tile dual reads halve DVE rate)
                xU = pool.tile([P, C * XW], F32, name="xU",
                               bufs=(4 if ac < 512 else 2))
                nc.gpsimd.tensor_copy(xU[:], xT[:])
                return xT, xU

            def emit_xdn(ci, xT):
                # x_dn[c][a] = x(z-1) planes; rows z=0 (p in {0,64}) zeroed.
                _, _, XW, _, _, _, _ = _geom(ci)
                if xdn_dma:
                    xd = pool.tile([P, C * XW], F32, name="xd",
                                   bufs=(3 if ac < 512 else 2))
                    nc.sync.dma_start(
                        _ap(xd, 0, 2, 0, [[1, C * XW]], pstep=Z),
                        _ap(zt, 0, 2, 0, [[1, C * XW]]))
                    for b in range(LB):
                        nc.sync.dma_start(
                            _ap(xd, b * Z + 1, Z - 1, 0, [[1, C * XW]]),
                            _ap(xT, b * Z, Z - 1, 0, [[1, C * XW]]))
                else:
                    xd = psum.tile([P, C * XW], F32, name="xd_ps", bufs=2)
                    nc.tensor.matmul(
                        _ap(xd, 0, P, 0, [[1, C * XW]]),
                        shd_t[:], _ap(xT, 0, P, 0, [[1, C * XW]]),
                        start=True, stop=True)
                return xd

            PF = pf
            xts, xTs, xds = {}, {}, {}
            for j in range(min(PF, N)):
                xts[j] = emit_load(j)
            for j in range(min(2, N)):
                xTs[j] = emit_xpose(j, xts[j])
            xds[0] = emit_xdn(0, xTs[0][0])

            for ci in range(N):
                a0, acw, XW, lo, hi, wlo, whi = _geom(ci)
                MW = acw + 4

                if ci + PF < N:
                    xts[ci + PF] = emit_load(ci + PF)
                if ci + 2 < N:
                    xTs[ci + 2] = emit_xpose(ci + 2, xts[ci + 2])
                if ci + 1 < N:
                    xds[ci + 1] = emit_xdn(ci + 1, xTs[ci + 1][0])
                xt = xts.pop(ci)
                xT, xU = xTs.pop(ci)
                xd = xds.pop(ci)

                # ---- d^2 planes [c][k][a], bf16, fully packed streams ----
                dt = pool.tile([P, C * 7 * MW], BF16, name="dt")
                if fuse_sq:
                    # custom-dve APs: <=2 free dims -> one call per c plane,
                    # k folded in via a stride-0 (center) / stride-1 axis.
                    # k5,6's in1 reads the INTERLEAVED xt tile: both streams
                    # from xT (same tile) halve DVE rate on SBUF port
                    # conflicts; a strided 4-byte read elsewhere is cheaper.
                    for c in range(C):
                        nc.vector._custom_dve(
                            sqdiff,
                            out=_ap(dt, 0, P, c * 7 * MW, [[MW, 5], [1, MW]]),
                            in0=_ap(xT, 0, P, c * XW + 2, [[0, 5], [1, MW]]),
                            in1=_ap(xd, 0, P, c * XW, [[1, 5], [1, MW]]))
                        nc.vector._custom_dve(
                            sqdiff,
                            out=_ap(dt, 0, P, c * 7 * MW + 5 * MW,
                                    [[MW, 2], [1, MW]]),
                            in0=_ap(xT, 0, P, c * XW + 2, [[0, 2], [1, MW]]),
                            in1=_ap(xU, 0, P, c * XW, [[1, 2], [1, MW]]))
                else:
                    for k in range(7):
                        src, off = (xd, k) if k < 5 else (xT, k - 5)
                        nc.vector._custom_dve(
                            sqdiff,
                            out=_ap(dt, 0, P, k * MW, [[7 * MW, C], [1, MW]]),
                            in0=_ap(xT, 0, P, 2, [[XW, C], [1, MW]]),
                            in1=_ap(src, 0, P, off, [[XW, C], [1, MW]]))

                # ---- M[k][a] = sum_c d^2 (packed bf16 adds, 2x DVE) ----
                dts = pool.tile([P, 7 * MW], BF16, name="dts")
                M = pool.tile([P, 7 * MW], BF16, name="M")
                add_ap = lambda c: _ap(dt, 0, P, c * 7 * MW, [[1, 7 * MW]])
                nc.vector.tensor_add(dts[:], add_ap(0), add_ap(1))
                nc.vector.tensor_add(M[:], dts[:], add_ap(2))

                # ---- x^2 planes (gpsimd); the c-sum for the phantom row
                # rides the SEL matmuls below, so no s-adds at all ----
                seng = nc.gpsimd if pool_s else nc.vector
                sqx = pool.tile([P, C * XW], BF16, name="sqx")
                seng.tensor_mul(sqx[:], xT[:], xT[:])

                # ---- dz-mirror planes in j-layout: J[j][a] = M(z+1-shift)
                # [plane 4-j][a+j] = SH2 @ M + SEL @ (sum_c x^2) (phantom
                # z=64 row: J[j][a] = s(a+4), stride-0 moving j axis; the
                # c-sum is 3 accumulating SEL matmuls over sqx planes).
                # Piece width <= 512 cols and PSUM-bank-aligned outputs.
                def _jmm(mu, off, jn, m_off):
                    nc.tensor.matmul(
                        _ap(mu, 0, P, off, [[1, jn * acw]]),
                        sh_t[:],
                        _ap(M, 0, P, m_off,
                            [[-(MW - 1), jn], [1, acw]] if jn > 1
                            else [[1, acw]]),
                        start=True, stop=False)
                    for c in range(C):
                        nc.tensor.matmul(
                            _ap(mu, 0, P, off, [[1, jn * acw]]),
                            sel_t[:],
                            _ap(sqx, 0, P, c * XW + 4,
                                [[0, jn], [1, acw]] if jn > 1
                                else [[1, acw]]),
                            start=False, stop=(c == C - 1))

                # matmul piece boundaries: multiples of 512 f32 (PSUM bank)
                # that are also j-plane boundaries; acw in {128, 256} works.
                J = psum.tile([P, 5 * acw], F32, name="J_ps")
                jper = max(1, 512 // acw)
                for j0 in range(0, 5, jper):
                    jn = min(jper, 5 - j0)
                    _jmm(J, j0 * acw, jn, (4 - j0) * MW + j0)

                # ---- exps into O planes [(t*14+k)*acw + a], bf16: all
                # operands stream a-innermost (contiguous runs) ----
                O = pool.tile([P, acw * 2 * K], BF16, name="O",
                              bufs=(2 if acw >= 256 else None))
                EXP = mybir.ActivationFunctionType.Exp
                for th, sc in ((0, SC0), (1, SC1)):
                    o0 = th * K * acw
                    nc.scalar.activation(
                        _ap(O, 0, P, o0, [[acw, 7], [1, acw]]),
                        _ap(M, 0, P, 2, [[MW, 7], [1, acw]]),
                        EXP, scale=sc)
                    # a-mirrors k'=7,8 <- k=6,5 at a+1+j
                    nc.scalar.activation(
                        _ap(O, 0, P, o0 + 7 * acw, [[acw, 2], [1, acw]]),
                        _ap(M, 0, P, 6 * MW + 3, [[-(MW - 1), 2], [1, acw]]),
                        EXP, scale=sc)
                    # dz-mirrors k'=9+j read J[j][a] directly
                    nc.scalar.activation(
                        _ap(O, 0, P, o0 + 9 * acw, [[acw, 5], [1, acw]]),
                        _ap(J, 0, P, 0, [[acw, 5], [1, acw]]),
                        EXP, scale=sc)

                # ---- store: split by (t,k) plane across the three DMA
                # queues; each descriptor run is acw*2 bytes ----
                rep_i = ci // NCH
                dst_ap = o_ap if rep_i == reps - 1 else scratch_aps[rep_i]
                if store_rings == 0:
                    continue
                engs = (nc.sync, nc.scalar, nc.gpsimd)[:store_rings]
                # sync also carries the x_dn remaps -> fewer store planes
                wts = ring_wts or ((8, 12, 8) if store_rings == 3
                                   else (1,) * store_rings)
                tot = sum(wts)
                cum = [sum(wts[:r]) for r in range(store_rings + 1)]
                edges = [c * 2 * K // tot for c in cum]
                for r, eng in enumerate(engs):
                    p0, p1 = edges[r], edges[r + 1]
                    if p1 == p0:
                        continue
                    eng.dma_start(
                        bass.AP(tensor=dst_ap.tensor,
                                offset=p0 * A + a0,
                                ap=[[O_P, P], [A, p1 - p0], [1, acw]]),
                        _ap(O, 0, P, p0 * acw, [[1, (p1 - p0) * acw]]))

    nc.compile()
    return nc


class _Runner:
    """Compile once; reuse the jitted sharded executable across calls.

    Mirrors bass2jax.run_bass_via_pjrt's multi-core path, but without
    donated output buffers (the kernel writes every output element, so the
    zero "output operands" are passed once from device-resident buffers and
    reused)."""

    def __init__(self, nc=None):
        import jax
        from jax.sharding import Mesh, PartitionSpec, NamedSharding
        try:
            from jax.experimental.shard_map import shard_map
        except ImportError:
            from jax.shard_map import shard_map  # newer jax
        from concourse import bass2jax

        bass2jax.install_neuronx_cc_hook()
        if nc is None:
            nc = _build2()
        self.nc = nc

        partition_name = (nc.partition_id_tensor.name
                          if nc.partition_id_tensor else None)
        in_names, out_names, out_avals = [], [], []
        in_dtypes = {}
        for alloc in nc.m.functions[0].allocations:
            if not isinstance(alloc, mybir.MemoryLocationSet):
                continue
            name = alloc.memorylocations[0].name
            if alloc.kind == "ExternalInput":
                if name != partition_name:
                    in_names.append(name)
                    in_dtypes[name] = mybir.dt.np(alloc.dtype)
            elif alloc.kind == "ExternalOutput":
                out_names.append(name)
                out_avals.append(jax.core.ShapedArray(
                    tuple(alloc.tensor_shape), mybir.dt.np(alloc.dtype)))
        self.in_dtypes = in_dtypes
        assert set(in_names) <= {"x", "shm", "sel", "shd"}, in_names
        assert out_names == ["out"], out_names
        all_in_names = in_names + out_names
        if partition_name is not None:
            all_in_names = all_in_names + [partition_name]
        self.in_names = in_names

        def _body(*args):
            operands = list(args)
            if partition_name is not None:
                operands.append(bass2jax.partition_id_tensor())
            return tuple(bass2jax._bass_exec_p.bind(
                *operands,
                out_avals=tuple(out_avals),
                in_names=tuple(all_in_names),
                out_names=tuple(out_names),
                lowering_input_output_aliases=(),
                sim_require_finite=True,
                sim_require_nnan=True,
                nc=nc,
            ))

        devices = jax.devices()[:N_CORES]
        assert len(devices) == N_CORES
        self.mesh = Mesh(np.asarray(devices), ("core",))
        spec = PartitionSpec("core")
        rep = PartitionSpec()
        self.sharding = NamedSharding(self.mesh, spec)
        in_specs = tuple(spec if n == "x" else rep for n in in_names) + (spec,)
        self.jitted = jax.jit(shard_map(
            _body, mesh=self.mesh, in_specs=in_specs, out_specs=(spec,),
            check_rep=False))
        # device-resident constant operands, created once
        oav = out_avals[0]
        full_shape = (N_CORES * oav.shape[0],) + tuple(oav.shape[1:])
        self.out_shape, self.out_dtype = full_shape, oav.dtype
        self.zeros_dev = jax.device_put(
            np.zeros(full_shape, oav.dtype), self.sharding)
        consts = {}
        if "shm" in in_names:
            shm, sel, shd = _host_shift_mats()
            rep_sh = NamedSharding(self.mesh, rep)
            for n, arr in (("shm", shm), ("sel", sel), ("shd", shd)):
                if n in in_names:
                    consts[n] = jax.device_put(
                        arr.astype(in_dtypes[n]), rep_sh)
        self.consts = consts
        self._jax = jax

    def put(self, x: np.ndarray):
        return self._jax.device_put(
            np.ascontiguousarray(np.asarray(x, np.float32)), self.sharding)

    def run_dev(self, x_dev):
        """Execute; returns device array (not fetched)."""
        args = [x_dev if n == "x" else self.consts[n] for n in self.in_names]
        return self.jitted(*args, self.zeros_dev)[0]

    def __call__(self, x: np.ndarray) -> np.ndarray:
        return _expand_out(np.asarray(self.run_dev(self.put(x))))


_RUNNER = None


def _expand_out(o: np.ndarray) -> np.ndarray:
    """Device output -> reference layout/dtype.  The two classes of each
    theta pair share one exp value (THETA_R pairs are equal), so the device
    stores [.., 2, K, A] (theta/k-plane-major so device writes and stores
    stream contiguous runs); permute to [.., A, K, 2], broadcast to
    [.., A, K, 4], widen bf16 -> f32."""
    if o.shape[-3:] == (2, K, A):
        o32 = np.asarray(o, np.float32).transpose(0, 1, 4, 3, 2)
        o = np.broadcast_to(o32[..., None],
                            o32.shape + (2,)).reshape(o32.shape[:-1] + (4,))
    return np.ascontiguousarray(o.astype(np.float32, copy=False))


def _get_runner():
    global _RUNNER
    if _RUNNER is None:
        _RUNNER = _Runner()
    return _RUNNER


def kernel(x: np.ndarray) -> np.ndarray:
    x = np.asarray(x, dtype=np.float32)
    assert x.shape == (B, Z, A, C), x.shape
    try:
        return _get_runner()(x)
    except Exception:
        # fallback: reference-quality but slower dispatch path
        nc = _build2()
        shm, sel, _ = _host_shift_mats()
        cdt = mybir.dt.np(BF16)
        extra = {"shm": shm.astype(cdt), "sel": sel.astype(cdt)}
        in_maps = [{"x": np.ascontiguousarray(x[i * LB:(i + 1) * LB]), **extra}
                   for i in range(N_CORES)]
        res = run_bass_kernel_spmd(nc, in_maps, list(range(N_CORES)))
        return _expand_out(np.concatenate(
            [res.results[i]["out"] for i in range(N_CORES)], axis=0))



# revision 45
# speedup vs baseline: 1.3875x; 1.0137x over previous
"""BilateralFilter (SqueezeSeg mc condensing-kernel gaussians) on 8 TRN2 cores.

Reference computes, for x: [16, 64, 512, 3] (B, Z, A, C=xyz):
    nbr   = 14 spatial neighbors of each pixel in a 3x5 window (zero-padded)
    diff2 = sum_c (x - nbr)^2                           [B, Z, A, 14]
    out   = exp(-diff2 / (2 * theta_r^2))               [B, Z, A, 14, 4]
with THETA_R = [0.015, 0.015, 0.01, 0.01] (only 2 distinct values).

Active implementation: _build2 (see its docstring).  Key ideas on top of the
v1 baseline (_build, kept for A/B): the two classes of a theta pair are
bit-identical, so the device computes/stores only 28 unique bf16 values per
pixel in (theta, k)-plane layout [LB, Z, 2, K, A] (4x less ACT exp work,
4x fewer store bytes) and the host broadcasts/permutes to [B, Z, A, 14, 4]
f32; all on-chip tensors are a-innermost planes because 2-byte scattered
accesses run 2-4x slow on ACT; x is transposed once per chunk into c-planes
so the fused custom DVE sqdiff streams fully packed.

v1 strategy notes (pure batch data-parallel, 2 batches per core):
  - partitions p = b*64 + z  (128), free dim = azimuth chunks (AC wide).
  - squared differences via a runtime-registered fused custom DVE op
    (out = (in0-in1)^2), channel sums via tensor_reduce.
  - mirror symmetry: m_k(q) = |x(q) - x(q+off_k)|^2 for the 7 "negative"
    offsets k=0..6 gives the other 7 via diff2_{13-k}(q) = m_k(q - off_k);
    the z+1-partition read (engines cannot shift partitions by 1) is
    materialized on the idle TensorE as an exact 0/1 permutation matmul
    into PSUM, with the phantom z=64 boundary row (out-of-image neighbor
    => diff2 = |x(center)|^2, from s = sum_c x^2) accumulated by a second
    selector matmul. (PE_SHIFT=False falls back to partition-remap DMAs.)
  - ACT computes exp with the free scale immediate; each exp is written to
    both classes of its theta pair via a stride-0 input axis, directly into
    the interleaved [a, k, c] staging layout.
  - the staging tile matches DRAM layout exactly, so the store is one
    contiguous 128-partition DMA (28 KB/partition runs at AC=128).
"""

import numpy as np

import concourse.bass as bass
import concourse.tile as tile
from concourse import bacc, mybir
from concourse.bass_utils import run_bass_kernel_spmd

N_CORES = 8
B, Z, A, C = 16, 64, 512, 3
K, NCLS = 14, 4
LB = B // N_CORES            # local batches per core = 2
P = LB * Z                   # 128 partitions
AC = 128                     # azimuth chunk
BUFS = 3                     # tile pool buffers
PE_SHIFT = True              # z+1 partition shift via PE matmul vs SBUF DMA
XDN_PE = True                # derive x_dn on PE too (no duplicate DRAM read)
F32 = mybir.dt.float32
BF16 = mybir.dt.bfloat16


def _host_shift_mats():
    """SH2[k, m] = 1 iff k == m+1 (and not m == 63: batch boundary);
    SEL[k, m] = 1 iff k == m in {63, 127} (phantom z=64 row selector);
    SHD[k, m] = 1 iff k == m-1 (and not m in {0, 64}: z=0 rows stay 0)."""
    sh = np.zeros((P, P), np.float32)
    for m in range(P - 1):
        if m != Z - 1:
            sh[m + 1, m] = 1.0
    sel = np.zeros((P, P), np.float32)
    sel[Z - 1, Z - 1] = 1.0
    sel[P - 1, P - 1] = 1.0
    shd = np.zeros((P, P), np.float32)
    for m in range(1, P):
        if m != Z:
            shd[m - 1, m] = 1.0
    return sh, sel, shd

# exp scales: -1 / (2 * theta^2), theta pairs (0.015, 0.01), f32 semantics
_t0 = np.float32(0.015)
_t1 = np.float32(0.01)
SC0 = -float(1.0 / np.float32(np.float32(2.0) * _t0 * _t0))
SC1 = -float(1.0 / np.float32(np.float32(2.0) * _t1 * _t1))

# DRAM strides (elements) of out [LB, Z, A, K, NCLS]
O_A = K * NCLS               # 56
O_Z = A * O_A                # 28672
O_B = Z * O_Z                # 1835008
X_Z = A * C                  # 1536
X_B = Z * X_Z


def _ap(t, poff, pcnt, foff, pairs, pstep=1):
    """AP on tile t: partitions [poff, poff+pcnt) (stride pstep rows), free
    `pairs` ([step, count] in elements) based at element foff."""
    row = t.ap[0][0]
    return bass.AP(tensor=t.tensor, offset=t.offset + poff * row + foff,
                   ap=[[pstep * row, pcnt]] + [list(p) for p in pairs])


_SQDIFF = None


def _get_sqdiff():
    """Register a runtime custom DVE op: out = (in0 - in1)^2 (fp32, one
    instruction instead of subtract + multiply)."""
    global _SQDIFF
    if _SQDIFF is not None:
        return _SQDIFF
    from concourse import dve_ops
    from concourse.dve_spec import Spec, Src0, Src1, sq, lower, _has_src1
    from concourse.dve_uop import DveOpSpec

    name = "SQDIFF_BILAT_ANT"
    if name not in dve_ops._SUB_OPCODE_FOR_NAME:
        spec = Spec(
            body=sq(Src0 - Src1),
            reference=lambda in0, in1, c0, c1, c2:
                (in0.astype(np.float32) - in1.astype(np.float32)) ** 2)
        row = 1 + len(dve_ops.OPS)
        assert row < 0x20
        shas = {}
        for ver in ("v3",):
            tmp = DveOpSpec(name=name, opcode=row, uops=lower(spec, ver=ver),
                            rd1_en=_has_src1(spec))
            shas[ver] = tmp.sha(ver)
        op = dve_ops.DveOp(name, spec, subdim=False, uops_sha=shas)
        dve_ops.OPS.append(op)
        dve_ops.CUSTOM_DVE_SPECS[name] = spec
        dve_ops._SUB_OPCODE_FOR_NAME[name] = row
    else:
        op = next(o for o in dve_ops.OPS if o.name == name)
    _SQDIFF = op
    return op


def _build(ac=AC, bufs=BUFS, reps=1, pe_shift=PE_SHIFT, xdn_pe=XDN_PE,
           chunks=None, psum_bufs=3, store_rings=3, dt_mode="planes",
           amir_dve=0, sq_dve=False, ring_wts=None):
    # chunk schedule: list of (a0, width).  Uniform chunks minimize the
    # per-chunk fixed instruction overhead (~185 ns per ACT instruction,
    # 6 of them per chunk); with deep load prefetch the pipeline fill no
    # longer needs smaller leading chunks, and fill amortizes over reps.
    if chunks is None:
        chunks = [(a0, ac) for a0 in range(0, A, ac)]
    assert sum(w for _, w in chunks) == A
    NCH = len(chunks)
    nc = bacc.Bacc("TRN2", target_bir_lowering=False, debug=False,
                   num_devices=N_CORES)
    x_h = nc.dram_tensor("x", [LB, Z, A, C], F32, kind="ExternalInput")
    o_h = nc.dram_tensor("out", [LB, Z, A, K, NCLS], F32, kind="ExternalOutput")
    x_ap, o_ap = x_h.ap(), o_h.ap()
    if pe_shift:
        # 0/1 shift matrices are exact in bf16 (2x PE when M is bf16)
        CDT = F32 if dt_mode == "reduce" else BF16
        shm_h = nc.dram_tensor("shm", [P, P], CDT, kind="ExternalInput")
        sel_h = nc.dram_tensor("sel", [P, P], CDT, kind="ExternalInput")
        if xdn_pe:
            shd_h = nc.dram_tensor("shd", [P, P], F32, kind="ExternalInput")
    # bench mode: reps > 1 re-runs the whole kernel; each non-final pass
    # stores to its own DRAM scratch so stores are real traffic
    scratch_aps = [
        nc.dram_tensor(f"scr{r}", [LB, Z, A, K, NCLS], F32).ap()
        for r in range(reps - 1)]

    from contextlib import ExitStack
    with tile.TileContext(nc) as tc, ExitStack() as es:
        if pe_shift:
            consts = es.enter_context(tc.tile_pool(name="consts", bufs=1))
            psum = es.enter_context(
                tc.tile_pool(name="psum", bufs=psum_bufs, space="PSUM"))
        with tc.tile_pool(name="pool", bufs=bufs) as pool:
            if pe_shift:
                sh_t = consts.tile([P, P], CDT, name="sh_t")
                nc.sync.dma_start(sh_t[:], shm_h.ap()[:])
                sel_t = consts.tile([P, P], CDT, name="sel_t")
                nc.sync.dma_start(sel_t[:], sel_h.ap()[:])
                if xdn_pe:
                    shd_t = consts.tile([P, P], F32, name="shd_t")
                    nc.sync.dma_start(shd_t[:], shd_h.ap()[:])
            N = NCH * reps

            def _geom(ci):
                a0, ac = chunks[ci % NCH]
                XW = ac + 8          # x window (halo 4 each side)
                lo, hi = max(0, a0 - 4), min(A, a0 + ac + 4)
                c_lo = (lo - (a0 - 4)) * C          # first valid xt col
                c_hi = (hi - (a0 - 4)) * C
                return a0, ac, XW, lo, hi, c_lo, c_hi

            def emit_load(ci):
                # ---- load x window (zero halo at image borders) ----
                # (b, z) rows are contiguous in DRAM: one 128-partition DMA.
                # Loads issue on the (otherwise idle) gpsimd SWDGE so they
                # are not program-ordered behind the big store issues on SP
                # — the next chunks' loads must cut ahead of queued stores
                # or compute stalls behind them.
                # deep rotation: loads must be queued well before the big
                # stores they contend with, or they wait out a full 10 us
                # store before landing (xt is tiny: 1.6 KB/partition/buf)
                _, _, XW, lo, hi, c_lo, c_hi = _geom(ci)
                xt = pool.tile([P, XW * C], F32, name="xt", bufs=8)
                if c_lo > 0:
                    nc.gpsimd.memset(_ap(xt, 0, P, 0, [[1, c_lo]]), 0.0)
                if c_hi < XW * C:
                    nc.gpsimd.memset(
                        _ap(xt, 0, P, c_hi, [[1, XW * C - c_hi]]), 0.0)
                nc.gpsimd.dma_start(
                    _ap(xt, 0, P, c_lo, [[C, hi - lo], [1, C]]),
                    bass.AP(tensor=x_ap.tensor, offset=lo * C,
                            ap=[[X_Z, P], [C, hi - lo], [1, C]]))
                return xt

            def emit_xdn(ci, xt):
                # ---- x_dn[p] = x at (z-1) (zeros at z=0 rows): exact PE
                # permutation shift of xt into PSUM; the zero columns of SHD
                # give the z=0 rows (and the xt halo the image-border zeros)
                # for free.  Emitted one chunk AHEAD of the consuming chunk:
                # the PE is in-order, so x_dn(i+1) must precede M_up(i) or
                # the serial loop DVE(i) -> M_up(i) -> x_dn(i+1) -> DVE(i+1)
                # paces the pipeline above the store rate.  bufs=2 so the
                # psum pool fits 8 banks (M_up 3x2 + x_dn 2x1).
                _, _, XW, _, _, _, _ = _geom(ci)
                x_dn = psum.tile([P, XW * C], F32, name="x_dn_ps", bufs=2)
                for n0 in range(0, XW * C, 512):
                    n1 = min(XW * C, n0 + 512)
                    nc.tensor.matmul(
                        _ap(x_dn, 0, P, n0, [[1, n1 - n0]]),
                        shd_t[:], _ap(xt, 0, P, n0, [[1, n1 - n0]]),
                        start=True, stop=True)
                return x_dn

            PF = 7               # load prefetch distance (chunks ahead)
            xts, xdns = {}, {}
            for j in range(min(PF, N)):
                xts[j] = emit_load(j)
            if pe_shift and xdn_pe:
                xdns[0] = emit_xdn(0, xts[0])

            for ci in range(N):
                a0, ac, XW, lo, hi, c_lo, c_hi = _geom(ci)
                MW = ac + 4          # m window (halo 2 each side)

                if ci + PF < N:
                    xts[ci + PF] = emit_load(ci + PF)
                if pe_shift and xdn_pe and ci + 1 < N:
                    xdns[ci + 1] = emit_xdn(ci + 1, xts[ci + 1])
                xt = xts.pop(ci)

                if pe_shift and xdn_pe:
                    x_dn = xdns.pop(ci)
                else:
                    x_dn = pool.tile([P, XW * C], F32, name="x_dn")
                    nc.gpsimd.memset(x_dn[:], 0.0)
                    for b in range(LB):
                        nc.gpsimd.dma_start(
                            _ap(x_dn, b * Z + 1, Z - 1, c_lo,
                                [[C, hi - lo], [1, C]]),
                            bass.AP(tensor=x_ap.tensor, offset=b * X_B + lo * C,
                                    ap=[[X_Z, Z - 1], [C, hi - lo], [1, C]]))

                # ---- s = sum_c x^2 ; m_k maps over a-window [a0-2, ...)
                # k=0..4: dz=-1, da=k-2 ; k=5,6: dz=0, da=k-7
                # d2 = (x - x_nbr)^2 in one fused custom op per k.
                # dt_mode picks how the c-sum is done:
                #  "reduce": f32 interleaved + TensorReduce (no fast mode)
                #  "iadds":  bf16 interleaved (packed writes) + 2 stride-3
                #            tensor_adds — each add processes N/3 elements,
                #            beating the reduce's full-N stream
                #  "planes": custom writes c-outer packed bf16 planes; the
                #            adds are fully packed and hit the DVE 2x mode
                # bf16 rounds only d^2 / s / M (<=0.4% rel => ~2e-3 max abs
                # on the exp output, far inside the 2e-2 tolerance).
                sqdiff = _get_sqdiff()
                MDT = F32 if dt_mode == "reduce" else BF16
                M = pool.tile([P, 7 * MW], MDT, name="M")
                if dt_mode == "reduce":
                    sqx = pool.tile([P, XW * C], F32, name="sqx")
                    nc.scalar.square(sqx[:], xt[:])
                    st = pool.tile([P, XW], F32, name="st")
                    nc.vector.tensor_reduce(
                        st[:], _ap(sqx, 0, P, 0, [[C, XW], [1, C]]),
                        axis=mybir.AxisListType.X, op=mybir.AluOpType.add)
                    dt5 = pool.tile([P, 5 * MW * C], F32, name="dt5")
                    for k in range(5):
                        nc.vector._custom_dve(
                            sqdiff,
                            out=_ap(dt5, 0, P, k * MW * C, [[C, MW], [1, C]]),
                            in0=_ap(xt, 0, P, 2 * C, [[C, MW], [1, C]]),
                            in1=_ap(x_dn, 0, P, k * C, [[C, MW], [1, C]]))
                    nc.vector.tensor_reduce(
                        _ap(M, 0, P, 0, [[1, 5 * MW]]),
                        _ap(dt5, 0, P, 0, [[C, 5 * MW], [1, C]]),
                        axis=mybir.AxisListType.X, op=mybir.AluOpType.add)
                    dt2 = pool.tile([P, 2 * MW * C], F32, name="dt2")
                    for k in (5, 6):
                        nc.vector._custom_dve(
                            sqdiff,
                            out=_ap(dt2, 0, P, (k - 5) * MW * C,
                                    [[C, MW], [1, C]]),
                            in0=_ap(xt, 0, P, 2 * C, [[C, MW], [1, C]]),
                            in1=_ap(xt, 0, P, (k - 5) * C, [[C, MW], [1, C]]))
                    nc.vector.tensor_reduce(
                        _ap(M, 0, P, 5 * MW, [[1, 2 * MW]]),
                        _ap(dt2, 0, P, 0, [[C, 2 * MW], [1, C]]),
                        axis=mybir.AxisListType.X, op=mybir.AluOpType.add)
                else:
                    # custom-dve APs are rank<=3: one call per map k
                    dt = pool.tile([P, 3 * 7 * MW], BF16, name="dt")
                    if dt_mode == "planes":
                        # c-outer stream: strided f32 reads, PACKED bf16
                        # plane writes (scattered 2-byte writes would RMW)
                        d_out = lambda k: _ap(dt, 0, P, k * MW,
                                              [[7 * MW, C], [1, MW]])
                        d_in = lambda t, off: _ap(t, 0, P, off,
                                                  [[1, C], [C, MW]])
                        add_ap = lambda c: _ap(dt, 0, P, c * 7 * MW,
                                               [[1, 7 * MW]])
                    else:  # iadds: natural interleaved stream, packed writes
                        d_out = lambda k: _ap(dt, 0, P, 3 * k * MW,
                                              [[C, MW], [1, C]])
                        d_in = lambda t, off: _ap(t, 0, P, off,
                                                  [[C, MW], [1, C]])
                        add_ap = lambda c: _ap(dt, 0, P, c,
                                               [[C, 7 * MW]])
                    for k in range(7):
                        src, off = (x_dn, k * C) if k < 5 else (xt, (k - 5) * C)
                        nc.vector._custom_dve(
                            sqdiff, out=d_out(k),
                            in0=d_in(xt, 2 * C), in1=d_in(src, off))
                    dts = pool.tile([P, 7 * MW], BF16, name="dts")
                    nc.vector.tensor_add(dts[:], add_ap(0), add_ap(1))
                    nc.vector.tensor_add(M[:], dts[:], add_ap(2))

                    # s = sum_c x^2 via the same layout trick
                    sqx = pool.tile([P, 3 * XW], BF16, name="sqx")
                    if dt_mode == "planes":
                        sq_out = _ap(sqx, 0, P, 0, [[XW, C], [1, XW]])
                        sq_in = _ap(xt, 0, P, 0, [[1, C], [C, XW]])
                        s_ap = lambda c: _ap(sqx, 0, P, c * XW, [[1, XW]])
                    else:
                        sq_out, sq_in = sqx[:], xt[:]
                        s_ap = lambda c: _ap(sqx, 0, P, c, [[C, XW]])
                    if sq_dve:
                        nc.vector.tensor_mul(sq_out, sq_in, sq_in)
                    else:
                        nc.scalar.square(sq_out, sq_in)
                    stt = pool.tile([P, XW], BF16, name="stt")
                    st = pool.tile([P, XW], BF16, name="st")
                    nc.vector.tensor_add(stt[:], s_ap(0), s_ap(1))
                    nc.vector.tensor_add(st[:], stt[:], s_ap(2))

                # ---- M_up[p] = M[p+1] for k=0..4 cols; phantom z=64 rows
                # ({63,127}) = s(z=63 row) with k-dependent a-shift ----
                if pe_shift:
                    # PE permutation matmul: M_up = SH2^T.T @ M + SEL.T @ SD
                    # (exact for 0/1 matrices, also in bf16); phantom rows
                    # ride the second accumulating matmul through SD.  In
                    # planes mode everything is bf16 => 2x PE rate and a 4x
                    # TensorCopy for SD.
                    SD = pool.tile([P, 5 * MW], MDT, name="SD")
                    nc.vector.tensor_copy(
                        _ap(SD, 0, P, 0, [[MW, 5], [1, MW]]),
                        _ap(st, 0, P, 0, [[1, 5], [1, MW]]))
                    M_up = psum.tile([P, 5 * MW], F32, name="M_up_ps")
                    for n0 in range(0, 5 * MW, 512):
                        n1 = min(5 * MW, n0 + 512)
                        nc.tensor.matmul(
                            _ap(M_up, 0, P, n0, [[1, n1 - n0]]),
                            sh_t[:], _ap(M, 0, P, n0, [[1, n1 - n0]]),
                            start=True, stop=False)
                        nc.tensor.matmul(
                            _ap(M_up, 0, P, n0, [[1, n1 - n0]]),
                            sel_t[:], _ap(SD, 0, P, n0, [[1, n1 - n0]]),
                            start=False, stop=True)
                else:
                    M_up = pool.tile([P, 5 * MW], F32, name="M_up")
                    # disjoint remaps per batch so the phantom DMA runs parallel
                    for b in range(LB):
                        nc.sync.dma_start(
                            _ap(M_up, b * Z, Z - 1, 0, [[1, 5 * MW]]),
                            _ap(M, b * Z + 1, Z - 1, 0, [[1, 5 * MW]]))
                    # phantom: M_up[{63,127}, k*MW + ar] = st[{63,127}, ar + k]
                    nc.sync.dma_start(
                        _ap(M_up, Z - 1, 2, 0, [[MW, 5], [1, MW]], pstep=Z),
                        _ap(st, Z - 1, 2, 0, [[1, 5], [1, MW]], pstep=Z))

                # ---- exps into O staging [p, ar*56 + k*4 + c] ----
                # amir_dve: the a-mirror slots k'=7,8 duplicate the direct
                # k=6,5 exps at shifted a — a same-partition DVE copy
                # (rebalances element writes from the bottleneck ACT onto
                # DVE), with a 2-column ACT patch at the chunk edge where
                # the copy source falls outside this O tile.
                O = pool.tile([P, ac * O_A], F32, name="O",
                              bufs=(1 if ac >= 512 else
                                    2 if ac >= 256 else None))
                EXP = mybir.ActivationFunctionType.Exp
                for th, sc in ((0, SC0), (1, SC1)):
                    co = 2 * th
                    # direct k=0..6: in M[p, k*MW + ar + 2]
                    nc.scalar.activation(
                        _ap(O, 0, P, co, [[4, 7], [O_A, ac], [1, 2]]),
                        _ap(M, 0, P, 2, [[MW, 7], [1, ac], [0, 2]]),
                        EXP, scale=sc)
                    if amir_dve:
                        # boundary patch: k'=7,8 at a in {ac-2, ac-1}
                        nc.scalar.activation(
                            _ap(O, 0, P, 28 + co + (ac - 2) * O_A,
                                [[4, 2], [O_A, 2], [1, 2]]),
                            _ap(M, 0, P, 6 * MW + 3 + (ac - 2),
                                [[-(MW - 1), 2], [1, 2], [0, 2]]),
                            EXP, scale=sc)
                    else:
                        # a-mirrors k'=7,8 <- k=6,5: col = k*MW + ar + (9-k)
                        nc.scalar.activation(
                            _ap(O, 0, P, 28 + co, [[4, 2], [O_A, ac], [1, 2]]),
                            _ap(M, 0, P, 6 * MW + 3,
                                [[-(MW - 1), 2], [1, ac], [0, 2]]),
                            EXP, scale=sc)
                    # dz-mirrors k'=9..13 <- k=4..0: M_up[p, k*MW + ar + 4-k]
                    # (partition-shifted values, not copyable within a lane)
                    nc.scalar.activation(
                        _ap(O, 0, P, 36 + co, [[4, 5], [O_A, ac], [1, 2]]),
                        _ap(M_up, 0, P, 4 * (MW - 1) + 4,
                            [[-(MW - 1), 5], [1, ac], [0, 2]]),
                        EXP, scale=sc)
                if amir_dve:
                    # O[a, 7+j, c] = O[a+1+j, 6-j, c] for a < ac-2, all c
                    nc.vector.tensor_copy(
                        _ap(O, 0, P, 28, [[4, 2], [O_A, ac - 2], [1, 4]]),
                        _ap(O, 0, P, 80, [[52, 2], [O_A, ac - 2], [1, 4]]))

                # ---- store: contiguous 128-partition DMAs.  Each HWDGE
                # ring sustains ~418 GB/s independently (SP + ACT pair
                # ~800 aggregate), so the store of EVERY chunk is split
                # into equal per-ring pieces — alternating whole chunks
                # leaves one ring with 62.5% of the bytes and that ring's
                # 21.9 us/pass becomes the kernel's floor.  store_rings=3
                # adds the gpsimd SWDGE ring. ----
                rep_i = ci // NCH
                dst_ap = o_ap if rep_i == reps - 1 else scratch_aps[rep_i]
                if store_rings == 0:   # timing probe: no output stores
                    continue
                engs = (nc.sync, nc.scalar, nc.gpsimd)[:store_rings]
                wts = ring_wts or (1,) * store_rings
                tot = sum(wts)
                cum = [sum(wts[:r]) for r in range(store_rings + 1)]
                edges = [c * ac // tot for c in cum]
                for r, eng in enumerate(engs):
                    w0, w1 = edges[r], edges[r + 1]
                    eng.dma_start(
                        bass.AP(tensor=dst_ap.tensor,
                                offset=(a0 + w0) * O_A,
                                ap=[[O_Z, P], [1, (w1 - w0) * O_A]]),
                        _ap(O, 0, P, w0 * O_A, [[1, (w1 - w0) * O_A]]))

    nc.compile()
    return nc


def _build2(ac=512, bufs=2, reps=1, chunks=None, psum_bufs=1, store_rings=3,
            fuse_sq=True, pool_s=True, xdn_dma=True, pool_xpose=False,
            xpose_act=1, pf=3, dup_xt=True, ring_wts=(9, 9, 10)):
    """v2: dedup bf16 output [LB, Z, 2, K, A] -- classes within a theta pair
    are bit-identical (host broadcasts 2 -> 4 classes and permutes axes).
    Everything on chip lives in PLANE layout ([t*14+k][a], a innermost):
    every engine access streams long contiguous runs -- 2-byte scattered
    accesses on ACT cost 2-4x.  x is transposed once per chunk into c-planes
    (split Q7/ACT) so the fused per-c DVE sqdiff streams fully packed; sq on
    gpsimd, s-adds on DVE; dz-mirror planes built on PE in j-layout (one
    PSUM J tile, matmul pieces at bank boundaries, phantom z=64 row via a
    stride-0 moving axis over st); x_dn via SBUF partition-remap DMAs;
    stores split by plane across the three DMA queues (512B runs)."""
    if chunks is None:
        chunks = [(a0, ac) for a0 in range(0, A, ac)]
    assert sum(w for _, w in chunks) == A
    NCH = len(chunks)
    O_T = K * A                  # out plane-group stride (t axis)
    O_P = 2 * K * A              # out partition stride (z axis)
    nc = bacc.Bacc("TRN2", target_bir_lowering=False, debug=False,
                   num_devices=N_CORES)
    x_h = nc.dram_tensor("x", [LB, Z, A, C], F32, kind="ExternalInput")
    o_h = nc.dram_tensor("out", [LB, Z, 2, K, A], BF16, kind="ExternalOutput")
    x_ap, o_ap = x_h.ap(), o_h.ap()
    shm_h = nc.dram_tensor("shm", [P, P], BF16, kind="ExternalInput")
    sel_h = nc.dram_tensor("sel", [P, P], BF16, kind="ExternalInput")
    shd_h = None
    if not xdn_dma:
        shd_h = nc.dram_tensor("shd", [P, P], F32, kind="ExternalInput")
    scratch_aps = [
        nc.dram_tensor(f"scr{r}", [LB, Z, 2, K, A], BF16).ap()
        for r in range(reps - 1)]

    sqdiff = _get_sqdiff()
    from contextlib import ExitStack
    with tile.TileContext(nc) as tc, ExitStack() as es:
        consts = es.enter_context(tc.tile_pool(name="consts", bufs=1))
        psum = es.enter_context(
            tc.tile_pool(name="psum", bufs=psum_bufs, space="PSUM"))
        with tc.tile_pool(name="pool", bufs=bufs) as pool:
            sh_t = consts.tile([P, P], BF16, name="sh_t")
            nc.sync.dma_start(sh_t[:], shm_h.ap()[:])
            sel_t = consts.tile([P, P], BF16, name="sel_t")
            nc.sync.dma_start(sel_t[:], sel_h.ap()[:])
            if xdn_dma:
                # zero source rows for x_dn partitions {0, 64} (z=0 rows)
                zt = consts.tile([P, 3 * (ac + 8)], F32, name="zt")
                nc.gpsimd.memset(zt[:], 0.0)
            else:
                shd_t = consts.tile([P, P], F32, name="shd_t")
                nc.sync.dma_start(shd_t[:], shd_h.ap()[:])
            N = NCH * reps

            def _geom(ci):
                a0, acw = chunks[ci % NCH]
                XW = acw + 8
                lo, hi = max(0, a0 - 4), min(A, a0 + acw + 4)
                wlo = lo - (a0 - 4)       # first valid a in window coords
                whi = hi - (a0 - 4)
                return a0, acw, XW, lo, hi, wlo, whi

            def emit_load(ci):
                # interleaved [a, c] x window; halo memsets happen on the
                # transposed tile, so only the DMA here.
                _, _, XW, lo, hi, wlo, whi = _geom(ci)
                xt = pool.tile([P, XW * C], F32, name="xt", bufs=pf + 1)
                nc.gpsimd.dma_start(
                    _ap(xt, 0, P, wlo * C, [[C, hi - lo], [1, C]]),
                    bass.AP(tensor=x_ap.tensor, offset=lo * C,
                            ap=[[X_Z, P], [C, hi - lo], [1, C]]))
                return xt

            def emit_xpose(ci, xt):
                # xt_T[c*XW + a] = xt[a*C + c]; zero halo columns per plane.
                # The strided-read copy is split between Q7 (pool) and ACT:
                # xpose_act c-planes on ACT, the rest on Q7.
                _, _, XW, lo, hi, wlo, whi = _geom(ci)
                xT = pool.tile([P, C * XW], F32, name="xT",
                               bufs=(4 if ac < 512 else 3))
                if wlo > 0:
                    nc.gpsimd.memset(
                        _ap(xT, 0, P, 0, [[XW, C], [1, wlo]]), 0.0)
                if whi < XW:
                    nc.gpsimd.memset(
                        _ap(xT, 0, P, whi, [[XW, C], [1, XW - whi]]), 0.0)
                na = xpose_act if not pool_xpose else C
                # Q7 planes [0, C-na), ACT planes [C-na, C)
                nq = C - na if not pool_xpose else C
                w = whi - wlo
                if nq > 0:
                    nc.gpsimd.tensor_copy(
                        _ap(xT, 0, P, wlo, [[XW, nq], [1, w]]),
                        _ap(xt, 0, P, wlo * C, [[1, nq], [C, w]]))
                if not pool_xpose and na > 0:
                    nc.scalar.copy(
                        _ap(xT, 0, P, nq * XW + wlo, [[XW, na], [1, w]]),
                        _ap(xt, 0, P, wlo * C + nq, [[1, na], [C, w]]))
                if not dup_xt:
                    return xT, xT
                # duplicate so the k5,6 sqdiff's two read streams hit
                # different SBUF tiles (same-tile dual reads halve DVE rate,
                # ~2.9 vs ~1.2 ns/el); SBUF->SBUF DMA on the scalar queue --
                # a Q7 copy of this size costs 5.4us and starves the chunk.
                xU = pool.tile([P, C * XW], F32, name="xU",
                               bufs=(4 if ac < 512 else 3))
                nc.scalar.dma_start(xU[:], xT[:])
                return xT, xU

            def emit_xdn(ci, xT):
                # x_dn[c][a] = x(z-1) planes; rows z=0 (p in {0,64}) zeroed.
                _, _, XW, _, _, _, _ = _geom(ci)
                if xdn_dma:
                    xd = pool.tile([P, C * XW], F32, name="xd",
                                   bufs=(3 if ac < 512 else 2))
                    nc.sync.dma_start(
                        _ap(xd, 0, 2, 0, [[1, C * XW]], pstep=Z),
                        _ap(zt, 0, 2, 0, [[1, C * XW]]))
                    for b in range(LB):
                        nc.sync.dma_start(
                            _ap(xd, b * Z + 1, Z - 1, 0, [[1, C * XW]]),
                            _ap(xT, b * Z, Z - 1, 0, [[1, C * XW]]))
                else:
                    xd = psum.tile([P, C * XW], F32, name="xd_ps", bufs=2)
                    nc.tensor.matmul(
                        _ap(xd, 0, P, 0, [[1, C * XW]]),
                        shd_t[:], _ap(xT, 0, P, 0, [[1, C * XW]]),
                        start=True, stop=True)
                return xd

            PF = pf
            xts, xTs, xds = {}, {}, {}
            for j in range(min(PF, N)):
                xts[j] = emit_load(j)
            for j in range(min(2, N)):
                xTs[j] = emit_xpose(j, xts[j])
            xds[0] = emit_xdn(0, xTs[0][0])

            for ci in range(N):
                a0, acw, XW, lo, hi, wlo, whi = _geom(ci)
                MW = acw + 4

                if ci + PF < N:
                    xts[ci + PF] = emit_load(ci + PF)
                if ci + 2 < N:
                    xTs[ci + 2] = emit_xpose(ci + 2, xts[ci + 2])
                if ci + 1 < N:
                    xds[ci + 1] = emit_xdn(ci + 1, xTs[ci + 1][0])
                xt = xts.pop(ci)
                xT, xU = xTs.pop(ci)
                xd = xds.pop(ci)

                # ---- d^2 planes [c][k][a], bf16, fully packed streams ----
                dt = pool.tile([P, C * 7 * MW], BF16, name="dt",
                               bufs=(1 if ac >= 512 else None))
                if fuse_sq:
                    # custom-dve APs: <=2 free dims -> one call per c plane,
                    # k folded in via a stride-0 (center) / stride-1 axis.
                    # k5,6's in1 reads the INTERLEAVED xt tile: both streams
                    # from xT (same tile) halve DVE rate on SBUF port
                    # conflicts; a strided 4-byte read elsewhere is cheaper.
                    for c in range(C):
                        nc.vector._custom_dve(
                            sqdiff,
                            out=_ap(dt, 0, P, c * 7 * MW, [[MW, 5], [1, MW]]),
                            in0=_ap(xT, 0, P, c * XW + 2, [[0, 5], [1, MW]]),
                            in1=_ap(xd, 0, P, c * XW, [[1, 5], [1, MW]]))
                        nc.vector._custom_dve(
                            sqdiff,
                            out=_ap(dt, 0, P, c * 7 * MW + 5 * MW,
                                    [[MW, 2], [1, MW]]),
                            in0=_ap(xT, 0, P, c * XW + 2, [[0, 2], [1, MW]]),
                            in1=_ap(xU, 0, P, c * XW, [[1, 2], [1, MW]]))
                else:
                    for k in range(7):
                        src, off = (xd, k) if k < 5 else (xT, k - 5)
                        nc.vector._custom_dve(
                            sqdiff,
                            out=_ap(dt, 0, P, k * MW, [[7 * MW, C], [1, MW]]),
                            in0=_ap(xT, 0, P, 2, [[XW, C], [1, MW]]),
                            in1=_ap(src, 0, P, off, [[XW, C], [1, MW]]))

                # ---- M[k][a] = sum_c d^2 (packed bf16 adds, 2x DVE) ----
                dts = pool.tile([P, 7 * MW], BF16, name="dts")
                M = pool.tile([P, 7 * MW], BF16, name="M")
                add_ap = lambda c: _ap(dt, 0, P, c * 7 * MW, [[1, 7 * MW]])
                nc.vector.tensor_add(dts[:], add_ap(0), add_ap(1))
                nc.vector.tensor_add(M[:], dts[:], add_ap(2))

                # ---- x^2 planes (gpsimd); the c-sum for the phantom row
                # rides the SEL matmuls below, so no s-adds at all ----
                seng = nc.gpsimd if pool_s else nc.vector
                sqx = pool.tile([P, C * XW], BF16, name="sqx")
                seng.tensor_mul(sqx[:], xT[:], xT[:])

                # ---- dz-mirror planes in j-layout: J[j][a] = M(z+1-shift)
                # [plane 4-j][a+j] = SH2 @ M + SEL @ (sum_c x^2) (phantom
                # z=64 row: J[j][a] = s(a+4), stride-0 moving j axis; the
                # c-sum is 3 accumulating SEL matmuls over sqx planes).
                # Piece width <= 512 cols and PSUM-bank-aligned outputs.
                def _jmm(mu, off, jn, m_off):
                    nc.tensor.matmul(
                        _ap(mu, 0, P, off, [[1, jn * acw]]),
                        sh_t[:],
                        _ap(M, 0, P, m_off,
                            [[-(MW - 1), jn], [1, acw]] if jn > 1
                            else [[1, acw]]),
                        start=True, stop=False)
                    for c in range(C):
                        nc.tensor.matmul(
                            _ap(mu, 0, P, off, [[1, jn * acw]]),
                            sel_t[:],
                            _ap(sqx, 0, P, c * XW + 4,
                                [[0, jn], [1, acw]] if jn > 1
                                else [[1, acw]]),
                            start=False, stop=(c == C - 1))

                # matmul piece boundaries: multiples of 512 f32 (PSUM bank)
                # that are also j-plane boundaries; acw in {128, 256} works.
                # Two tiles (j=0,1 | j=2..4) so the next pass's J01 matmuls
                # can start as soon as this pass's dz01 exps retire.
                J01 = psum.tile([P, 2 * acw], F32, name="J01_ps")
                J234 = psum.tile([P, 3 * acw], F32, name="J234_ps")
                jper = max(1, 512 // acw)
                for j0 in range(0, 2, min(2, jper)):
                    jn = min(jper, 2 - j0)
                    _jmm(J01, j0 * acw, jn, (4 - j0) * MW + j0)
                for j0 in range(2, 5, min(3, jper)):
                    jn = min(jper, 5 - j0)
                    _jmm(J234, (j0 - 2) * acw, jn, (4 - j0) * MW + j0)

                # ---- exps into O planes [(t*14+k)*acw + a], bf16: all
                # operands stream a-innermost (contiguous runs).  ACT is
                # in-order: issue every M-dependent exp (direct + a-mirror,
                # both thetas) BEFORE any J-dependent dz exp, so ACT overlaps
                # the PE matmuls instead of stalling behind them ----
                O = pool.tile([P, acw * 2 * K], BF16, name="O",
                              bufs=(2 if acw >= 256 else None))
                EXP = mybir.ActivationFunctionType.Exp
                for th, sc in ((0, SC0), (1, SC1)):
                    o0 = th * K * acw
                    nc.scalar.activation(
                        _ap(O, 0, P, o0, [[acw, 7], [1, acw]]),
                        _ap(M, 0, P, 2, [[MW, 7], [1, acw]]),
                        EXP, scale=sc)
                    # a-mirrors k'=7,8 <- k=6,5 at a+1+j
                    nc.scalar.activation(
                        _ap(O, 0, P, o0 + 7 * acw, [[acw, 2], [1, acw]]),
                        _ap(M, 0, P, 6 * MW + 3, [[-(MW - 1), 2], [1, acw]]),
                        EXP, scale=sc)
                for th, sc in ((0, SC0), (1, SC1)):
                    o0 = th * K * acw
                    # dz-mirrors k'=9+j read J[j][a]; split by tile so J01
                    # frees early
                    nc.scalar.activation(
                        _ap(O, 0, P, o0 + 9 * acw, [[acw, 2], [1, acw]]),
                        _ap(J01, 0, P, 0, [[acw, 2], [1, acw]]),
                        EXP, scale=sc)
                    nc.scalar.activation(
                        _ap(O, 0, P, o0 + 11 * acw, [[acw, 3], [1, acw]]),
                        _ap(J234, 0, P, 0, [[acw, 3], [1, acw]]),
                        EXP, scale=sc)

                # ---- store: split by (t,k) plane across the three DMA
                # queues; each descriptor run is acw*2 bytes ----
                rep_i = ci // NCH
                dst_ap = o_ap if rep_i == reps - 1 else scratch_aps[rep_i]
                if store_rings == 0:
                    continue
                engs = (nc.sync, nc.scalar, nc.gpsimd)[:store_rings]
                # sync also carries the x_dn remaps -> fewer store planes
                wts = ring_wts or ((8, 12, 8) if store_rings == 3
                                   else (1,) * store_rings)
                tot = sum(wts)
                cum = [sum(wts[:r]) for r in range(store_rings + 1)]
                edges = [c * 2 * K // tot for c in cum]
                for r, eng in enumerate(engs):
                    p0, p1 = edges[r], edges[r + 1]
                    if p1 == p0:
                        continue
                    eng.dma_start(
                        bass.AP(tensor=dst_ap.tensor,
                                offset=p0 * A + a0,
                                ap=[[O_P, P], [A, p1 - p0], [1, acw]]),
                        _ap(O, 0, P, p0 * acw, [[1, (p1 - p0) * acw]]))

    nc.compile()
    return nc


class _Runner:
    """Compile once; reuse the jitted sharded executable across calls.

    Mirrors bass2jax.run_bass_via_pjrt's multi-core path, but without
    donated output buffers (the kernel writes every output element, so the
    zero "output operands" are passed once from device-resident buffers and
    reused)."""

    def __init__(self, nc=None):
        import jax
        from jax.sharding import Mesh, PartitionSpec, NamedSharding
        try:
            from jax.experimental.shard_map import shard_map
        except ImportError:
            from jax.shard_map import shard_map  # newer jax
        from concourse import bass2jax

        bass2jax.install_neuronx_cc_hook()
        if nc is None:
            nc = _build2()
        self.nc = nc

        partition_name = (nc.partition_id_tensor.name
                          if nc.partition_id_tensor else None)
        in_names, out_names, out_avals = [], [], []
        in_dtypes = {}
        for alloc in nc.m.functions[0].allocations:
            if not isinstance(alloc, mybir.MemoryLocationSet):
                continue
            name = alloc.memorylocations[0].name
            if alloc.kind == "ExternalInput":
                if name != partition_name:
                    in_names.append(name)
                    in_dtypes[name] = mybir.dt.np(alloc.dtype)
            elif alloc.kind == "ExternalOutput":
                out_names.append(name)
                out_avals.append(jax.core.ShapedArray(
                    tuple(alloc.tensor_shape), mybir.dt.np(alloc.dtype)))
        self.in_dtypes = in_dtypes
        assert set(in_names) <= {"x", "shm", "sel", "shd"}, in_names
        assert out_names == ["out"], out_names
        all_in_names = in_names + out_names
        if partition_name is not None:
            all_in_names = all_in_names + [partition_name]
        self.in_names = in_names

        def _body(*args):
            operands = list(args)
            if partition_name is not None:
                operands.append(bass2jax.partition_id_tensor())
            return tuple(bass2jax._bass_exec_p.bind(
                *operands,
                out_avals=tuple(out_avals),
                in_names=tuple(all_in_names),
                out_names=tuple(out_names),
                lowering_input_output_aliases=(),
                sim_require_finite=True,
                sim_require_nnan=True,
                nc=nc,
            ))

        devices = jax.devices()[:N_CORES]
        assert len(devices) == N_CORES
        self.mesh = Mesh(np.asarray(devices), ("core",))
        spec = PartitionSpec("core")
        rep = PartitionSpec()
        self.sharding = NamedSharding(self.mesh, spec)
        in_specs = tuple(spec if n == "x" else rep for n in in_names) + (spec,)
        self.jitted = jax.jit(shard_map(
            _body, mesh=self.mesh, in_specs=in_specs, out_specs=(spec,),
            check_rep=False))
        # device-resident constant operands, created once
        oav = out_avals[0]
        full_shape = (N_CORES * oav.shape[0],) + tuple(oav.shape[1:])
        self.out_shape, self.out_dtype = full_shape, oav.dtype
        self.zeros_dev = jax.device_put(
            np.zeros(full_shape, oav.dtype), self.sharding)
        consts = {}
        if "shm" in in_names:
            shm, sel, shd = _host_shift_mats()
            rep_sh = NamedSharding(self.mesh, rep)
            for n, arr in (("shm", shm), ("sel", sel), ("shd", shd)):
                if n in in_names:
                    consts[n] = jax.device_put(
                        arr.astype(in_dtypes[n]), rep_sh)
        self.consts = consts
        self._jax = jax

    def put(self, x: np.ndarray):
        return self._jax.device_put(
            np.ascontiguousarray(np.asarray(x, np.float32)), self.sharding)

    def run_dev(self, x_dev):
        """Execute; returns device array (not fetched)."""
        args = [x_dev if n == "x" else self.consts[n] for n in self.in_names]
        return self.jitted(*args, self.zeros_dev)[0]

    def __call__(self, x: np.ndarray) -> np.ndarray:
        return _expand_out(np.asarray(self.run_dev(self.put(x))))


_RUNNER = None


def _expand_out(o: np.ndarray) -> np.ndarray:
    """Device output -> reference layout/dtype.  The two classes of each
    theta pair share one exp value (THETA_R pairs are equal), so the device
    stores [.., 2, K, A] (theta/k-plane-major so device writes and stores
    stream contiguous runs); permute to [.., A, K, 2], broadcast to
    [.., A, K, 4], widen bf16 -> f32."""
    if o.shape[-3:] == (2, K, A):
        o32 = np.asarray(o, np.float32).transpose(0, 1, 4, 3, 2)
        o = np.broadcast_to(o32[..., None],
                            o32.shape + (2,)).reshape(o32.shape[:-1] + (4,))
    return np.ascontiguousarray(o.astype(np.float32, copy=False))


def _get_runner():
    global _RUNNER
    if _RUNNER is None:
        _RUNNER = _Runner()
    return _RUNNER


def kernel(x: np.ndarray) -> np.ndarray:
    x = np.asarray(x, dtype=np.float32)
    assert x.shape == (B, Z, A, C), x.shape
    try:
        return _get_runner()(x)
    except Exception:
        # fallback: reference-quality but slower dispatch path
        nc = _build2()
        shm, sel, _ = _host_shift_mats()
        cdt = mybir.dt.np(BF16)
        extra = {"shm": shm.astype(cdt), "sel": sel.astype(cdt)}
        in_maps = [{"x": np.ascontiguousarray(x[i * LB:(i + 1) * LB]), **extra}
                   for i in range(N_CORES)]
        res = run_bass_kernel_spmd(nc, in_maps, list(range(N_CORES)))
        return _expand_out(np.concatenate(
            [res.results[i]["out"] for i in range(N_CORES)], axis=0))



# revision 64
# speedup vs baseline: 1.4761x; 1.0638x over previous
"""BilateralFilter (SqueezeSeg mc condensing-kernel gaussians) on 8 TRN2 cores.

Reference computes, for x: [16, 64, 512, 3] (B, Z, A, C=xyz):
    nbr   = 14 spatial neighbors of each pixel in a 3x5 window (zero-padded)
    diff2 = sum_c (x - nbr)^2                           [B, Z, A, 14]
    out   = exp(-diff2 / (2 * theta_r^2))               [B, Z, A, 14, 4]
with THETA_R = [0.015, 0.015, 0.01, 0.01] (only 2 distinct values).

Active implementation: _build2 (see its docstring).  Key ideas on top of the
v1 baseline (_build, kept for A/B): the two classes of a theta pair are
bit-identical, so the device computes/stores only 28 unique bf16 values per
pixel in (theta, k)-plane layout [LB, Z, 2, K, A] (4x less ACT exp work,
4x fewer store bytes) and the host broadcasts/permutes to [B, Z, A, 14, 4]
f32; all on-chip tensors are a-innermost planes because 2-byte scattered
accesses run 2-4x slow on ACT; x is transposed once per chunk into c-planes
so the fused custom DVE sqdiff streams fully packed.

v1 strategy notes (pure batch data-parallel, 2 batches per core):
  - partitions p = b*64 + z  (128), free dim = azimuth chunks (AC wide).
  - squared differences via a runtime-registered fused custom DVE op
    (out = (in0-in1)^2), channel sums via tensor_reduce.
  - mirror symmetry: m_k(q) = |x(q) - x(q+off_k)|^2 for the 7 "negative"
    offsets k=0..6 gives the other 7 via diff2_{13-k}(q) = m_k(q - off_k);
    the z+1-partition read (engines cannot shift partitions by 1) is
    materialized on the idle TensorE as an exact 0/1 permutation matmul
    into PSUM, with the phantom z=64 boundary row (out-of-image neighbor
    => diff2 = |x(center)|^2, from s = sum_c x^2) accumulated by a second
    selector matmul. (PE_SHIFT=False falls back to partition-remap DMAs.)
  - ACT computes exp with the free scale immediate; each exp is written to
    both classes of its theta pair via a stride-0 input axis, directly into
    the interleaved [a, k, c] staging layout.
  - the staging tile matches DRAM layout exactly, so the store is one
    contiguous 128-partition DMA (28 KB/partition runs at AC=128).
"""

import numpy as np

import concourse.bass as bass
import concourse.tile as tile
from concourse import bacc, mybir
from concourse.bass_utils import run_bass_kernel_spmd

N_CORES = 8
B, Z, A, C = 16, 64, 512, 3
K, NCLS = 14, 4
LB = B // N_CORES            # local batches per core = 2
P = LB * Z                   # 128 partitions
AC = 128                     # azimuth chunk
BUFS = 3                     # tile pool buffers
PE_SHIFT = True              # z+1 partition shift via PE matmul vs SBUF DMA
XDN_PE = True                # derive x_dn on PE too (no duplicate DRAM read)
F32 = mybir.dt.float32
BF16 = mybir.dt.bfloat16


def _host_shift_mats():
    """SH2[k, m] = 1 iff k == m+1 (and not m == 63: batch boundary);
    SEL[k, m] = 1 iff k == m in {63, 127} (phantom z=64 row selector);
    SHD[k, m] = 1 iff k == m-1 (and not m in {0, 64}: z=0 rows stay 0)."""
    sh = np.zeros((P, P), np.float32)
    for m in range(P - 1):
        if m != Z - 1:
            sh[m + 1, m] = 1.0
    sel = np.zeros((P, P), np.float32)
    sel[Z - 1, Z - 1] = 1.0
    sel[P - 1, P - 1] = 1.0
    shd = np.zeros((P, P), np.float32)
    for m in range(1, P):
        if m != Z:
            shd[m - 1, m] = 1.0
    return sh, sel, shd

# exp scales: -1 / (2 * theta^2), theta pairs (0.015, 0.01), f32 semantics
_t0 = np.float32(0.015)
_t1 = np.float32(0.01)
SC0 = -float(1.0 / np.float32(np.float32(2.0) * _t0 * _t0))
SC1 = -float(1.0 / np.float32(np.float32(2.0) * _t1 * _t1))

# DRAM strides (elements) of out [LB, Z, A, K, NCLS]
O_A = K * NCLS               # 56
O_Z = A * O_A                # 28672
O_B = Z * O_Z                # 1835008
X_Z = A * C                  # 1536
X_B = Z * X_Z


def _ap(t, poff, pcnt, foff, pairs, pstep=1):
    """AP on tile t: partitions [poff, poff+pcnt) (stride pstep rows), free
    `pairs` ([step, count] in elements) based at element foff."""
    row = t.ap[0][0]
    return bass.AP(tensor=t.tensor, offset=t.offset + poff * row + foff,
                   ap=[[pstep * row, pcnt]] + [list(p) for p in pairs])


_SQDIFF = None


def _get_sqdiff():
    """Register a runtime custom DVE op: out = (in0 - in1)^2 (fp32, one
    instruction instead of subtract + multiply)."""
    global _SQDIFF
    if _SQDIFF is not None:
        return _SQDIFF
    from concourse import dve_ops
    from concourse.dve_spec import Spec, Src0, Src1, sq, lower, _has_src1
    from concourse.dve_uop import DveOpSpec

    name = "SQDIFF_BILAT_ANT"
    if name not in dve_ops._SUB_OPCODE_FOR_NAME:
        spec = Spec(
            body=sq(Src0 - Src1),
            reference=lambda in0, in1, c0, c1, c2:
                (in0.astype(np.float32) - in1.astype(np.float32)) ** 2)
        row = 1 + len(dve_ops.OPS)
        assert row < 0x20
        shas = {}
        for ver in ("v3",):
            tmp = DveOpSpec(name=name, opcode=row, uops=lower(spec, ver=ver),
                            rd1_en=_has_src1(spec))
            shas[ver] = tmp.sha(ver)
        op = dve_ops.DveOp(name, spec, subdim=False, uops_sha=shas)
        dve_ops.OPS.append(op)
        dve_ops.CUSTOM_DVE_SPECS[name] = spec
        dve_ops._SUB_OPCODE_FOR_NAME[name] = row
    else:
        op = next(o for o in dve_ops.OPS if o.name == name)
    _SQDIFF = op
    return op


def _build(ac=AC, bufs=BUFS, reps=1, pe_shift=PE_SHIFT, xdn_pe=XDN_PE,
           chunks=None, psum_bufs=3, store_rings=3, dt_mode="planes",
           amir_dve=0, sq_dve=False, ring_wts=None):
    # chunk schedule: list of (a0, width).  Uniform chunks minimize the
    # per-chunk fixed instruction overhead (~185 ns per ACT instruction,
    # 6 of them per chunk); with deep load prefetch the pipeline fill no
    # longer needs smaller leading chunks, and fill amortizes over reps.
    if chunks is None:
        chunks = [(a0, ac) for a0 in range(0, A, ac)]
    assert sum(w for _, w in chunks) == A
    NCH = len(chunks)
    nc = bacc.Bacc("TRN2", target_bir_lowering=False, debug=False,
                   num_devices=N_CORES)
    x_h = nc.dram_tensor("x", [LB, Z, A, C], F32, kind="ExternalInput")
    o_h = nc.dram_tensor("out", [LB, Z, A, K, NCLS], F32, kind="ExternalOutput")
    x_ap, o_ap = x_h.ap(), o_h.ap()
    if pe_shift:
        # 0/1 shift matrices are exact in bf16 (2x PE when M is bf16)
        CDT = F32 if dt_mode == "reduce" else BF16
        shm_h = nc.dram_tensor("shm", [P, P], CDT, kind="ExternalInput")
        sel_h = nc.dram_tensor("sel", [P, P], CDT, kind="ExternalInput")
        if xdn_pe:
            shd_h = nc.dram_tensor("shd", [P, P], F32, kind="ExternalInput")
    # bench mode: reps > 1 re-runs the whole kernel; each non-final pass
    # stores to its own DRAM scratch so stores are real traffic
    scratch_aps = [
        nc.dram_tensor(f"scr{r}", [LB, Z, A, K, NCLS], F32).ap()
        for r in range(reps - 1)]

    from contextlib import ExitStack
    with tile.TileContext(nc) as tc, ExitStack() as es:
        if pe_shift:
            consts = es.enter_context(tc.tile_pool(name="consts", bufs=1))
            psum = es.enter_context(
                tc.tile_pool(name="psum", bufs=psum_bufs, space="PSUM"))
        with tc.tile_pool(name="pool", bufs=bufs) as pool:
            if pe_shift:
                sh_t = consts.tile([P, P], CDT, name="sh_t")
                nc.sync.dma_start(sh_t[:], shm_h.ap()[:])
                sel_t = consts.tile([P, P], CDT, name="sel_t")
                nc.sync.dma_start(sel_t[:], sel_h.ap()[:])
                if xdn_pe:
                    shd_t = consts.tile([P, P], F32, name="shd_t")
                    nc.sync.dma_start(shd_t[:], shd_h.ap()[:])
            N = NCH * reps

            def _geom(ci):
                a0, ac = chunks[ci % NCH]
                XW = ac + 8          # x window (halo 4 each side)
                lo, hi = max(0, a0 - 4), min(A, a0 + ac + 4)
                c_lo = (lo - (a0 - 4)) * C          # first valid xt col
                c_hi = (hi - (a0 - 4)) * C
                return a0, ac, XW, lo, hi, c_lo, c_hi

            def emit_load(ci):
                # ---- load x window (zero halo at image borders) ----
                # (b, z) rows are contiguous in DRAM: one 128-partition DMA.
                # Loads issue on the (otherwise idle) gpsimd SWDGE so they
                # are not program-ordered behind the big store issues on SP
                # — the next chunks' loads must cut ahead of queued stores
                # or compute stalls behind them.
                # deep rotation: loads must be queued well before the big
                # stores they contend with, or they wait out a full 10 us
                # store before landing (xt is tiny: 1.6 KB/partition/buf)
                _, _, XW, lo, hi, c_lo, c_hi = _geom(ci)
                xt = pool.tile([P, XW * C], F32, name="xt", bufs=8)
                if c_lo > 0:
                    nc.gpsimd.memset(_ap(xt, 0, P, 0, [[1, c_lo]]), 0.0)
                if c_hi < XW * C:
                    nc.gpsimd.memset(
                        _ap(xt, 0, P, c_hi, [[1, XW * C - c_hi]]), 0.0)
                nc.gpsimd.dma_start(
                    _ap(xt, 0, P, c_lo, [[C, hi - lo], [1, C]]),
                    bass.AP(tensor=x_ap.tensor, offset=lo * C,
                            ap=[[X_Z, P], [C, hi - lo], [1, C]]))
                return xt

            def emit_xdn(ci, xt):
                # ---- x_dn[p] = x at (z-1) (zeros at z=0 rows): exact PE
                # permutation shift of xt into PSUM; the zero columns of SHD
                # give the z=0 rows (and the xt halo the image-border zeros)
                # for free.  Emitted one chunk AHEAD of the consuming chunk:
                # the PE is in-order, so x_dn(i+1) must precede M_up(i) or
                # the serial loop DVE(i) -> M_up(i) -> x_dn(i+1) -> DVE(i+1)
                # paces the pipeline above the store rate.  bufs=2 so the
                # psum pool fits 8 banks (M_up 3x2 + x_dn 2x1).
                _, _, XW, _, _, _, _ = _geom(ci)
                x_dn = psum.tile([P, XW * C], F32, name="x_dn_ps", bufs=2)
                for n0 in range(0, XW * C, 512):
                    n1 = min(XW * C, n0 + 512)
                    nc.tensor.matmul(
                        _ap(x_dn, 0, P, n0, [[1, n1 - n0]]),
                        shd_t[:], _ap(xt, 0, P, n0, [[1, n1 - n0]]),
                        start=True, stop=True)
                return x_dn

            PF = 7               # load prefetch distance (chunks ahead)
            xts, xdns = {}, {}
            for j in range(min(PF, N)):
                xts[j] = emit_load(j)
            if pe_shift and xdn_pe:
                xdns[0] = emit_xdn(0, xts[0])

            for ci in range(N):
                a0, ac, XW, lo, hi, c_lo, c_hi = _geom(ci)
                MW = ac + 4          # m window (halo 2 each side)

                if ci + PF < N:
                    xts[ci + PF] = emit_load(ci + PF)
                if pe_shift and xdn_pe and ci + 1 < N:
                    xdns[ci + 1] = emit_xdn(ci + 1, xts[ci + 1])
                xt = xts.pop(ci)

                if pe_shift and xdn_pe:
                    x_dn = xdns.pop(ci)
                else:
                    x_dn = pool.tile([P, XW * C], F32, name="x_dn")
                    nc.gpsimd.memset(x_dn[:], 0.0)
                    for b in range(LB):
                        nc.gpsimd.dma_start(
                            _ap(x_dn, b * Z + 1, Z - 1, c_lo,
                                [[C, hi - lo], [1, C]]),
                            bass.AP(tensor=x_ap.tensor, offset=b * X_B + lo * C,
                                    ap=[[X_Z, Z - 1], [C, hi - lo], [1, C]]))

                # ---- s = sum_c x^2 ; m_k maps over a-window [a0-2, ...)
                # k=0..4: dz=-1, da=k-2 ; k=5,6: dz=0, da=k-7
                # d2 = (x - x_nbr)^2 in one fused custom op per k.
                # dt_mode picks how the c-sum is done:
                #  "reduce": f32 interleaved + TensorReduce (no fast mode)
                #  "iadds":  bf16 interleaved (packed writes) + 2 stride-3
                #            tensor_adds — each add processes N/3 elements,
                #            beating the reduce's full-N stream
                #  "planes": custom writes c-outer packed bf16 planes; the
                #            adds are fully packed and hit the DVE 2x mode
                # bf16 rounds only d^2 / s / M (<=0.4% rel => ~2e-3 max abs
                # on the exp output, far inside the 2e-2 tolerance).
                sqdiff = _get_sqdiff()
                MDT = F32 if dt_mode == "reduce" else BF16
                M = pool.tile([P, 7 * MW], MDT, name="M")
                if dt_mode == "reduce":
                    sqx = pool.tile([P, XW * C], F32, name="sqx")
                    nc.scalar.square(sqx[:], xt[:])
                    st = pool.tile([P, XW], F32, name="st")
                    nc.vector.tensor_reduce(
                        st[:], _ap(sqx, 0, P, 0, [[C, XW], [1, C]]),
                        axis=mybir.AxisListType.X, op=mybir.AluOpType.add)
                    dt5 = pool.tile([P, 5 * MW * C], F32, name="dt5")
                    for k in range(5):
                        nc.vector._custom_dve(
                            sqdiff,
                            out=_ap(dt5, 0, P, k * MW * C, [[C, MW], [1, C]]),
                            in0=_ap(xt, 0, P, 2 * C, [[C, MW], [1, C]]),
                            in1=_ap(x_dn, 0, P, k * C, [[C, MW], [1, C]]))
                    nc.vector.tensor_reduce(
                        _ap(M, 0, P, 0, [[1, 5 * MW]]),
                        _ap(dt5, 0, P, 0, [[C, 5 * MW], [1, C]]),
                        axis=mybir.AxisListType.X, op=mybir.AluOpType.add)
                    dt2 = pool.tile([P, 2 * MW * C], F32, name="dt2")
                    for k in (5, 6):
                        nc.vector._custom_dve(
                            sqdiff,
                            out=_ap(dt2, 0, P, (k - 5) * MW * C,
                                    [[C, MW], [1, C]]),
                            in0=_ap(xt, 0, P, 2 * C, [[C, MW], [1, C]]),
                            in1=_ap(xt, 0, P, (k - 5) * C, [[C, MW], [1, C]]))
                    nc.vector.tensor_reduce(
                        _ap(M, 0, P, 5 * MW, [[1, 2 * MW]]),
                        _ap(dt2, 0, P, 0, [[C, 2 * MW], [1, C]]),
                        axis=mybir.AxisListType.X, op=mybir.AluOpType.add)
                else:
                    # custom-dve APs are rank<=3: one call per map k
                    dt = pool.tile([P, 3 * 7 * MW], BF16, name="dt")
                    if dt_mode == "planes":
                        # c-outer stream: strided f32 reads, PACKED bf16
                        # plane writes (scattered 2-byte writes would RMW)
                        d_out = lambda k: _ap(dt, 0, P, k * MW,
                                              [[7 * MW, C], [1, MW]])
                        d_in = lambda t, off: _ap(t, 0, P, off,
                                                  [[1, C], [C, MW]])
                        add_ap = lambda c: _ap(dt, 0, P, c * 7 * MW,
                                               [[1, 7 * MW]])
                    else:  # iadds: natural interleaved stream, packed writes
                        d_out = lambda k: _ap(dt, 0, P, 3 * k * MW,
                                              [[C, MW], [1, C]])
                        d_in = lambda t, off: _ap(t, 0, P, off,
                                                  [[C, MW], [1, C]])
                        add_ap = lambda c: _ap(dt, 0, P, c,
                                               [[C, 7 * MW]])
                    for k in range(7):
                        src, off = (x_dn, k * C) if k < 5 else (xt, (k - 5) * C)
                        nc.vector._custom_dve(
                            sqdiff, out=d_out(k),
                            in0=d_in(xt, 2 * C), in1=d_in(src, off))
                    dts = pool.tile([P, 7 * MW], BF16, name="dts")
                    nc.vector.tensor_add(dts[:], add_ap(0), add_ap(1))
                    nc.vector.tensor_add(M[:], dts[:], add_ap(2))

                    # s = sum_c x^2 via the same layout trick
                    sqx = pool.tile([P, 3 * XW], BF16, name="sqx")
                    if dt_mode == "planes":
                        sq_out = _ap(sqx, 0, P, 0, [[XW, C], [1, XW]])
                        sq_in = _ap(xt, 0, P, 0, [[1, C], [C, XW]])
                        s_ap = lambda c: _ap(sqx, 0, P, c * XW, [[1, XW]])
                    else:
                        sq_out, sq_in = sqx[:], xt[:]
                        s_ap = lambda c: _ap(sqx, 0, P, c, [[C, XW]])
                    if sq_dve:
                        nc.vector.tensor_mul(sq_out, sq_in, sq_in)
                    else:
                        nc.scalar.square(sq_out, sq_in)
                    stt = pool.tile([P, XW], BF16, name="stt")
                    st = pool.tile([P, XW], BF16, name="st")
                    nc.vector.tensor_add(stt[:], s_ap(0), s_ap(1))
                    nc.vector.tensor_add(st[:], stt[:], s_ap(2))

                # ---- M_up[p] = M[p+1] for k=0..4 cols; phantom z=64 rows
                # ({63,127}) = s(z=63 row) with k-dependent a-shift ----
                if pe_shift:
                    # PE permutation matmul: M_up = SH2^T.T @ M + SEL.T @ SD
                    # (exact for 0/1 matrices, also in bf16); phantom rows
                    # ride the second accumulating matmul through SD.  In
                    # planes mode everything is bf16 => 2x PE rate and a 4x
                    # TensorCopy for SD.
                    SD = pool.tile([P, 5 * MW], MDT, name="SD")
                    nc.vector.tensor_copy(
                        _ap(SD, 0, P, 0, [[MW, 5], [1, MW]]),
                        _ap(st, 0, P, 0, [[1, 5], [1, MW]]))
                    M_up = psum.tile([P, 5 * MW], F32, name="M_up_ps")
                    for n0 in range(0, 5 * MW, 512):
                        n1 = min(5 * MW, n0 + 512)
                        nc.tensor.matmul(
                            _ap(M_up, 0, P, n0, [[1, n1 - n0]]),
                            sh_t[:], _ap(M, 0, P, n0, [[1, n1 - n0]]),
                            start=True, stop=False)
                        nc.tensor.matmul(
                            _ap(M_up, 0, P, n0, [[1, n1 - n0]]),
                            sel_t[:], _ap(SD, 0, P, n0, [[1, n1 - n0]]),
                            start=False, stop=True)
                else:
                    M_up = pool.tile([P, 5 * MW], F32, name="M_up")
                    # disjoint remaps per batch so the phantom DMA runs parallel
                    for b in range(LB):
                        nc.sync.dma_start(
                            _ap(M_up, b * Z, Z - 1, 0, [[1, 5 * MW]]),
                            _ap(M, b * Z + 1, Z - 1, 0, [[1, 5 * MW]]))
                    # phantom: M_up[{63,127}, k*MW + ar] = st[{63,127}, ar + k]
                    nc.sync.dma_start(
                        _ap(M_up, Z - 1, 2, 0, [[MW, 5], [1, MW]], pstep=Z),
                        _ap(st, Z - 1, 2, 0, [[1, 5], [1, MW]], pstep=Z))

                # ---- exps into O staging [p, ar*56 + k*4 + c] ----
                # amir_dve: the a-mirror slots k'=7,8 duplicate the direct
                # k=6,5 exps at shifted a — a same-partition DVE copy
                # (rebalances element writes from the bottleneck ACT onto
                # DVE), with a 2-column ACT patch at the chunk edge where
                # the copy source falls outside this O tile.
                O = pool.tile([P, ac * O_A], F32, name="O",
                              bufs=(1 if ac >= 512 else
                                    2 if ac >= 256 else None))
                EXP = mybir.ActivationFunctionType.Exp
                for th, sc in ((0, SC0), (1, SC1)):
                    co = 2 * th
                    # direct k=0..6: in M[p, k*MW + ar + 2]
                    nc.scalar.activation(
                        _ap(O, 0, P, co, [[4, 7], [O_A, ac], [1, 2]]),
                        _ap(M, 0, P, 2, [[MW, 7], [1, ac], [0, 2]]),
                        EXP, scale=sc)
                    if amir_dve:
                        # boundary patch: k'=7,8 at a in {ac-2, ac-1}
                        nc.scalar.activation(
                            _ap(O, 0, P, 28 + co + (ac - 2) * O_A,
                                [[4, 2], [O_A, 2], [1, 2]]),
                            _ap(M, 0, P, 6 * MW + 3 + (ac - 2),
                                [[-(MW - 1), 2], [1, 2], [0, 2]]),
                            EXP, scale=sc)
                    else:
                        # a-mirrors k'=7,8 <- k=6,5: col = k*MW + ar + (9-k)
                        nc.scalar.activation(
                            _ap(O, 0, P, 28 + co, [[4, 2], [O_A, ac], [1, 2]]),
                            _ap(M, 0, P, 6 * MW + 3,
                                [[-(MW - 1), 2], [1, ac], [0, 2]]),
                            EXP, scale=sc)
                    # dz-mirrors k'=9..13 <- k=4..0: M_up[p, k*MW + ar + 4-k]
                    # (partition-shifted values, not copyable within a lane)
                    nc.scalar.activation(
                        _ap(O, 0, P, 36 + co, [[4, 5], [O_A, ac], [1, 2]]),
                        _ap(M_up, 0, P, 4 * (MW - 1) + 4,
                            [[-(MW - 1), 5], [1, ac], [0, 2]]),
                        EXP, scale=sc)
                if amir_dve:
                    # O[a, 7+j, c] = O[a+1+j, 6-j, c] for a < ac-2, all c
                    nc.vector.tensor_copy(
                        _ap(O, 0, P, 28, [[4, 2], [O_A, ac - 2], [1, 4]]),
                        _ap(O, 0, P, 80, [[52, 2], [O_A, ac - 2], [1, 4]]))

                # ---- store: contiguous 128-partition DMAs.  Each HWDGE
                # ring sustains ~418 GB/s independently (SP + ACT pair
                # ~800 aggregate), so the store of EVERY chunk is split
                # into equal per-ring pieces — alternating whole chunks
                # leaves one ring with 62.5% of the bytes and that ring's
                # 21.9 us/pass becomes the kernel's floor.  store_rings=3
                # adds the gpsimd SWDGE ring. ----
                rep_i = ci // NCH
                dst_ap = o_ap if rep_i == reps - 1 else scratch_aps[rep_i]
                if store_rings == 0:   # timing probe: no output stores
                    continue
                engs = (nc.sync, nc.scalar, nc.gpsimd)[:store_rings]
                wts = ring_wts or (1,) * store_rings
                tot = sum(wts)
                cum = [sum(wts[:r]) for r in range(store_rings + 1)]
                edges = [c * ac // tot for c in cum]
                for r, eng in enumerate(engs):
                    w0, w1 = edges[r], edges[r + 1]
                    eng.dma_start(
                        bass.AP(tensor=dst_ap.tensor,
                                offset=(a0 + w0) * O_A,
                                ap=[[O_Z, P], [1, (w1 - w0) * O_A]]),
                        _ap(O, 0, P, w0 * O_A, [[1, (w1 - w0) * O_A]]))

    nc.compile()
    return nc


def _build2(ac=512, bufs=2, reps=1, chunks=None, psum_bufs=1, store_rings=3,
            fuse_sq=True, pool_s=True, xdn_dma=True, pool_xpose=False,
            xpose_act=1, pf=2, dup_xt=True, ring_wts=(9, 9, 10)):
    """v2: dedup bf16 output [LB, Z, 2, K, A] -- classes within a theta pair
    are bit-identical (host broadcasts 2 -> 4 classes and permutes axes).
    Everything on chip lives in PLANE layout ([t*14+k][a], a innermost):
    every engine access streams long contiguous runs -- 2-byte scattered
    accesses on ACT cost 2-4x.  x is transposed once per chunk into c-planes
    (split Q7/ACT) so the fused per-c DVE sqdiff streams fully packed; sq on
    gpsimd, s-adds on DVE; dz-mirror planes built on PE in j-layout (one
    PSUM J tile, matmul pieces at bank boundaries, phantom z=64 row via a
    stride-0 moving axis over st); x_dn via SBUF partition-remap DMAs;
    stores split by plane across the three DMA queues (512B runs)."""
    if chunks is None:
        chunks = [(a0, ac) for a0 in range(0, A, ac)]
    assert sum(w for _, w in chunks) == A
    NCH = len(chunks)
    O_T = K * A                  # out plane-group stride (t axis)
    O_P = 2 * K * A              # out partition stride (z axis)
    nc = bacc.Bacc("TRN2", target_bir_lowering=False, debug=False,
                   num_devices=N_CORES)
    # x arrives HOST-TRANSPOSED to c-planes [LB, Z, C, A]: the load DMA then
    # fills the on-chip plane layout directly with contiguous A-runs (a
    # DRAM-side [A, C] gather would be 4-byte descriptors), eliminating the
    # per-chunk on-chip transpose entirely.
    x_h = nc.dram_tensor("x", [LB, Z, C, A], F32, kind="ExternalInput")
    o_h = nc.dram_tensor("out", [LB, Z, 2, K, A], BF16, kind="ExternalOutput")
    x_ap, o_ap = x_h.ap(), o_h.ap()
    shm_h = nc.dram_tensor("shm", [P, P], BF16, kind="ExternalInput")
    sel_h = nc.dram_tensor("sel", [P, P], BF16, kind="ExternalInput")
    shd_h = None
    if not xdn_dma:
        shd_h = nc.dram_tensor("shd", [P, P], F32, kind="ExternalInput")
    scratch_aps = [
        nc.dram_tensor(f"scr{r}", [LB, Z, 2, K, A], BF16).ap()
        for r in range(reps - 1)]

    sqdiff = _get_sqdiff()
    from contextlib import ExitStack
    with tile.TileContext(nc) as tc, ExitStack() as es:
        consts = es.enter_context(tc.tile_pool(name="consts", bufs=1))
        psum = es.enter_context(
            tc.tile_pool(name="psum", bufs=psum_bufs, space="PSUM"))
        with tc.tile_pool(name="pool", bufs=bufs) as pool:
            sh_t = consts.tile([P, P], BF16, name="sh_t")
            nc.sync.dma_start(sh_t[:], shm_h.ap()[:])
            sel_t = consts.tile([P, P], BF16, name="sel_t")
            nc.sync.dma_start(sel_t[:], sel_h.ap()[:])
            if xdn_dma:
                # zero source rows for x_dn partitions {0, 64} (z=0 rows)
                zt = consts.tile([P, 3 * (ac + 8)], F32, name="zt")
                nc.gpsimd.memset(zt[:], 0.0)
            else:
                shd_t = consts.tile([P, P], F32, name="shd_t")
                nc.sync.dma_start(shd_t[:], shd_h.ap()[:])
            N = NCH * reps

            def _geom(ci):
                a0, acw = chunks[ci % NCH]
                XW = acw + 8
                lo, hi = max(0, a0 - 4), min(A, a0 + acw + 4)
                wlo = lo - (a0 - 4)       # first valid a in window coords
                whi = hi - (a0 - 4)
                return a0, acw, XW, lo, hi, wlo, whi

            def emit_load(ci):
                # c-plane x window straight from the host-transposed DRAM
                # layout (contiguous A-runs); halo columns zeroed per plane.
                # A second independent load makes xU, so the k5,6 sqdiff's
                # two read streams hit different SBUF tiles (same-tile dual
                # reads halve DVE rate, ~2.9 vs ~1.2 ns/el).
                _, _, XW, lo, hi, wlo, whi = _geom(ci)
                tiles = []
                names = ("xT", "xU") if dup_xt else ("xT",)
                for nm, eng in zip(names, (nc.gpsimd, nc.scalar)):
                    xT = pool.tile([P, C * XW], F32, name=nm,
                                   bufs=(pf + 1))
                    if wlo > 0:
                        nc.gpsimd.memset(
                            _ap(xT, 0, P, 0, [[XW, C], [1, wlo]]), 0.0)
                    if whi < XW:
                        nc.gpsimd.memset(
                            _ap(xT, 0, P, whi, [[XW, C], [1, XW - whi]]), 0.0)
                    eng.dma_start(
                        _ap(xT, 0, P, wlo, [[XW, C], [1, hi - lo]]),
                        bass.AP(tensor=x_ap.tensor, offset=lo,
                                ap=[[X_Z, P], [A, C], [1, hi - lo]]))
                    tiles.append(xT)
                return tiles if dup_xt else (tiles[0], tiles[0])

            def emit_xdn(ci, xT):
                # x_dn[c][a] = x(z-1) planes; rows z=0 (p in {0,64}) zeroed.
                _, _, XW, _, _, _, _ = _geom(ci)
                if xdn_dma:
                    xd = pool.tile([P, C * XW], F32, name="xd",
                                   bufs=(3 if ac < 512 else 2))
                    nc.sync.dma_start(
                        _ap(xd, 0, 2, 0, [[1, C * XW]], pstep=Z),
                        _ap(zt, 0, 2, 0, [[1, C * XW]]))
                    for b in range(LB):
                        nc.sync.dma_start(
                            _ap(xd, b * Z + 1, Z - 1, 0, [[1, C * XW]]),
                            _ap(xT, b * Z, Z - 1, 0, [[1, C * XW]]))
                else:
                    xd = psum.tile([P, C * XW], F32, name="xd_ps", bufs=2)
                    nc.tensor.matmul(
                        _ap(xd, 0, P, 0, [[1, C * XW]]),
                        shd_t[:], _ap(xT, 0, P, 0, [[1, C * XW]]),
                        start=True, stop=True)
                return xd

            PF = pf
            xTs, xds = {}, {}
            for j in range(min(PF, N)):
                xTs[j] = emit_load(j)
            xds[0] = emit_xdn(0, xTs[0][0])

            for ci in range(N):
                a0, acw, XW, lo, hi, wlo, whi = _geom(ci)
                MW = acw + 4

                if ci + PF < N:
                    xTs[ci + PF] = emit_load(ci + PF)
                if ci + 1 < N:
                    xds[ci + 1] = emit_xdn(ci + 1, xTs[ci + 1][0])
                xT, xU = xTs.pop(ci)
                xd = xds.pop(ci)

                # ---- d^2 planes [c][k][a], bf16, fully packed streams ----
                # one dt tile PER CHANNEL so the c-sum adds read two
                # different tiles (same-tile dual reads drop DVE off the
                # 2x mode: 3024 vs 2028 ns measured)
                dtc = [pool.tile([P, 7 * MW], BF16, name=f"dt{c}",
                                 bufs=(1 if ac >= 512 else None))
                       for c in range(C)]
                if fuse_sq:
                    # custom-dve APs: <=2 free dims -> one call per c plane,
                    # k folded in via a stride-0 (center) / stride-1 axis.
                    # k5,6's in1 reads the xU duplicate: both streams from
                    # xT (same tile) halve DVE rate on SBUF port conflicts.
                    for c in range(C):
                        nc.vector._custom_dve(
                            sqdiff,
                            out=_ap(dtc[c], 0, P, 0, [[MW, 5], [1, MW]]),
                            in0=_ap(xT, 0, P, c * XW + 2, [[0, 5], [1, MW]]),
                            in1=_ap(xd, 0, P, c * XW, [[1, 5], [1, MW]]))
                        nc.vector._custom_dve(
                            sqdiff,
                            out=_ap(dtc[c], 0, P, 5 * MW, [[MW, 2], [1, MW]]),
                            in0=_ap(xT, 0, P, c * XW + 2, [[0, 2], [1, MW]]),
                            in1=_ap(xU, 0, P, c * XW, [[1, 2], [1, MW]]))
                else:
                    for k in range(7):
                        src, off = (xd, k) if k < 5 else (xT, k - 5)
                        for c in range(C):
                            nc.vector._custom_dve(
                                sqdiff,
                                out=_ap(dtc[c], 0, P, k * MW, [[1, MW]]),
                                in0=_ap(xT, 0, P, c * XW + 2, [[1, MW]]),
                                in1=_ap(src, 0, P, c * XW + off, [[1, MW]]))

                # ---- M[k][a] = sum_c d^2 (packed bf16 adds, 2x DVE) ----
                dts = pool.tile([P, 7 * MW], BF16, name="dts")
                M = pool.tile([P, 7 * MW], BF16, name="M")
                nc.vector.tensor_add(dts[:], dtc[0][:], dtc[1][:])
                nc.vector.tensor_add(M[:], dts[:], dtc[2][:])

                # ---- x^2 planes (gpsimd); the c-sum for the phantom row
                # rides the SEL matmuls below, so no s-adds at all ----
                seng = nc.gpsimd if pool_s else nc.vector
                sqx = pool.tile([P, C * XW], BF16, name="sqx")
                seng.tensor_mul(sqx[:], xT[:], xU[:])

                # ---- dz-mirror planes in j-layout: J[j][a] = M(z+1-shift)
                # [plane 4-j][a+j] = SH2 @ M + SEL @ (sum_c x^2) (phantom
                # z=64 row: J[j][a] = s(a+4), stride-0 moving j axis; the
                # c-sum is 3 accumulating SEL matmuls over sqx planes).
                # Piece width <= 512 cols and PSUM-bank-aligned outputs.
                def _jmm(mu, off, jn, m_off):
                    nc.tensor.matmul(
                        _ap(mu, 0, P, off, [[1, jn * acw]]),
                        sh_t[:],
                        _ap(M, 0, P, m_off,
                            [[-(MW - 1), jn], [1, acw]] if jn > 1
                            else [[1, acw]]),
                        start=True, stop=False)
                    for c in range(C):
                        nc.tensor.matmul(
                            _ap(mu, 0, P, off, [[1, jn * acw]]),
                            sel_t[:],
                            _ap(sqx, 0, P, c * XW + 4,
                                [[0, jn], [1, acw]] if jn > 1
                                else [[1, acw]]),
                            start=False, stop=(c == C - 1))

                # matmul piece boundaries: multiples of 512 f32 (PSUM bank)
                # that are also j-plane boundaries; acw in {128, 256} works.
                # Two tiles (j=0,1 | j=2..4) so the next pass's J01 matmuls
                # can start as soon as this pass's dz01 exps retire.
                # J01 double-buffered (2 banks x 2 + 3 banks = 7 <= 8): the
                # next pass's J01 matmuls need not wait for this pass's dz01
                J01 = psum.tile([P, 2 * acw], F32, name="J01_ps",
                                bufs=(2 if acw >= 512 else None))
                J234 = psum.tile([P, 3 * acw], F32, name="J234_ps")
                jper = max(1, 512 // acw)
                for j0 in range(0, 2, min(2, jper)):
                    jn = min(jper, 2 - j0)
                    _jmm(J01, j0 * acw, jn, (4 - j0) * MW + j0)
                for j0 in range(2, 5, min(3, jper)):
                    jn = min(jper, 5 - j0)
                    _jmm(J234, (j0 - 2) * acw, jn, (4 - j0) * MW + j0)

                # ---- exps into O planes [(t*14+k)*acw + a], bf16: all
                # operands stream a-innermost (contiguous runs).  ACT is
                # in-order: issue every M-dependent exp (direct + a-mirror,
                # both thetas) BEFORE any J-dependent dz exp, so ACT overlaps
                # the PE matmuls instead of stalling behind them ----
                O = pool.tile([P, acw * 2 * K], BF16, name="O",
                              bufs=(2 if acw >= 256 else None))
                EXP = mybir.ActivationFunctionType.Exp
                for th, sc in ((0, SC0), (1, SC1)):
                    o0 = th * K * acw
                    nc.scalar.activation(
                        _ap(O, 0, P, o0, [[acw, 7], [1, acw]]),
                        _ap(M, 0, P, 2, [[MW, 7], [1, acw]]),
                        EXP, scale=sc)
                    # a-mirrors k'=7,8 <- k=6,5 at a+1+j
                    nc.scalar.activation(
                        _ap(O, 0, P, o0 + 7 * acw, [[acw, 2], [1, acw]]),
                        _ap(M, 0, P, 6 * MW + 3, [[-(MW - 1), 2], [1, acw]]),
                        EXP, scale=sc)
                for th, sc in ((0, SC0), (1, SC1)):
                    o0 = th * K * acw
                    # dz-mirrors k'=9+j read J[j][a]; split by tile so J01
                    # frees early
                    nc.scalar.activation(
                        _ap(O, 0, P, o0 + 9 * acw, [[acw, 2], [1, acw]]),
                        _ap(J01, 0, P, 0, [[acw, 2], [1, acw]]),
                        EXP, scale=sc)
                    nc.scalar.activation(
                        _ap(O, 0, P, o0 + 11 * acw, [[acw, 3], [1, acw]]),
                        _ap(J234, 0, P, 0, [[acw, 3], [1, acw]]),
                        EXP, scale=sc)

                # ---- store: split by (t,k) plane across the three DMA
                # queues; each descriptor run is acw*2 bytes ----
                rep_i = ci // NCH
                dst_ap = o_ap if rep_i == reps - 1 else scratch_aps[rep_i]
                if store_rings == 0:
                    continue
                engs = (nc.sync, nc.scalar, nc.gpsimd)[:store_rings]
                # sync also carries the x_dn remaps -> fewer store planes
                wts = ring_wts or ((8, 12, 8) if store_rings == 3
                                   else (1,) * store_rings)
                tot = sum(wts)
                cum = [sum(wts[:r]) for r in range(store_rings + 1)]
                edges = [c * 2 * K // tot for c in cum]
                for r, eng in enumerate(engs):
                    p0, p1 = edges[r], edges[r + 1]
                    if p1 == p0:
                        continue
                    eng.dma_start(
                        bass.AP(tensor=dst_ap.tensor,
                                offset=p0 * A + a0,
                                ap=[[O_P, P], [A, p1 - p0], [1, acw]]),
                        _ap(O, 0, P, p0 * acw, [[1, (p1 - p0) * acw]]))

    nc.compile()
    return nc


class _Runner:
    """Compile once; reuse the jitted sharded executable across calls.

    Mirrors bass2jax.run_bass_via_pjrt's multi-core path, but without
    donated output buffers (the kernel writes every output element, so the
    zero "output operands" are passed once from device-resident buffers and
    reused)."""

    def __init__(self, nc=None):
        import jax
        from jax.sharding import Mesh, PartitionSpec, NamedSharding
        try:
            from jax.experimental.shard_map import shard_map
        except ImportError:
            from jax.shard_map import shard_map  # newer jax
        from concourse import bass2jax

        bass2jax.install_neuronx_cc_hook()
        if nc is None:
            nc = _build2()
        self.nc = nc

        partition_name = (nc.partition_id_tensor.name
                          if nc.partition_id_tensor else None)
        in_names, out_names, out_avals = [], [], []
        in_dtypes = {}
        for alloc in nc.m.functions[0].allocations:
            if not isinstance(alloc, mybir.MemoryLocationSet):
                continue
            name = alloc.memorylocations[0].name
            if alloc.kind == "ExternalInput":
                if name != partition_name:
                    in_names.append(name)
                    in_dtypes[name] = mybir.dt.np(alloc.dtype)
            elif alloc.kind == "ExternalOutput":
                out_names.append(name)
                out_avals.append(jax.core.ShapedArray(
                    tuple(alloc.tensor_shape), mybir.dt.np(alloc.dtype)))
        self.in_dtypes = in_dtypes
        assert set(in_names) <= {"x", "shm", "sel", "shd"}, in_names
        assert out_names == ["out"], out_names
        all_in_names = in_names + out_names
        if partition_name is not None:
            all_in_names = all_in_names + [partition_name]
        self.in_names = in_names

        def _body(*args):
            operands = list(args)
            if partition_name is not None:
                operands.append(bass2jax.partition_id_tensor())
            return tuple(bass2jax._bass_exec_p.bind(
                *operands,
                out_avals=tuple(out_avals),
                in_names=tuple(all_in_names),
                out_names=tuple(out_names),
                lowering_input_output_aliases=(),
                sim_require_finite=True,
                sim_require_nnan=True,
                nc=nc,
            ))

        devices = jax.devices()[:N_CORES]
        assert len(devices) == N_CORES
        self.mesh = Mesh(np.asarray(devices), ("core",))
        spec = PartitionSpec("core")
        rep = PartitionSpec()
        self.sharding = NamedSharding(self.mesh, spec)
        in_specs = tuple(spec if n == "x" else rep for n in in_names) + (spec,)
        self.jitted = jax.jit(shard_map(
            _body, mesh=self.mesh, in_specs=in_specs, out_specs=(spec,),
            check_rep=False))
        # device-resident constant operands, created once
        oav = out_avals[0]
        full_shape = (N_CORES * oav.shape[0],) + tuple(oav.shape[1:])
        self.out_shape, self.out_dtype = full_shape, oav.dtype
        self.zeros_dev = jax.device_put(
            np.zeros(full_shape, oav.dtype), self.sharding)
        consts = {}
        if "shm" in in_names:
            shm, sel, shd = _host_shift_mats()
            rep_sh = NamedSharding(self.mesh, rep)
            for n, arr in (("shm", shm), ("sel", sel), ("shd", shd)):
                if n in in_names:
                    consts[n] = jax.device_put(
                        arr.astype(in_dtypes[n]), rep_sh)
        self.consts = consts
        self._jax = jax

    def put(self, x: np.ndarray):
        # device expects c-planes [B, Z, C, A] (see _build2)
        xp = np.asarray(x, np.float32).transpose(0, 1, 3, 2)
        return self._jax.device_put(np.ascontiguousarray(xp), self.sharding)

    def run_dev(self, x_dev):
        """Execute; returns device array (not fetched)."""
        args = [x_dev if n == "x" else self.consts[n] for n in self.in_names]
        return self.jitted(*args, self.zeros_dev)[0]

    def __call__(self, x: np.ndarray) -> np.ndarray:
        return _expand_out(np.asarray(self.run_dev(self.put(x))))


_RUNNER = None


def _expand_out(o: np.ndarray) -> np.ndarray:
    """Device output -> reference layout/dtype.  The two classes of each
    theta pair share one exp value (THETA_R pairs are equal), so the device
    stores [.., 2, K, A] (theta/k-plane-major so device writes and stores
    stream contiguous runs); permute to [.., A, K, 2], broadcast to
    [.., A, K, 4], widen bf16 -> f32."""
    if o.shape[-3:] == (2, K, A):
        o32 = np.asarray(o, np.float32).transpose(0, 1, 4, 3, 2)
        o = np.broadcast_to(o32[..., None],
                            o32.shape + (2,)).reshape(o32.shape[:-1] + (4,))
    return np.ascontiguousarray(o.astype(np.float32, copy=False))


def _get_runner():
    global _RUNNER
    if _RUNNER is None:
        _RUNNER = _Runner()
    return _RUNNER


def kernel(x: np.ndarray) -> np.ndarray:
    x = np.asarray(x, dtype=np.float32)
    assert x.shape == (B, Z, A, C), x.shape
    try:
        out = _get_runner()(x)
        # out = exp(..) is finite by construction; a NaN/inf means a
        # transient execution fault -> retry once, then fall back
        if not np.isfinite(out).all():
            out = _get_runner()(x)
        if np.isfinite(out).all():
            return out
        raise RuntimeError("non-finite device output")
    except Exception:
        # fallback: reference-quality but slower dispatch path
        nc = _build2()
        shm, sel, _ = _host_shift_mats()
        cdt = mybir.dt.np(BF16)
        extra = {"shm": shm.astype(cdt), "sel": sel.astype(cdt)}
        xp = x.transpose(0, 1, 3, 2)   # c-planes [B, Z, C, A]
        in_maps = [{"x": np.ascontiguousarray(xp[i * LB:(i + 1) * LB]),
                    **extra}
                   for i in range(N_CORES)]
        res = run_bass_kernel_spmd(nc, in_maps, list(range(N_CORES)))
        return _expand_out(np.concatenate(
            [res.results[i]["out"] for i in range(N_CORES)], axis=0))

